# revision 1
# baseline (speedup 1.0000x reference)
"""Trainium2 Bass kernel for nn_CNN_BiMACL_31860067401819 (retrieval_knn).

Self-contained: hardcodes all shapes/sharding. kernel(**inputs) accepts FULL
inputs keyed as in setup_inputs(), shards queries across 8 NeuronCores
(data-parallel over the query axis), and returns the FULL [2, 320, 5] f32
output. The only collective is a tiny AllReduce of the per-class `rec`
statistics (which couple all queries in the reference).

Per-core query-tuple row order is i = t*40 + q (t-major); per-query means are
recovered with a small constant selection matmul (Sel). Support-tuple
embeddings are written permuted to class-major order (c,sh,t) during the
PSUM->SBUF relu pass.
"""
import os
from itertools import combinations

import numpy as np

import concourse.bass as bass
import concourse.tile as tile
from concourse import bacc, mybir
from concourse.bass_utils import run_bass_kernel_spmd

# ---- static problem config ----
WAY, SHOT, SEQ_LEN, TSS = 5, 16, 10, 2
DIN, DOUT = 2048, 1152
N_QUERIES = 320
T = 45
S = SHOT * T                 # 720
SALL = WAY * S               # 3600
NCORES = 8
NQ = N_QUERIES // NCORES     # 40
R = NQ * T                   # 1800 valid rows/core
RHAT = 1920                  # 48 t-slots * 40 q = 15*128
ITILES = RHAT // 128         # 15
K = TSS * DIN                # 4096
KC = K // 128                # 32
DC = DOUT // 128             # 9
TUPLES = np.array(list(combinations(range(SEQ_LEN), TSS)), dtype=np.int32)
SPAD = 3712                  # padded sup cols (29*128)
PTILES = 29
PROW = 3712                  # p_dram row pitch in bf16 elems (bytes % 256 == 0)
SC = 450                     # D/SS matmul free-dim chunk
NSC = SALL // SC             # 8
QIC = 480                    # q emb row chunk = 12 t-groups of 40
NQIC = RHAT // QIC           # 4
SIC = 480                    # sup emb row chunk = 6 t-groups of 80
SHAT = 48 * 80               # 3840 padded sup rows, (t,u) order
NSIC = SHAT // SIC           # 8

F32 = mybir.dt.float32
BF16 = mybir.dt.bfloat16
U32 = mybir.dt.uint32
I16 = mybir.dt.int16

_CACHE = {}


def _ap(tensor, offset, dims):
    return bass.AP(tensor=tensor, offset=offset, ap=[list(d) for d in dims])


def build(debug=False, sim1=False, stop_after=None):
    """Build the per-core program. sim1: replace the AllReduce with a local
    copy so single-core simulators can run it."""
    nc = bacc.Bacc(num_swdge_queues=4)
    q_d = nc.dram_tensor("qT", [128, 16, NQ * SEQ_LEN], BF16, kind="ExternalInput")
    s_d = nc.dram_tensor("sT", [128, 16, 80 * SEQ_LEN], BF16, kind="ExternalInput")
    w_d = nc.dram_tensor("wT", [KC, 128, DOUT], BF16, kind="ExternalInput")
    b_d = nc.dram_tensor("b", [DOUT], F32, kind="ExternalInput")
    sel_d = nc.dram_tensor("sel", [ITILES, 128, NQ], F32, kind="ExternalInput")
    padv_d = nc.dram_tensor("padv", [128, 1], F32, kind="ExternalInput")
    out_d = nc.dram_tensor("out", [2, NQ, WAY], F32, kind="ExternalOutput")
    dbg = {}
    if debug:
        dbg["D"] = nc.dram_tensor("dbg_D", [RHAT, SALL], F32, kind="ExternalOutput")
        dbg["ave"] = nc.dram_tensor("dbg_ave", [128, ITILES, WAY], F32, kind="ExternalOutput")
        dbg["pos"] = nc.dram_tensor("dbg_pos", [128, ITILES, WAY], F32, kind="ExternalOutput")
        dbg["rec"] = nc.dram_tensor("dbg_rec", [WAY, SALL], F32, kind="ExternalOutput")
        dbg["mask"] = nc.dram_tensor("dbg_mask", [WAY, SALL], F32, kind="ExternalOutput")
        dbg["dmax"] = nc.dram_tensor("dbg_dmax", [128, ITILES, WAY], F32, kind="ExternalOutput")
        dbg["semb"] = nc.dram_tensor("dbg_semb", [128, DC, SPAD], F32, kind="ExternalOutput")

    with tile.TileContext(nc) as tc:
        _body(nc, tc, q_d, s_d, w_d, b_d, sel_d, padv_d, out_d, dbg, sim1, stop_after)
    nc.finalize()
    return nc


def _body(nc, tc, q_d, s_d, w_d, b_d, sel_d, padv_d, out_d, dbg, sim1, stop_after):
    AT = mybir.AluOpType
    ACTF = mybir.ActivationFunctionType
    X = mybir.AxisListType.X

    persist = tc.alloc_tile_pool(name="persist", bufs=1)
    dram = tc.alloc_tile_pool(name="dram", bufs=1, space="DRAM")

    # DRAM scratch (pool tiles so Tile tracks cross-phase deps)
    p_dram = dram.tile([SALL, PROW], BF16, tag="p_scratch")
    qembT_dram = dram.tile([DC, 128, RHAT], BF16, tag="qembT")
    dbf_dram = dram.tile([RHAT, SALL], BF16, tag="dbf")
    posw_dram = dram.tile([WAY, 16, ITILES * 8], I16, tag="posw")
    snorm_dram = dram.tile([1, SALL], F32, tag="snormd")
    mask_dram = dram.tile([WAY, SALL], BF16, tag="maskd")
    msum_dram = dram.tile([WAY, 1], F32, tag="msumd")
    cc_in = dram.tile([WAY, SALL], F32, tag="cc_in")
    cc_out = dram.tile([WAY, SALL], F32, tag="cc_out")

    # persistent SBUF (whole-kernel lifetime -- keep this SMALL)
    s_embT = persist.tile([128, DC, SPAD], BF16, tag="s_embT")
    ones_col = persist.tile([128, 1], BF16, tag="ones_col")
    nc.vector.memset(ones_col[:], 1.0)
    onesf_col = persist.tile([128, 1], F32, tag="onesf_col")
    nc.vector.memset(onesf_col[:], 1.0)
    qnorm = persist.tile([128, ITILES], F32, tag="qnorm")
    pnorm = persist.tile([128, PTILES], F32, tag="pnorm")
    ave_all = persist.tile([128, ITILES, WAY], F32, tag="ave_all")
    dmax_all = persist.tile([128, ITILES, WAY], F32, tag="dmax_all")
    pos16 = persist.tile([128, ITILES, WAY], I16, tag="pos16")
    msum = persist.tile([WAY, 1], F32, tag="msum")
    padv = persist.tile([128, 1], F32, tag="padv")
    nc.sync.dma_start(padv[:], padv_d[:, :])
    rowacc = persist.tile([128, ITILES, WAY], F32, tag="rowacc")
    dmaxq = persist.tile([1, WAY, NQ], F32, tag="dmaxq")
    ctq = persist.tile([1, WAY, NQ], F32, tag="ctq")

    nc.vector.memset(s_embT[:, :, SALL:SPAD], 0.0)

    # ================= Phase 1: embeddings =================
    # Host supplies wT/qT/sT already bf16 + transposed (k on partitions).
    with tc.tile_pool(name="emb", bufs=1) as emb, \
         tc.tile_pool(name="embg", bufs=2) as embg, \
         tc.tile_pool(name="embps", bufs=4, space="PSUM") as embps:
        wT = emb.tile([128, KC, DOUT], BF16, tag="wT")
        nc.sync.dma_start(wT[:], w_d.rearrange("kc p d -> p kc d"))

        # ---- q-side: gather xg from DRAM qT; spill embT to DRAM ----
        for ic in range(NQIC):
            xg = embg.tile([128, KC, QIC], BF16, tag="xg")
            t0 = ic * 12
            if t0 + 12 > T:
                nc.vector.memset(xg[:, :, (T - t0) * 40:], 0.0)
            for tl in range(min(12, T - t0)):
                t = t0 + tl
                for h in range(2):
                    fr = int(TUPLES[t][h])
                    nc.sync.dma_start(
                        xg[:, h * 16:(h + 1) * 16, tl * 40:(tl + 1) * 40],
                        q_d[:, :, fr * 40:(fr + 1) * 40])
            for dc in range(DC):
                ps = embps.tile([128, QIC], F32, tag="emb_ps")
                for kc in range(KC):
                    nc.tensor.matmul(ps[:], wT[:, kc, dc * 128:(dc + 1) * 128],
                                     xg[:, kc], start=(kc == 0), stop=True)
                qe = embg.tile([128, QIC], BF16, tag="qe")
                nc.scalar.activation(qe[:], ps[:], ACTF.Relu)
                nc.sync.dma_start(
                    qembT_dram[dc, :, ic * QIC:(ic + 1) * QIC], qe[:])

        # ---- sup-side: gather xg from SBUF sT; permuted relu writes ----
        for ic in range(NSIC):
            xg = embg.tile([128, KC, SIC], BF16, tag="xg")
            t0 = ic * 6
            if t0 + 6 > T:
                nc.vector.memset(xg[:, :, (T - t0) * 80:], 0.0)
            for tl in range(min(6, T - t0)):
                t = t0 + tl
                for h in range(2):
                    fr = int(TUPLES[t][h])
                    nc.sync.dma_start(
                        xg[:, h * 16:(h + 1) * 16, tl * 80:(tl + 1) * 80],
                        s_d[:, :, fr * 80:(fr + 1) * 80])
            for dc in range(DC):
                ps = embps.tile([128, SIC], F32, tag="emb_ps")
                for kc in range(KC):
                    nc.tensor.matmul(ps[:], wT[:, kc, dc * 128:(dc + 1) * 128],
                                     xg[:, kc], start=(kc == 0), stop=True)
                for tl in range(min(6, T - t0)):
                    t = t0 + tl
                    dstp = s_embT[:, dc, :SALL].rearrange(
                        "p (u t) -> p t u", t=T)[:, t]
                    nc.scalar.activation(dstp, ps[:, tl * 80:(tl + 1) * 80],
                                         ACTF.Relu)

    # ================= Phase 2: reload q_embT, norms, SS, D ================
    ph2 = tc.alloc_tile_pool(name="ph2", bufs=1)
    q_embT = ph2.tile([128, DC, RHAT], BF16, tag="q_embT")
    nc.sync.dma_start(q_embT[:], qembT_dram.rearrange("d p i -> p d i"))

    def _stop(tag):
        if stop_after == tag:
            ph2.release(); persist.release(); dram.release()
            return True
        return False

    if _stop("emb"):
        return

    if dbg:
        with tc.tile_pool(name="dbge", bufs=1) as dbge:
            t2 = dbge.tile([128, DC, SPAD], F32, tag="t2")
            nc.vector.tensor_copy(t2[:], s_embT[:])
            nc.sync.dma_start(dbg["semb"].ap(), t2[:])

    # ---- norms ----
    with tc.tile_pool(name="nrm", bufs=2) as nrm, \
         tc.tile_pool(name="nrmps", bufs=2, space="PSUM") as nrmps:
        snorm_row = nrm.tile([1, SALL], F32, tag="snorm_row")
        for (nt, src_t, dst_t) in ((ITILES, q_embT, qnorm), (PTILES, s_embT, pnorm)):
            for it in range(nt):
                ps = nrmps.tile([128, 1], F32, tag="qn_ps", name="qnps")
                sqa = nrm.tile([128, DC, 128], BF16, tag="qn_sqb")
                for dc in range(DC):
                    nc.scalar.activation(sqa[:, dc],
                                         src_t[:, dc, it * 128:(it + 1) * 128],
                                         ACTF.Square)
                for dc in range(DC):
                    nc.tensor.matmul(ps[:], sqa[:, dc], ones_col[:],
                                     start=(dc == 0), stop=(dc == DC - 1))
                nc.vector.tensor_copy(dst_t[:, it:it + 1], ps[:])
        for scn in range(8):
            ps = nrmps.tile([1, 450], F32, tag="sn_ps")
            for dc in range(DC):
                sq = nrm.tile([128, 450], BF16, tag="sn_sqb")
                nc.scalar.activation(sq[:], s_embT[:, dc, scn * 450:(scn + 1) * 450],
                                     ACTF.Square)
                nc.tensor.matmul(ps[:], ones_col[:], sq[:],
                                 start=(dc == 0), stop=(dc == DC - 1))
            nc.vector.tensor_copy(snorm_row[:, scn * 450:(scn + 1) * 450], ps[:])
        nc.sync.dma_start(snorm_dram[:, :], snorm_row[:])

    if _stop("norms"):
        return

    # ---- SS all-pairs -> p_dram, then D + reductions ----
    with tc.tile_pool(name="snb", bufs=1) as snb, \
         tc.tile_pool(name="ssp", bufs=2) as ssp, \
         tc.tile_pool(name="ssps", bufs=1, space="PSUM") as ssps:
        snorm_bc = snb.tile([128, SALL], F32, tag="snorm_bc")
        nc.sync.dma_start(snorm_bc[:], _ap(snorm_dram.tensor, snorm_dram.offset,
                                           [(0, 128), (1, SALL)]))
        for pt in range(PTILES):
            prow = min(128, SALL - pt * 128)
            ss_sb = ssp.tile([128, SALL], F32, tag="ss_sb")
            pss = [ssps.tile([128, SC], F32, tag=f"ss_ps{sc}", name=f"ssps{sc}") for sc in range(NSC)]
            for dc in range(DC):
                for sc in range(NSC):
                    nc.tensor.matmul(pss[sc][:],
                                     s_embT[:, dc, pt * 128:(pt + 1) * 128],
                                     s_embT[:, dc, sc * SC:(sc + 1) * SC],
                                     start=(dc == 0), stop=(dc == DC - 1))
            for sc in range(NSC):
                dst = ss_sb[:, sc * SC:(sc + 1) * SC]
                nc.vector.scalar_tensor_tensor(
                    dst, pss[sc][:], -2.0, snorm_bc[:, sc * SC:(sc + 1) * SC],
                    op0=AT.mult, op1=AT.add)
                if (sc * SC < (pt + 1) * 128) and ((sc + 1) * SC > pt * 128):
                    nc.vector.tensor_scalar(dst, dst, pnorm[:, pt:pt + 1], 1e-12,
                                            AT.add, op1=AT.max)
                    nc.scalar.activation(dst, dst, ACTF.Sqrt)
                else:
                    nc.scalar.activation(dst, dst, ACTF.Sqrt,
                                         bias=pnorm[:, pt:pt + 1])
            ss_bf = ssp.tile([128, SALL], BF16, tag="ss_bf")
            nc.vector.tensor_copy(ss_bf[:], ss_sb[:])
            nc.sync.dma_start(
                _ap(p_dram.tensor, p_dram.offset + pt * 128 * PROW,
                    [(PROW, prow), (1, SALL)]),
                ss_bf[:prow])

        with tc.tile_pool(name="dp", bufs=2) as dp, \
             tc.tile_pool(name="redt", bufs=4) as redt, \
             tc.tile_pool(name="cdp", bufs=1) as cdp, \
             tc.tile_pool(name="cdg", bufs=2) as cdg:
            SC2 = 360
            for c in range(WAY):
                for it in range(ITILES):
                    d_sb = dp.tile([128, S], F32, tag="d_sb")
                    pss = [ssps.tile([128, SC], F32,
                                     tag=f"ss_ps{(it % 2) * 2 + sc}",
                                     name=f"dps{sc}")
                           for sc in range(2)]
                    for dc in range(DC):
                        for sc in range(2):
                            nc.tensor.matmul(
                                pss[sc][:, :SC2],
                                q_embT[:, dc, it * 128:(it + 1) * 128],
                                s_embT[:, dc, c * S + sc * SC2:c * S + (sc + 1) * SC2],
                                start=(dc == 0), stop=(dc == DC - 1))
                    for sc in range(2):
                        dst = d_sb[:, sc * SC2:(sc + 1) * SC2]
                        nc.vector.scalar_tensor_tensor(
                            dst, pss[sc][:, :SC2], -2.0,
                            snorm_bc[:, c * S + sc * SC2:c * S + (sc + 1) * SC2],
                            op0=AT.mult, op1=AT.add)
                        nc.scalar.activation(dst, dst, ACTF.Sqrt,
                                             bias=qnorm[:, it:it + 1])
                    if dbg:
                        nc.sync.dma_start(
                            dbg["D"][it * 128:(it + 1) * 128, c * S:(c + 1) * S],
                            d_sb[:])
                    d_bf = dp.tile([128, S], BF16, tag="d_bf")
                    nc.vector.tensor_copy(d_bf[:], d_sb[:])
                    nc.sync.dma_start(
                        dbf_dram[it * 128:(it + 1) * 128, c * S:(c + 1) * S],
                        d_bf[:])
                    m16 = redt.tile([128, 16], F32, tag="m16")
                    nc.vector.tensor_reduce(
                        m16[:], d_sb[:].rearrange("p (a b) -> p b a", b=16),
                        X, AT.max)
                    asum = redt.tile([128, 1], F32, tag="asum")
                    nc.vector.tensor_reduce(asum[:], m16[:], X, AT.add)
                    nc.vector.tensor_scalar(ave_all[:, it, c:c + 1], asum[:],
                                            1.0 / 16.0, None, AT.mult)
                    nc.vector.tensor_reduce(dmax_all[:, it, c:c + 1], m16[:],
                                            X, AT.max)
                    mx8 = redt.tile([128, 8], F32, tag="mx8")
                    ix8 = redt.tile([128, 8], U32, tag="ix8")
                    nc.vector.max(mx8[:], d_sb[:])
                    nc.vector.max_index(ix8[:], mx8[:], d_sb[:])
                    posf = redt.tile([128, 1], F32, tag="posf")
                    nc.vector.tensor_scalar(posf[:], ix8[:, 0:1], float(c * S),
                                            None, AT.add)
                    nc.vector.tensor_copy(pos16[:, it, c:c + 1], posf[:])
                    if it == ITILES - 1:
                        nc.vector.tensor_scalar(
                            ave_all[:, it, c:c + 1], ave_all[:, it, c:c + 1],
                            padv[:], None, AT.add)
                # ---- CD gather + rec for class c (overlaps next class's D) --
                nc.sync.dma_start(
                    _ap(posw_dram.tensor, posw_dram.offset + c * 16 * ITILES * 8,
                        [(1, 8), (ITILES * 8, 16), (8, ITILES)]),
                    pos16[:, :, c])
                idxs = cdp.tile([128, ITILES * 8], I16, tag="idxs")
                nc.sync.dma_start(
                    idxs[:],
                    _ap(posw_dram.tensor, posw_dram.offset + c * 16 * ITILES * 8,
                        [(0, 8), (ITILES * 8, 16), (1, ITILES * 8)]))
                acc = cdp.tile([128, SALL], F32, tag="acc01")
                nc.vector.memset(acc[:], 0.0)
                for g in range(ITILES):
                    cd = cdg.tile([128, 1, PROW], BF16, tag="cd")
                    nc.gpsimd.dma_gather(
                        cd[:], p_dram[:, :], idxs[:, g * 8:(g + 1) * 8],
                        128, 128, PROW, queue_num=g % 4)
                    nc.vector.scalar_tensor_tensor(
                        acc[:], cd[:, 0, :SALL], ave_all[:, g, c:c + 1], acc[:],
                        op0=AT.is_gt, op1=AT.add)
                for ch in range(8):
                    ps = ssps.tile([1, 450], F32, tag=f"ss_ps{2 + (ch % 6)}",
                                   name=f"recps{ch}")
                    nc.tensor.matmul(ps[:], onesf_col[:],
                                     acc[:, ch * 450:(ch + 1) * 450],
                                     start=True, stop=True)
                    rc_sb = cdg.tile([1, 450], F32, tag="rec_sb")
                    nc.scalar.copy(rc_sb[:], ps[:])
                    nc.sync.dma_start(cc_in[c:c + 1, ch * 450:(ch + 1) * 450],
                                      rc_sb[:])

    if dbg:
        nc.sync.dma_start(dbg["ave"].ap(), ave_all[:])
        nc.sync.dma_start(dbg["dmax"].ap(), dmax_all[:])
        with tc.tile_pool(name="dbgp", bufs=1) as dbgp:
            pf = dbgp.tile([128, ITILES, WAY], F32, tag="pf")
            nc.vector.tensor_copy(pf[:], pos16[:])
            nc.sync.dma_start(dbg["pos"].ap(), pf[:])

    if _stop("ssd"):
        return

    if _stop("gather"):
        return

    # ================= AllReduce rec =================
    if sim1:
        nc.sync.dma_start(cc_out[:, :], cc_in[:, :])
    else:
        nc.gpsimd.collective_compute(
            "AllReduce", AT.add, replica_groups=[list(range(NCORES))],
            ins=[cc_in[:, :].opt()], outs=[cc_out[:, :].opt()])

    # ================= Phase 3: thr/mask (base-0 partition ops only) =======
    with tc.tile_pool(name="thrp", bufs=2) as thrp, \
         tc.tile_pool(name="thrbig", bufs=1) as thrbig:
        rec_slots = thrbig.tile([WAY, WAY - 1, S], F32, tag="rec_slots")
        for c in range(WAY):
            for k in range(WAY - 1):
                oc = k if k < c else k + 1
                nc.sync.dma_start(rec_slots[c:c + 1, k],
                                  cc_out[c:c + 1, oc * S:(oc + 1) * S])
        if dbg:
            with tc.tile_pool(name="dbgr", bufs=1) as dbgr:
                rg = dbgr.tile([WAY, SALL], F32, tag="rg")
                nc.sync.dma_start(rg[:], cc_out[:, :])
                nc.sync.dma_start(dbg["rec"].ap(), rg[:])
        rsum = thrp.tile([WAY, WAY - 1], F32, tag="rsum")
        nc.vector.tensor_reduce(rsum[:], rec_slots[:], X, AT.add)
        gt0 = thrbig.tile([WAY, WAY - 1, S], F32, tag="gt0")
        nc.vector.tensor_scalar(gt0[:], rec_slots[:], 0.0, None, AT.is_gt)
        nz = thrp.tile([WAY, WAY - 1], F32, tag="nz")
        nc.vector.tensor_reduce(nz[:], gt0[:], X, AT.add)
        nc.vector.tensor_scalar(nz[:], nz[:], 1.0, None, AT.max)
        thr = thrp.tile([WAY, WAY - 1], F32, tag="thr")
        nc.vector.reciprocal(thr[:], nz[:])
        nc.vector.tensor_tensor(thr[:], thr[:], rsum[:], AT.mult)
        mask_slots = thrbig.tile([WAY, WAY - 1, S], F32, tag="mask_slots")
        nc.vector.tensor_tensor(
            mask_slots[:], rec_slots[:],
            thr[:, :, None].to_broadcast((WAY, WAY - 1, S)), AT.is_lt)
        maskf = thrbig.tile([WAY, SALL], F32, tag="maskf")
        nc.vector.memset(maskf[:], 0.0)
        for c in range(WAY):
            for k in range(WAY - 1):
                oc = k if k < c else k + 1
                nc.sync.dma_start(maskf[c:c + 1, oc * S:(oc + 1) * S],
                                  mask_slots[c:c + 1, k])
        nc.vector.tensor_reduce(msum[:], maskf[:], X, AT.add)
        nc.vector.tensor_scalar(msum[:], msum[:], 1.0, None, AT.max)
        # msum -> row layout [1, WAY] for per-class ACT scale in phase 4
        nc.sync.dma_start(msum_dram[:, :], msum[:])
        if dbg:
            nc.sync.dma_start(dbg["mask"].ap(), maskf[:])
        mb = thrbig.tile([WAY, SALL], BF16, tag="mb")
        nc.vector.tensor_copy(mb[:], maskf[:])
        nc.sync.dma_start(mask_dram[:, :], mb[:])

    # ================= Phase 4: contrast row sums + finals =================
    with tc.tile_pool(name="p4", bufs=2) as p4, \
         tc.tile_pool(name="p4m", bufs=1) as p4m, \
         tc.tile_pool(name="finps", bufs=2, space="PSUM") as finps:
        sel_sb = p4m.tile([128, ITILES, NQ], F32, tag="sel_sb")
        nc.sync.dma_start(sel_sb[:], sel_d.rearrange("t p q -> p t q"))
        mask_bc = p4m.tile([128, WAY, SALL], BF16, tag="mask_bc")
        for c in range(WAY):
            nc.sync.dma_start(
                mask_bc[:, c],
                _ap(mask_dram.tensor, mask_dram.offset + c * SALL,
                    [(0, 128), (1, SALL)]))
        scratch = p4m.tile([128, SALL], BF16, tag="scr")
        msum_row = p4m.tile([1, WAY], F32, tag="msum_row")
        nc.sync.dma_start(msum_row[:], _ap(msum_dram.tensor, msum_dram.offset,
                                           [(0, 1), (1, WAY)]))
        sc_row = p4m.tile([1, WAY], F32, tag="sc_row")
        nc.vector.reciprocal(sc_row[:], msum_row[:])
        nc.vector.tensor_scalar(sc_row[:], sc_row[:], 1.0 / 180.0, None, AT.mult)
        for it in range(ITILES):
            dbfl = p4.tile([128, SALL], BF16, tag="dbf_l")
            nc.sync.dma_start(dbfl[:], dbf_dram[it * 128:(it + 1) * 128])
            for c in range(WAY):
                nc.vector.scalar_tensor_tensor(
                    scratch[:], dbfl[:], 1.0, mask_bc[:, c],
                    op0=AT.mult, op1=AT.mult,
                    accum_out=rowacc[:, it, c:c + 1])
        for c in range(WAY):
            ps = finps.tile([1, NQ], F32, tag="dm_ps")
            for it in range(ITILES):
                nc.tensor.matmul(ps[:], dmax_all[:, it, c:c + 1], sel_sb[:, it],
                                 start=(it == 0), stop=(it == ITILES - 1))
            nc.scalar.activation(dmaxq[:, c], ps[:], ACTF.Copy, scale=1.0 / 45.0)
            ps2 = finps.tile([1, NQ], F32, tag="ct_ps")
            for it in range(ITILES):
                nc.tensor.matmul(ps2[:], rowacc[:, it, c:c + 1], sel_sb[:, it],
                                 start=(it == 0), stop=(it == ITILES - 1))
            nc.scalar.mul(ctq[:, c], ps2[:], sc_row[:, c:c + 1])

        for c in range(WAY):
            ssum = p4.tile([1, NQ], F32, tag="ssum")
            nc.vector.tensor_tensor(ssum[:], dmaxq[:, c], ctq[:, c], AT.add)
            rcp = p4.tile([1, NQ], F32, tag="rcp")
            nc.vector.reciprocal(rcp[:], ssum[:])
            lg = p4.tile([1, NQ], F32, tag="lg")
            nc.vector.tensor_tensor(lg[:], dmaxq[:, c], rcp[:], AT.mult)
            nc.sync.dma_start(_ap(out_d, c, [(0, 1), (WAY, NQ)]), dmaxq[:, c])
            nc.sync.dma_start(_ap(out_d, NQ * WAY + c, [(0, 1), (WAY, NQ)]), lg[:])

    ph2.release()
    persist.release()
    dram.release()


# ---------------- host side ----------------

def _sel_host():
    sel = np.zeros((ITILES, 128, NQ), np.float32)
    for i in range(R):
        sel[i // 128, i % 128, i % NQ] = 1.0
    return sel


def _prep_inputs(support_set, queries, support_labels, W, b):
    import ml_dtypes
    bf16 = ml_dtypes.bfloat16
    support_set = np.asarray(support_set, dtype=np.float32)
    queries = np.asarray(queries, dtype=np.float32)
    labels = np.asarray(support_labels).astype(np.int64)
    W = np.asarray(W, dtype=np.float32)
    b = np.asarray(b, dtype=np.float32)
    assert not np.any(b), "kernel built without bias support (reference b==0)"
    order = np.argsort(labels, kind="stable")
    support_sorted = support_set[order]
    # wT [KC, 128, DOUT]: wT[kc, p, d] = W[d, kc*128+p]
    wT = np.ascontiguousarray(
        W.T.astype(bf16).reshape(KC, 128, DOUT))
    # sT [128, 16, f*80+u]: sT[p, kc2, f*80+u] = support_sorted[u, f, kc2*128+p]
    sbf = support_sorted.astype(bf16)           # [80, 10, 2048]
    sT = np.ascontiguousarray(
        sbf.reshape(80, SEQ_LEN, 16, 128).transpose(3, 2, 1, 0)
           .reshape(128, 16, SEQ_LEN * 80))
    qbf_all = queries.astype(bf16)              # [320, 10, 2048]
    sel = _sel_host()
    padv = np.zeros((128, 1), np.float32)
    padv[8:] = 1.0e30
    out = []
    for k in range(NCORES):
        qk = qbf_all[k * NQ:(k + 1) * NQ]       # [40, 10, 2048]
        qT = np.ascontiguousarray(
            qk.reshape(NQ, SEQ_LEN, 16, 128).transpose(3, 2, 1, 0)
              .reshape(128, 16, SEQ_LEN * NQ))
        out.append({
            "qT": qT,
            "sT": sT,
            "wT": wT,
            "b": b,
            "sel": sel,
            "padv": padv,
        })
    return out


def kernel(**inputs):
    per_core = _prep_inputs(**inputs)
    if "nc" not in _CACHE:
        _CACHE["nc"] = build(debug=bool(os.environ.get("BIMACL_DEBUG")))
    nc = _CACHE["nc"]
    res = run_bass_kernel_spmd(nc, per_core, core_ids=list(range(NCORES)))
    _CACHE["last_results"] = res
    full = np.concatenate([res.results[k]["out"] for k in range(NCORES)], axis=1)
    return np.ascontiguousarray(full.astype(np.float32))



# revision 8
# speedup vs baseline: 2.4971x; 2.4971x over previous
"""Trainium2 Bass kernel for nn_CNN_BiMACL_31860067401819 (retrieval_knn).

Self-contained: hardcodes all shapes/sharding. kernel(**inputs) accepts FULL
inputs keyed as in setup_inputs(), shards queries across 8 NeuronCores
(data-parallel over the query axis), and returns the FULL [2, 320, 5] f32
output. The only collective is an AllReduce of the per-class `rec` counts.

Design (v2):
- Frame-factorized embeddings: emb(tuple t=(f1,f2)) = relu(W1^T x_f1 +
  W2^T x_f2); per-frame half-products are computed once with fp8 DoubleRow
  matmuls, tuples assembled with bf16 adds + ACT relu into fp8 embeddings.
- All distance matmuls fp8 + DoubleRow (256-deep contraction per instr).
- SS (support-support) stays in d^2 space: psum = s_i.s_j - sn_j/2 (column
  norm folded in via a 1-row matmul), scaled by -2 on the psum->SBUF copy;
  row norm rides along as an extra gathered column. rec compare is then
  cd_raw + n_i > ave^2  <=>  d^2 > ave^2 (no sqrt for SS at all).
- D (query-support): psum = q.s - sn/2; ACT Sqrt(scale=-2, bias=qnorm)
  emits bf16 distances directly.
- rec compare+accumulate: ACT Sign for SIGN_CLASSES (affine-corrected after
  the AllReduce), fused DVE scalar_tensor_tensor (is_gt,add) for the rest.
- Phase-6 masked row sums via PE transpose of D (stored fp8) + mask matmuls.
"""
import os
from itertools import combinations

import numpy as np

import concourse.bass as bass
import concourse.tile as tile
from concourse import bacc, mybir
from concourse.bass_utils import run_bass_kernel_spmd

# ---- static problem config ----
WAY, SHOT, SEQ_LEN, TSS = 5, 16, 10, 2
DIN, DOUT = 2048, 1152
N_QUERIES = 320
T = 45
S = SHOT * T                 # 720 support tuples per class
SALL = WAY * S               # 3600
NCORES = 8
NQ = N_QUERIES // NCORES     # 40
R = NQ * T                   # 1800 valid rows/core
RHAT = 1920                  # 15*128 padded rows
ITILES = RHAT // 128         # 15
TUPLES = np.array(list(combinations(range(SEQ_LEN), TSS)), dtype=np.int32)
DC = DOUT // 128             # 9
NDR = 4                      # DoubleRow matmuls per 1152 contraction (4*256)
OW = (WAY - 1) * S           # 2880 other-class columns
PROW2 = 2944                 # per-class region row pitch (2880 data + norm + pad)
CTILES = 6                   # 128-row tiles per class region (768 rows)
RROWS = CTILES * 128         # 768
SIGN_CLASSES = (2, 3, 4)     # rec compare on ACT (Sign); others fused on DVE

F32 = mybir.dt.float32
BF16 = mybir.dt.bfloat16
F8 = mybir.dt.float8e4
U32 = mybir.dt.uint32
I16 = mybir.dt.int16
DR = mybir.MatmulPerfMode.DoubleRow

_CACHE = {}


def _ap(tensor, offset, dims):
    return bass.AP(tensor=tensor, offset=offset, ap=[list(d) for d in dims])


def _chunks_for_class(c):
    """480-wide dst chunks over the 2880 other-class cols of class c.
    Returns list of (dst_off, [(src_col, dst_delta, width), ...])."""
    spans = []
    if c > 0:
        spans.append((0, 0, S * c))              # (dst0, src0, len)
    spans.append((S * c, S * (c + 1), OW - S * c))
    out = []
    for dst0 in range(0, OW, 480):
        pieces = []
        for sd, ss, ln in spans:
            lo = max(dst0, sd)
            hi = min(dst0 + 480, sd + ln)
            if lo < hi:
                pieces.append((ss + (lo - sd), lo - dst0, hi - lo))
        out.append((dst0, pieces))
    return out


def build(debug=False, sim1=False):
    nc = bacc.Bacc(num_swdge_queues=4)
    qd_d = nc.dram_tensor("qd", [128, 16, SEQ_LEN * NQ], F8, kind="ExternalInput")
    sd_d = nc.dram_tensor("sd", [128, 16, SEQ_LEN * 80], F8, kind="ExternalInput")
    w_d = nc.dram_tensor("wT", [2, 8, 128, 2, DOUT], F8, kind="ExternalInput")
    id_d = nc.dram_tensor("ident", [128, 128], BF16, kind="ExternalInput")
    sel_d = nc.dram_tensor("sel", [ITILES, 128, NQ], F32, kind="ExternalInput")
    padv_d = nc.dram_tensor("padv", [128, 1], F32, kind="ExternalInput")
    cc_d = nc.dram_tensor("cconst", [WAY, 2], F32, kind="ExternalInput")
    out_d = nc.dram_tensor("out", [2, NQ, WAY], F32, kind="ExternalOutput")
    dbg = {}
    if debug:
        dbg["qemb"] = nc.dram_tensor("dbg_qemb", [128, DC, RHAT], F32, kind="ExternalOutput")
        dbg["semb"] = nc.dram_tensor("dbg_semb", [128, DC, SALL], F32, kind="ExternalOutput")
        dbg["snorm"] = nc.dram_tensor("dbg_snorm", [1, SALL], F32, kind="ExternalOutput")
        dbg["qnorm"] = nc.dram_tensor("dbg_qnorm", [128, ITILES], F32, kind="ExternalOutput")
        dbg["rec"] = nc.dram_tensor("dbg_rec", [WAY, OW], F32, kind="ExternalOutput")
        dbg["mask"] = nc.dram_tensor("dbg_mask", [WAY, WAY - 1, S], F32, kind="ExternalOutput")
        dbg["dmax"] = nc.dram_tensor("dbg_dmax", [128, ITILES, WAY], F32, kind="ExternalOutput")
        dbg["nave2"] = nc.dram_tensor("dbg_nave2", [128, ITILES, WAY], F32, kind="ExternalOutput")
        dbg["pos"] = nc.dram_tensor("dbg_pos", [128, ITILES, WAY], F32, kind="ExternalOutput")
        dbg["ct"] = nc.dram_tensor("dbg_ct", [WAY, RHAT], F32, kind="ExternalOutput")

    with tile.TileContext(nc) as tc:
        _body(nc, tc, qd_d, sd_d, w_d, id_d, sel_d, padv_d, cc_d, out_d, dbg, sim1)
    nc.finalize()
    return nc


def _body(nc, tc, qd_d, sd_d, w_d, id_d, sel_d, padv_d, cc_d, out_d, dbg, sim1):
    AT = mybir.AluOpType
    ACTF = mybir.ActivationFunctionType
    X = mybir.AxisListType.X

    persist = tc.alloc_tile_pool(name="persist", bufs=1)
    dram = tc.alloc_tile_pool(name="dram", bufs=1, space="DRAM")

    # DRAM scratch
    p_dram = dram.tile([WAY, RROWS, PROW2], BF16, tag="p_scratch")
    posw_dram = dram.tile([WAY, 16, ITILES * 8], I16, tag="posw")
    snorm_dram = dram.tile([1, 3840], F32, tag="snormd")
    mask_dram = dram.tile([WAY, WAY - 1, RROWS], BF16, tag="maskd")
    cc_in = dram.tile([WAY, OW], F32, tag="cc_in")
    cc_out = dram.tile([WAY, OW], F32, tag="cc_out")

    # persistent SBUF
    q_embT = persist.tile([128, DC, RHAT], F8, tag="q_embT")
    s_embT = persist.tile([128, DC, SALL], F8, tag="s_embT")
    snh = persist.tile([1, SALL], BF16, tag="snh")          # -snorm/2
    qnorm = persist.tile([128, ITILES], F32, tag="qnorm")
    pnorm = persist.tile([128, WAY * CTILES], F32, tag="pnorm")
    m16a = persist.tile([128, ITILES, 16], F32, tag="m16a")
    dmax_all = persist.tile([128, ITILES, WAY], F32, tag="dmax_all")
    nave2 = persist.tile([128, ITILES, WAY], F32, tag="nave2")
    pos16 = persist.tile([128, ITILES, WAY], I16, tag="pos16")
    ident = persist.tile([128, 128], BF16, tag="ident")
    ones_bf = persist.tile([128, 1], BF16, tag="ones_bf")
    ones_f = persist.tile([128, 1], F32, tag="ones_f")
    ones_row = persist.tile([1, 128], BF16, tag="ones_row")
    padv = persist.tile([128, 1], F32, tag="padv")
    cconst = persist.tile([WAY, 2], F32, tag="cconst")

    nc.vector.memset(ones_bf[:], 1.0)
    nc.vector.memset(ones_f[:], 1.0)
    nc.vector.memset(ones_row[:], 1.0)
    nc.sync.dma_start(padv[:], padv_d[:, :])
    nc.sync.dma_start(ident[:], id_d[:, :])
    nc.sync.dma_start(cconst[:], cc_d[:, :])
    nc.vector.memset(q_embT[:, :, R:RHAT], 0.0)

    # ================= Phase 1: per-frame half products + tuple assembly ====
    with tc.tile_pool(name="emb", bufs=1) as emb, \
         tc.tile_pool(name="embsm", bufs=3) as embsm, \
         tc.tile_pool(name="embps", bufs=4, space="PSUM") as embps:
        wT = emb.tile([128, 2, 8, 2, DOUT], F8, tag="wT")
        nc.sync.dma_start(wT[:], w_d.rearrange("a b p c d -> p a b c d"))
        qd = emb.tile([128, 16, SEQ_LEN * NQ], F8, tag="qd")
        nc.sync.dma_start(qd[:], qd_d[:, :, :])
        sd = emb.tile([128, 16, SEQ_LEN * 80], F8, tag="sd")
        nc.sync.dma_start(sd[:], sd_d[:, :, :])
        Pq = emb.tile([128, DC, 2, SEQ_LEN * NQ], BF16, tag="Pq")
        Ps = emb.tile([128, DC, 2, SEQ_LEN * 80], BF16, tag="Ps")

        for half in range(2):
            for dc in range(DC):
                ps = embps.tile([128, SEQ_LEN * NQ], F32, tag="emb_ps")
                for kc2 in range(8):
                    nc.tensor.matmul(
                        ps[:], wT[:, half, kc2, :, dc * 128:(dc + 1) * 128],
                        qd[:, 2 * kc2:2 * kc2 + 2, :],
                        start=(kc2 == 0), stop=(kc2 == 7), perf_mode=DR)
                nc.scalar.activation(Pq[:, dc, half], ps[:], ACTF.Copy)
                for ch in range(2):
                    ps2 = embps.tile([128, SEQ_LEN * NQ], F32, tag="emb_ps")
                    for kc2 in range(8):
                        nc.tensor.matmul(
                            ps2[:], wT[:, half, kc2, :, dc * 128:(dc + 1) * 128],
                            sd[:, 2 * kc2:2 * kc2 + 2, ch * 400:(ch + 1) * 400],
                            start=(kc2 == 0), stop=(kc2 == 7), perf_mode=DR)
                    nc.scalar.activation(Ps[:, dc, half, ch * 400:(ch + 1) * 400],
                                         ps2[:], ACTF.Copy)

        # tuple assembly: q side (cols t*40+q), s side (cols u*45+t)
        for t in range(T):
            f1, f2 = int(TUPLES[t][0]), int(TUPLES[t][1])
            preq = embsm.tile([128, DC, NQ], BF16, tag="preq")
            nc.vector.tensor_tensor(
                preq[:], Pq[:, :, 0, f1 * NQ:(f1 + 1) * NQ],
                Pq[:, :, 1, f2 * NQ:(f2 + 1) * NQ], AT.add)
            nc.scalar.activation(q_embT[:, :, t * NQ:(t + 1) * NQ], preq[:],
                                 ACTF.Relu, scale=1.0 / 64.0)
            pres = embsm.tile([128, DC, 80], BF16, tag="pres")
            nc.vector.tensor_tensor(
                pres[:], Ps[:, :, 0, f1 * 80:(f1 + 1) * 80],
                Ps[:, :, 1, f2 * 80:(f2 + 1) * 80], AT.add)
            dst = s_embT.rearrange("p d (u t) -> p d t u", t=T)[:, :, t]
            nc.scalar.activation(dst, pres[:], ACTF.Relu, scale=1.0 / 64.0)

    # dT allocated after the emb pool frees wT/Pq/Ps space
    dtp = tc.alloc_tile_pool(name="dtp", bufs=1)
    dT = [dtp.tile([128, CTILES, RHAT], F8, tag=f"dT{c}", name=f"dT{c}")
          for c in range(WAY)]

    if dbg:
        with tc.tile_pool(name="dbge", bufs=1) as dbge:
            t1 = dbge.tile([128, DC, RHAT], F32, tag="dbq")
            nc.vector.tensor_copy(t1[:], q_embT[:])
            nc.sync.dma_start(dbg["qemb"].ap(), t1[:])
            t2 = dbge.tile([128, DC, SALL], F32, tag="dbs")
            nc.vector.tensor_copy(t2[:], s_embT[:])
            nc.sync.dma_start(dbg["semb"].ap(), t2[:])

    # ================= Phase 2: norms =================
    with tc.tile_pool(name="nrm", bufs=2) as nrm, \
         tc.tile_pool(name="nrmps", bufs=2, space="PSUM") as nrmps:
        for it in range(ITILES):
            ps = nrmps.tile([128, 1], F32, tag="qn_ps", name="qnps")
            sqa = nrm.tile([128, DC, 128], BF16, tag="qn_sqb")
            for dc in range(DC):
                nc.scalar.activation(sqa[:, dc],
                                     q_embT[:, dc, it * 128:(it + 1) * 128],
                                     ACTF.Square)
            for dc in range(DC):
                nc.tensor.matmul(ps[:], sqa[:, dc], ones_bf[:],
                                 start=(dc == 0), stop=(dc == DC - 1))
            nc.vector.tensor_copy(qnorm[:, it:it + 1], ps[:])
        snrow = nrm.tile([1, SALL], F32, tag="snrow")
        for scn in range(8):
            ps = nrmps.tile([1, 450], F32, tag="sn_ps")
            for dc in range(DC):
                sq = nrm.tile([128, 450], BF16, tag="sn_sqb")
                nc.scalar.activation(sq[:], s_embT[:, dc, scn * 450:(scn + 1) * 450],
                                     ACTF.Square)
                nc.tensor.matmul(ps[:], ones_bf[:], sq[:],
                                 start=(dc == 0), stop=(dc == DC - 1))
            nc.scalar.activation(snrow[:, scn * 450:(scn + 1) * 450], ps[:], ACTF.Copy)
        nc.vector.tensor_scalar(snh[:], snrow[:], -0.5, None, AT.mult)
        nc.sync.dma_start(snorm_dram[:, :SALL], snrow[:])
        # pnorm[p, c*6+j] = snorm[720c + 128j + p]
        for c in range(WAY):
            nc.sync.dma_start(
                pnorm[:, c * CTILES:(c + 1) * CTILES],
                _ap(snorm_dram.tensor, snorm_dram.offset + c * S,
                    [(1, 128), (128, CTILES)]))
        if dbg:
            nc.sync.dma_start(dbg["snorm"].ap(), snrow[:])
            nc.sync.dma_start(dbg["qnorm"].ap(), qnorm[:])

    # ================= Phase 3: SS d^2 slabs (class-major regions) ==========
    with tc.tile_pool(name="ssst", bufs=2) as ssst, \
         tc.tile_pool(name="ssps", bufs=2, space="PSUM") as ssps:
        for c in range(WAY):
            staging = ssst.tile([128, CTILES, PROW2], BF16, tag="ss_stage")
            chunks = _chunks_for_class(c)
            for j in range(CTILES):
                p0 = S * c + 128 * j
                pw = min(128, S - 128 * j)
                for (dst0, pieces) in chunks:
                    ps = ssps.tile([128, 480], F32, tag="ss_ps")
                    for (src0, doff, w) in pieces:
                        for dc2 in range(NDR):
                            nc.tensor.matmul(
                                ps[:pw, doff:doff + w],
                                s_embT[:, 2 * dc2:2 * dc2 + 2, p0:p0 + pw],
                                s_embT[:, 2 * dc2:2 * dc2 + 2, src0:src0 + w],
                                start=(dc2 == 0), stop=False, perf_mode=DR)
                        nc.tensor.matmul(
                            ps[:pw, doff:doff + w], s_embT[:, 8, p0:p0 + pw],
                            s_embT[:, 8, src0:src0 + w], start=False, stop=False)
                        nc.tensor.matmul(
                            ps[:pw, doff:doff + w], ones_row[:, :pw],
                            snh[:, src0:src0 + w], start=False, stop=True)
                    nc.scalar.activation(staging[:pw, j, dst0:dst0 + 480],
                                         ps[:pw], ACTF.Copy, scale=-2.0)
            # row-norm column (col 2880) for the gathered threshold
            nc.vector.tensor_copy(staging[:, :, OW:OW + 1],
                                  pnorm[:, c * CTILES:(c + 1) * CTILES, None])
            # write region, skipping the undefined pad rows of the last tile
            nc.sync.dma_start(
                _ap(p_dram.tensor, p_dram.offset + c * RROWS * PROW2,
                    [(PROW2, 128), (128 * PROW2, CTILES - 1), (1, PROW2)]),
                staging[:, :CTILES - 1])
            nc.sync.dma_start(
                _ap(p_dram.tensor,
                    p_dram.offset + (c * RROWS + (CTILES - 1) * 128) * PROW2,
                    [(PROW2, 80), (1, PROW2)]),
                staging[:80, CTILES - 1])

    # ================= Phase 4: D + reductions + gather/rec per class =======
    with tc.tile_pool(name="dph", bufs=2) as dph, \
         tc.tile_pool(name="dsm", bufs=4) as dsm, \
         tc.tile_pool(name="dps", bufs=2, space="PSUM") as dps, \
         tc.tile_pool(name="tps", bufs=2, space="PSUM") as tps, \
         tc.tile_pool(name="rps", bufs=1, space="PSUM") as rps, \
         tc.tile_pool(name="cdp", bufs=2) as cdp, \
         tc.tile_pool(name="cdg", bufs=2) as cdg:
        for c in range(WAY):
            # ---- D tiles ----
            for it in range(ITILES):
                d_bf = dph.tile([128, RROWS], BF16, tag="d_bf")
                for sc in range(2):
                    ps = dps.tile([128, 360], F32, tag=f"d_ps{sc}", name=f"dps{sc}")
                    s0 = c * S + sc * 360
                    for dc2 in range(NDR):
                        nc.tensor.matmul(
                            ps[:], q_embT[:, 2 * dc2:2 * dc2 + 2, it * 128:(it + 1) * 128],
                            s_embT[:, 2 * dc2:2 * dc2 + 2, s0:s0 + 360],
                            start=(dc2 == 0), stop=False, perf_mode=DR)
                    nc.tensor.matmul(ps[:], q_embT[:, 8, it * 128:(it + 1) * 128],
                                     s_embT[:, 8, s0:s0 + 360], start=False, stop=False)
                    nc.tensor.matmul(ps[:], ones_row[:], snh[:, s0:s0 + 360],
                                     start=False, stop=True)
                    nc.scalar.activation(d_bf[:, sc * 360:(sc + 1) * 360], ps[:],
                                         ACTF.Sqrt, bias=qnorm[:, it:it + 1],
                                         scale=-2.0)
                nc.vector.memset(d_bf[:, S:RROWS], 0.0)
                # reductions
                nc.vector.tensor_reduce(
                    m16a[:, it], d_bf[:, :S].rearrange("p (a b) -> p b a", b=16),
                    X, AT.max)
                mx8 = dsm.tile([128, 8], F32, tag="mx8")
                ix8 = dsm.tile([128, 8], U32, tag="ix8")
                nc.vector.max(mx8[:], d_bf[:, :S])
                nc.vector.max_index(ix8[:], mx8[:], d_bf[:, :S])
                posf = dsm.tile([128, 1], F32, tag="posf")
                nc.vector.tensor_scalar(posf[:], ix8[:, 0:1], 0.0, None, AT.add)
                nc.vector.tensor_copy(pos16[:, it, c:c + 1], posf[:])
                # transpose into dT (fp8)
                psT = tps.tile([128, CTILES * 128], BF16, tag="psT")
                for j in range(CTILES):
                    nc.tensor.matmul(psT[:, j * 128:(j + 1) * 128],
                                     d_bf[:, j * 128:(j + 1) * 128], ident[:],
                                     start=True, stop=True, is_transpose=True)
                nc.vector.tensor_copy(
                    dT[c][:, :, it * 128:(it + 1) * 128],
                    psT[:].rearrange("p (j q) -> p j q", j=CTILES))
            nc.vector.memset(dT[c][:, :, R:RHAT], 0.0)
            # batched per-class stats: nave2 = -(asum/16)^2 ; dmax
            asum = dsm.tile([128, ITILES], F32, tag="asum")
            nc.vector.tensor_reduce(asum[:], m16a[:], X, AT.add)
            nc.vector.tensor_scalar(asum[:, ITILES - 1:ITILES],
                                    asum[:, ITILES - 1:ITILES], padv[:], None, AT.add)
            nc.vector.tensor_tensor(asum[:], asum[:], asum[:], AT.mult)
            nc.vector.tensor_scalar(nave2[:, :, c], asum[:], -1.0 / 256.0, None,
                                    AT.mult)
            nc.vector.tensor_reduce(dmax_all[:, :, c], m16a[:], X, AT.max)

            # ---- gather + rec ----
            nc.sync.dma_start(
                _ap(posw_dram.tensor, posw_dram.offset + c * 16 * ITILES * 8,
                    [(1, 8), (ITILES * 8, 16), (8, ITILES)]),
                pos16[:, :, c])
            idxs = cdp.tile([128, ITILES * 8], I16, tag="idxs")
            nc.sync.dma_start(
                idxs[:],
                _ap(posw_dram.tensor, posw_dram.offset + c * 16 * ITILES * 8,
                    [(0, 8), (ITILES * 8, 16), (1, ITILES * 8)]))
            use_sign = c in SIGN_CLASSES
            if use_sign:
                acc = cdp.tile([128, OW], BF16, tag="accb")
            else:
                acc = cdp.tile([128, OW], F32, tag="accf")
            region = _ap(p_dram.tensor, p_dram.offset + c * RROWS * PROW2,
                         [(PROW2, RROWS), (1, PROW2)])
            for g in range(ITILES):
                cd = cdg.tile([128, 1, PROW2], BF16, tag="cd")
                nc.gpsimd.dma_gather(
                    cd[:], region, idxs[:, g * 8:(g + 1) * 8],
                    128, 128, PROW2, queue_num=g % 4)
                # bias = n_i - ave^2 : sign(cd_raw + bias) = sign(d^2 - ave^2)
                nbias = cdg.tile([128, 1], F32, tag="nbias")
                nc.vector.tensor_tensor(nbias[:], cd[:, 0, OW:OW + 1],
                                        nave2[:, g, c:c + 1], AT.add)
                if use_sign:
                    cmp = cdg.tile([128, OW], BF16, tag="cmp")
                    nc.scalar.activation(cmp[:], cd[:, 0, :OW], ACTF.Sign,
                                         bias=nbias[:])
                    if g == 0:
                        nc.vector.tensor_copy(acc[:], cmp[:])
                    else:
                        nc.vector.tensor_tensor(acc[:], acc[:], cmp[:], AT.add)
                else:
                    # threshold: cd_raw > -(bias)
                    nc.vector.tensor_scalar(nbias[:], nbias[:], -1.0, None, AT.mult)
                    if g == 0:
                        nc.vector.memset(acc[:], 0.0)
                    nc.vector.scalar_tensor_tensor(
                        acc[:], cd[:, 0, :OW], nbias[:], acc[:],
                        op0=AT.is_gt, op1=AT.add)
            recrow = cdp.tile([1, OW], F32, tag="recrow")
            for k in range(6):
                ps = rps.tile([1, 480], F32, tag="rec_ps")
                if use_sign:
                    nc.tensor.matmul(ps[:], ones_bf[:], acc[:, k * 480:(k + 1) * 480],
                                     start=True, stop=True)
                else:
                    nc.tensor.matmul(ps[:], ones_f[:], acc[:, k * 480:(k + 1) * 480],
                                     start=True, stop=True)
                nc.scalar.activation(recrow[:, k * 480:(k + 1) * 480], ps[:],
                                     ACTF.Copy)
            nc.sync.dma_start(cc_in[c:c + 1, :], recrow[:])

    # ================= AllReduce rec =================
    if sim1:
        nc.sync.dma_start(cc_out[:, :], cc_in[:, :])
    else:
        nc.gpsimd.collective_compute(
            "AllReduce", mybir.AluOpType.add,
            replica_groups=[list(range(NCORES))],
            ins=[cc_in[:, :].opt()], outs=[cc_out[:, :].opt()])

    # ================= Phase 5: thr/mask =================
    with tc.tile_pool(name="thrp", bufs=2) as thrp, \
         tc.tile_pool(name="thrbig", bufs=1) as thrbig:
        rec = thrbig.tile([WAY, WAY - 1, S], F32, tag="rec")
        nc.sync.dma_start(rec[:], cc_out[:, :].rearrange("c (k s) -> c k s", k=WAY - 1))
        # per-class affine: sign classes hold sum(+-1); counts = (x+15360)/2
        nc.vector.tensor_scalar(rec[:], rec[:], cconst[:, 0:1], None, AT.add)
        nc.vector.tensor_scalar(rec[:], rec[:], cconst[:, 1:2], None, AT.mult)
        if dbg:
            with tc.tile_pool(name="dbgr", bufs=1) as dbgr:
                rg = dbgr.tile([WAY, OW], F32, tag="rg")
                nc.vector.tensor_copy(rg[:], rec[:].rearrange("c k s -> c (k s)"))
                nc.sync.dma_start(dbg["rec"].ap(), rg[:])
        rsum = thrp.tile([WAY, WAY - 1], F32, tag="rsum")
        nc.vector.tensor_reduce(rsum[:], rec[:], X, AT.add)
        gt0 = thrbig.tile([WAY, WAY - 1, S], F32, tag="gt0")
        nc.vector.tensor_scalar(gt0[:], rec[:], 0.5, None, AT.is_gt)
        nz = thrp.tile([WAY, WAY - 1], F32, tag="nz")
        nc.vector.tensor_reduce(nz[:], gt0[:], X, AT.add)
        nc.vector.tensor_scalar(nz[:], nz[:], 1.0, None, AT.max)
        thr = thrp.tile([WAY, WAY - 1], F32, tag="thr")
        nc.vector.reciprocal(thr[:], nz[:])
        nc.vector.tensor_tensor(thr[:], thr[:], rsum[:], AT.mult)
        mask_slots = thrbig.tile([WAY, WAY - 1, S], BF16, tag="mask_slots")
        nc.vector.tensor_tensor(
            mask_slots[:], rec[:],
            thr[:, :, None].to_broadcast((WAY, WAY - 1, S)), AT.is_lt)
        if dbg:
            with tc.tile_pool(name="dbgm", bufs=1) as dbgm:
                mg = dbgm.tile([WAY, WAY - 1, S], F32, tag="mg")
                nc.vector.tensor_copy(mg[:], mask_slots[:])
                nc.sync.dma_start(dbg["mask"].ap(), mg[:])
        msum = thrp.tile([WAY, 1], F32, tag="msum")
        nc.vector.tensor_reduce(msum[:], mask_slots[:].rearrange("c k s -> c (k s)"),
                                X, AT.add)
        nc.vector.tensor_scalar(msum[:], msum[:], 1.0, None, AT.max)
        scv = thrp.tile([WAY, 1], F32, tag="scv")
        nc.vector.reciprocal(scv[:], msum[:])
        nc.vector.tensor_scalar(scv[:], scv[:], 1.0 / (4.0 * T), None, AT.mult)
        nc.sync.dma_start(mask_dram[:, :, :S], mask_slots[:])

        # ============= Phase 6: contrast sums + finals =============
        with tc.tile_pool(name="p6", bufs=1) as p6, \
             tc.tile_pool(name="p6ps", bufs=1, space="PSUM") as p6ps:
            maskT = p6.tile([128, WAY * CTILES, WAY], BF16, tag="maskT")
            nc.vector.memset(maskT[:], 0.0)
            for c in range(WAY):
                for cr in range(WAY):
                    if cr == c:
                        continue
                    k = cr if cr < c else cr - 1
                    nc.sync.dma_start(
                        maskT[:, cr * CTILES:(cr + 1) * CTILES, c],
                        _ap(mask_dram.tensor,
                            mask_dram.offset + (c * (WAY - 1) + k) * RROWS,
                            [(1, 128), (128, CTILES)]))
            psC = [p6ps.tile([WAY, 480], F32, tag=f"ct_ps{qc}", name=f"ctps{qc}")
                   for qc in range(4)]
            for cr in range(WAY):
                for j in range(CTILES):
                    for qc in range(4):
                        nc.tensor.matmul(
                            psC[qc][:], maskT[:, cr * CTILES + j, :],
                            dT[cr][:, j, qc * 480:(qc + 1) * 480],
                            start=(cr == 0 and j == 0),
                            stop=(cr == WAY - 1 and j == CTILES - 1))
            ctrows = p6.tile([WAY, RHAT], F32, tag="ctrows")
            for qc in range(4):
                nc.scalar.activation(ctrows[:, qc * 480:(qc + 1) * 480], psC[qc][:],
                                     ACTF.Copy)
            if dbg:
                nc.sync.dma_start(dbg["ct"].ap(), ctrows[:])
            ctq = p6.tile([WAY, NQ], F32, tag="ctq")
            nc.vector.tensor_reduce(
                ctq[:], ctrows[:].rearrange("c (s q) -> c q s", q=NQ), X, AT.add)
            nc.vector.tensor_scalar(ctq[:], ctq[:], scv[:], None, AT.mult)

            sel_sb = p6.tile([128, ITILES, NQ], F32, tag="sel_sb")
            nc.sync.dma_start(sel_sb[:], sel_d.rearrange("t p q -> p t q"))
            dmq = p6.tile([WAY, NQ], F32, tag="dmq")
            psD = p6ps.tile([WAY, NQ], F32, tag="dm_ps")
            for it in range(ITILES):
                nc.tensor.matmul(psD[:], dmax_all[:, it, :], sel_sb[:, it],
                                 start=(it == 0), stop=(it == ITILES - 1))
            nc.scalar.activation(dmq[:], psD[:], ACTF.Copy, scale=1.0 / T)
            if dbg:
                nc.sync.dma_start(dbg["dmax"].ap(), dmax_all[:])
                nc.sync.dma_start(dbg["nave2"].ap(), nave2[:])
                with tc.tile_pool(name="dbgp", bufs=1) as dbgp:
                    pf = dbgp.tile([128, ITILES, WAY], F32, tag="pf")
                    nc.vector.tensor_copy(pf[:], pos16[:])
                    nc.sync.dma_start(dbg["pos"].ap(), pf[:])

            ssum = p6.tile([WAY, NQ], F32, tag="ssum")
            nc.vector.tensor_tensor(ssum[:], dmq[:], ctq[:], AT.add)
            rcp = p6.tile([WAY, NQ], F32, tag="rcp")
            nc.vector.reciprocal(rcp[:], ssum[:])
            lg = p6.tile([WAY, NQ], F32, tag="lg")
            nc.vector.tensor_tensor(lg[:], dmq[:], rcp[:], AT.mult)
            nc.sync.dma_start(_ap(out_d, 0, [(1, WAY), (WAY, NQ)]), dmq[:])
            nc.sync.dma_start(_ap(out_d, NQ * WAY, [(1, WAY), (WAY, NQ)]), lg[:])

    dtp.release()
    persist.release()
    dram.release()


# ---------------- host side ----------------

def _sel_host():
    sel = np.zeros((ITILES, 128, NQ), np.float32)
    for i in range(R):
        sel[i // 128, i % 128, i % NQ] = 1.0
    return sel


def _prep_inputs(support_set, queries, support_labels, W, b):
    import ml_dtypes
    f8 = ml_dtypes.float8_e4m3fn
    support_set = np.asarray(support_set, dtype=np.float32)
    queries = np.asarray(queries, dtype=np.float32)
    labels = np.asarray(support_labels).astype(np.int64)
    W = np.asarray(W, dtype=np.float32)
    b = np.asarray(b, dtype=np.float32)
    assert not np.any(b), "kernel built without bias support (reference b==0)"
    order = np.argsort(labels, kind="stable")
    support_sorted = support_set[order]

    # wT [2, 8, 128, 2, 1152]: wT[half, kc2, p, h2, d] =
    #   64*W[d, half*2048 + kc2*256 + h2*128 + p]
    w8 = (W * 64.0).astype(f8)                     # [1152, 4096]
    wT = np.ascontiguousarray(
        w8.reshape(DOUT, 2, 8, 2, 128).transpose(1, 2, 4, 3, 0))

    s8 = support_sorted.astype(f8)                 # [80, 10, 2048]
    sd = np.ascontiguousarray(
        s8.reshape(80, SEQ_LEN, 16, 128).transpose(3, 2, 1, 0)
          .reshape(128, 16, SEQ_LEN * 80))
    q8 = queries.astype(f8)                        # [320, 10, 2048]
    sel = _sel_host()
    padv = np.zeros((128, 1), np.float32)
    padv[8:] = 1.0e15
    ident = np.eye(128).astype(ml_dtypes.bfloat16)
    cconst = np.zeros((WAY, 2), np.float32)
    for c in range(WAY):
        if c in SIGN_CLASSES:
            cconst[c] = (NCORES * RHAT, 0.5)
        else:
            cconst[c] = (0.0, 1.0)
    out = []
    for k in range(NCORES):
        qk = q8[k * NQ:(k + 1) * NQ]               # [40, 10, 2048]
        qd = np.ascontiguousarray(
            qk.reshape(NQ, SEQ_LEN, 16, 128).transpose(3, 2, 1, 0)
              .reshape(128, 16, SEQ_LEN * NQ))
        out.append({
            "qd": qd,
            "sd": sd,
            "wT": wT,
            "ident": ident,
            "sel": sel,
            "padv": padv,
            "cconst": cconst,
        })
    return out


def kernel(**inputs):
    per_core = _prep_inputs(**inputs)
    if "nc" not in _CACHE:
        _CACHE["nc"] = build(debug=bool(os.environ.get("BIMACL_DEBUG")))
    nc = _CACHE["nc"]
    res = run_bass_kernel_spmd(nc, per_core, core_ids=list(range(NCORES)))
    _CACHE["last_results"] = res
    full = np.concatenate([res.results[k]["out"] for k in range(NCORES)], axis=1)
    return np.ascontiguousarray(full.astype(np.float32))


# revision 15
# speedup vs baseline: 2.7455x; 1.0995x over previous
"""Trainium2 Bass kernel for nn_CNN_BiMACL_31860067401819 (retrieval_knn).

Self-contained: hardcodes all shapes/sharding. kernel(**inputs) accepts FULL
inputs keyed as in setup_inputs(), shards queries across 8 NeuronCores
(data-parallel over the query axis), and returns the FULL [2, 320, 5] f32
output. The only collective is an AllReduce of the per-class `rec` counts.

Design (v2):
- Frame-factorized embeddings: emb(tuple t=(f1,f2)) = relu(W1^T x_f1 +
  W2^T x_f2); per-frame half-products are computed once with fp8 DoubleRow
  matmuls, tuples assembled with bf16 adds + ACT relu into fp8 embeddings.
- All distance matmuls fp8 + DoubleRow (256-deep contraction per instr).
- SS (support-support) stays in d^2 space: psum = s_i.s_j - sn_j/2 (column
  norm folded in via a 1-row matmul), scaled by -2 on the psum->SBUF copy;
  row norm rides along as an extra gathered column. rec compare is then
  cd_raw + n_i > ave^2  <=>  d^2 > ave^2 (no sqrt for SS at all).
- D (query-support): psum = q.s - sn/2; ACT Sqrt(scale=-2, bias=qnorm)
  emits bf16 distances directly.
- rec compare+accumulate: ACT Sign for SIGN_CLASSES (affine-corrected after
  the AllReduce), fused DVE scalar_tensor_tensor (is_gt,add) for the rest.
- Phase-6 masked row sums via PE transpose of D (stored fp8) + mask matmuls.
"""
import os
from itertools import combinations

import numpy as np

import concourse.bass as bass
import concourse.tile as tile
from concourse import bacc, mybir
from concourse.bass_utils import run_bass_kernel_spmd

# ---- static problem config ----
WAY, SHOT, SEQ_LEN, TSS = 5, 16, 10, 2
DIN, DOUT = 2048, 1152
N_QUERIES = 320
T = 45
S = SHOT * T                 # 720 support tuples per class
SALL = WAY * S               # 3600
NCORES = 8
NQ = N_QUERIES // NCORES     # 40
R = NQ * T                   # 1800 valid rows/core
RHAT = 1920                  # 15*128 padded rows
ITILES = RHAT // 128         # 15
TUPLES = np.array(list(combinations(range(SEQ_LEN), TSS)), dtype=np.int32)
DC = DOUT // 128             # 9
NDR = 5                      # DoubleRow matmuls per padded 1280 contraction (5*256)
OW = (WAY - 1) * S           # 2880 other-class columns
PROW2 = 2944                 # per-class region row pitch (2880 data + norm + pad)
CTILES = 6                   # 128-row tiles per class region (768 rows)
RROWS = CTILES * 128         # 768
SIGN_CLASSES = (0, 1, 2, 3, 4)  # rec compare on ACT (Sign)

F32 = mybir.dt.float32
BF16 = mybir.dt.bfloat16
F8 = mybir.dt.float8e4
U32 = mybir.dt.uint32
I16 = mybir.dt.int16
DR = mybir.MatmulPerfMode.DoubleRow

_CACHE = {}


def _ap(tensor, offset, dims):
    return bass.AP(tensor=tensor, offset=offset, ap=[list(d) for d in dims])


def _chunks_for_class(c):
    """480-wide dst chunks over the 2880 other-class cols of class c.
    Returns list of (dst_off, [(src_col, dst_delta, width), ...])."""
    spans = []
    if c > 0:
        spans.append((0, 0, S * c))              # (dst0, src0, len)
    spans.append((S * c, S * (c + 1), OW - S * c))
    out = []
    for dst0 in range(0, OW, 480):
        pieces = []
        for sd, ss, ln in spans:
            lo = max(dst0, sd)
            hi = min(dst0 + 480, sd + ln)
            if lo < hi:
                pieces.append((ss + (lo - sd), lo - dst0, hi - lo))
        out.append((dst0, pieces))
    return out


def build(debug=False, sim1=False):
    nc = bacc.Bacc(num_swdge_queues=4)
    qd_d = nc.dram_tensor("qd", [128, 16, SEQ_LEN * NQ], F8, kind="ExternalInput")
    sd_d = nc.dram_tensor("sd", [128, 16, SEQ_LEN * 80], F8, kind="ExternalInput")
    w_d = nc.dram_tensor("wT", [2, 8, 128, 2, DOUT], F8, kind="ExternalInput")
    id_d = nc.dram_tensor("ident", [128, 128], BF16, kind="ExternalInput")
    sel_d = nc.dram_tensor("sel", [ITILES, 128, NQ], F32, kind="ExternalInput")
    padv_d = nc.dram_tensor("padv", [128, 1], F32, kind="ExternalInput")
    cc_d = nc.dram_tensor("cconst", [WAY, 2], F32, kind="ExternalInput")
    out_d = nc.dram_tensor("out", [2, NQ, WAY], F32, kind="ExternalOutput")
    dbg = {}
    if debug:
        dbg["qemb"] = nc.dram_tensor("dbg_qemb", [128, DC, RHAT], F32, kind="ExternalOutput")
        dbg["semb"] = nc.dram_tensor("dbg_semb", [128, DC, SALL], F32, kind="ExternalOutput")
        dbg["snorm"] = nc.dram_tensor("dbg_snorm", [1, SALL], F32, kind="ExternalOutput")
        dbg["qnorm"] = nc.dram_tensor("dbg_qnorm", [128, ITILES], F32, kind="ExternalOutput")
        dbg["rec"] = nc.dram_tensor("dbg_rec", [WAY, OW], F32, kind="ExternalOutput")
        dbg["mask"] = nc.dram_tensor("dbg_mask", [WAY, WAY - 1, S], F32, kind="ExternalOutput")
        dbg["dmax"] = nc.dram_tensor("dbg_dmax", [128, ITILES, WAY], F32, kind="ExternalOutput")
        dbg["nave2"] = nc.dram_tensor("dbg_nave2", [128, ITILES, WAY], F32, kind="ExternalOutput")
        dbg["pos"] = nc.dram_tensor("dbg_pos", [128, ITILES, WAY], F32, kind="ExternalOutput")
        dbg["ct"] = nc.dram_tensor("dbg_ct", [WAY, RHAT], F32, kind="ExternalOutput")

    with tile.TileContext(nc) as tc:
        _body(nc, tc, qd_d, sd_d, w_d, id_d, sel_d, padv_d, cc_d, out_d, dbg, sim1)
    nc.finalize()
    return nc


def _body(nc, tc, qd_d, sd_d, w_d, id_d, sel_d, padv_d, cc_d, out_d, dbg, sim1):
    AT = mybir.AluOpType
    ACTF = mybir.ActivationFunctionType
    X = mybir.AxisListType.X

    persist = tc.alloc_tile_pool(name="persist", bufs=1)
    dram = tc.alloc_tile_pool(name="dram", bufs=1, space="DRAM")

    # DRAM scratch
    p_dram = dram.tile([WAY, RROWS, PROW2], BF16, tag="p_scratch")
    posw_dram = dram.tile([WAY, 16, ITILES * 8], I16, tag="posw")
    snorm_dram = dram.tile([1, 3840], F32, tag="snormd")
    mask_dram = dram.tile([WAY, WAY - 1, RROWS], BF16, tag="maskd")
    cc_in = dram.tile([WAY, OW], F32, tag="cc_in")
    cc_out = dram.tile([WAY, OW], F32, tag="cc_out")

    # persistent SBUF
    q_embT = persist.tile([128, DC + 1, RHAT], F8, tag="q_embT")
    s_embT = persist.tile([128, DC + 1, SALL], F8, tag="s_embT")
    snh = persist.tile([1, SALL], BF16, tag="snh")          # -snorm/2
    qnorm = persist.tile([128, ITILES], F32, tag="qnorm")
    pnorm = persist.tile([128, WAY * CTILES], F32, tag="pnorm")
    m16a = persist.tile([128, ITILES, 16], F32, tag="m16a")
    dmax_all = persist.tile([128, ITILES, WAY], F32, tag="dmax_all")
    nave2 = persist.tile([128, ITILES, WAY], F32, tag="nave2")
    pos16 = persist.tile([128, ITILES, WAY], I16, tag="pos16")
    ident = persist.tile([128, 128], BF16, tag="ident")
    ones_bf = persist.tile([128, 1], BF16, tag="ones_bf")
    ones_f = persist.tile([128, 1], F32, tag="ones_f")
    ones_row = persist.tile([1, 128], BF16, tag="ones_row")
    padv = persist.tile([128, 1], F32, tag="padv")
    cconst = persist.tile([WAY, 2], F32, tag="cconst")

    nc.vector.memset(ones_bf[:], 1.0)
    nc.vector.memset(ones_f[:], 1.0)
    nc.vector.memset(ones_row[:], 1.0)
    nc.sync.dma_start(padv[:], padv_d[:, :])
    nc.sync.dma_start(ident[:], id_d[:, :])
    nc.sync.dma_start(cconst[:], cc_d[:, :])
    nc.vector.memset(q_embT[:, :, R:RHAT], 0.0)
    nc.vector.memset(q_embT[:, DC], 0.0)
    nc.vector.memset(s_embT[:, DC], 0.0)

    # ================= Phase 1: per-frame half products + tuple assembly ====
    with tc.tile_pool(name="emb", bufs=1) as emb, \
         tc.tile_pool(name="embsm", bufs=3) as embsm, \
         tc.tile_pool(name="embps", bufs=4, space="PSUM") as embps:
        wT = emb.tile([128, 2, 8, 2, DOUT], F8, tag="wT")
        nc.sync.dma_start(wT[:], w_d.rearrange("a b p c d -> p a b c d"))
        qd = emb.tile([128, 16, SEQ_LEN * NQ], F8, tag="qd")
        nc.sync.dma_start(qd[:], qd_d[:, :, :])
        sd = emb.tile([128, 16, SEQ_LEN * 80], F8, tag="sd")
        nc.sync.dma_start(sd[:], sd_d[:, :, :])
        Pq = emb.tile([128, DC, 2, SEQ_LEN * NQ], BF16, tag="Pq")
        Ps = emb.tile([128, DC, 2, SEQ_LEN * 80], BF16, tag="Ps")

        # s-side first: its embeddings gate snorm -> SS -> gathers
        for half in range(2):
            for dc in range(DC):
                for ch in range(2):
                    ps2 = embps.tile([128, SEQ_LEN * NQ], F32, tag="emb_ps")
                    for kc2 in range(8):
                        nc.tensor.matmul(
                            ps2[:], wT[:, half, kc2, :, dc * 128:(dc + 1) * 128],
                            sd[:, 2 * kc2:2 * kc2 + 2, ch * 400:(ch + 1) * 400],
                            start=(kc2 == 0), stop=(kc2 == 7), perf_mode=DR)
                    nc.vector.tensor_copy(
                        Ps[:, dc, half, ch * 400:(ch + 1) * 400], ps2[:])
        for t in range(T):
            f1, f2 = int(TUPLES[t][0]), int(TUPLES[t][1])
            pres = embsm.tile([128, DC, 80], BF16, tag="pres")
            nc.vector.tensor_tensor(
                pres[:], Ps[:, :, 0, f1 * 80:(f1 + 1) * 80],
                Ps[:, :, 1, f2 * 80:(f2 + 1) * 80], AT.add)
            dst = s_embT[:, :DC].rearrange("p d (u t) -> p d t u", t=T)[:, :, t]
            nc.scalar.activation(dst, pres[:], ACTF.Relu, scale=1.0 / 64.0)
        # q side
        for half in range(2):
            for dc in range(DC):
                ps = embps.tile([128, SEQ_LEN * NQ], F32, tag="emb_ps")
                for kc2 in range(8):
                    nc.tensor.matmul(
                        ps[:], wT[:, half, kc2, :, dc * 128:(dc + 1) * 128],
                        qd[:, 2 * kc2:2 * kc2 + 2, :],
                        start=(kc2 == 0), stop=(kc2 == 7), perf_mode=DR)
                nc.vector.tensor_copy(Pq[:, dc, half], ps[:])
        for t in range(T):
            f1, f2 = int(TUPLES[t][0]), int(TUPLES[t][1])
            preq = embsm.tile([128, DC, NQ], BF16, tag="preq")
            nc.vector.tensor_tensor(
                preq[:], Pq[:, :, 0, f1 * NQ:(f1 + 1) * NQ],
                Pq[:, :, 1, f2 * NQ:(f2 + 1) * NQ], AT.add)
            nc.scalar.activation(q_embT[:, :DC, t * NQ:(t + 1) * NQ], preq[:],
                                 ACTF.Relu, scale=1.0 / 64.0)

    # dT allocated after the emb pool frees wT/Pq/Ps space
    dtp = tc.alloc_tile_pool(name="dtp", bufs=1)
    dT = [dtp.tile([128, CTILES, RHAT], F8, tag=f"dT{c}", name=f"dT{c}")
          for c in range(WAY)]

    if dbg:
        with tc.tile_pool(name="dbge", bufs=1) as dbge:
            t1 = dbge.tile([128, DC, RHAT], F32, tag="dbq")
            nc.vector.tensor_copy(t1[:], q_embT[:, :DC])
            nc.sync.dma_start(dbg["qemb"].ap(), t1[:])
            t2 = dbge.tile([128, DC, SALL], F32, tag="dbs")
            nc.vector.tensor_copy(t2[:], s_embT[:, :DC])
            nc.sync.dma_start(dbg["semb"].ap(), t2[:])

    # ================= Phase 2: norms =================
    with tc.tile_pool(name="nrm", bufs=2) as nrm, \
         tc.tile_pool(name="nrmps", bufs=2, space="PSUM") as nrmps:
        snrow = nrm.tile([1, SALL], F32, tag="snrow")
        for scn in range(8):
            ps = nrmps.tile([1, 450], F32, tag="sn_ps")
            for dc in range(DC):
                sq = nrm.tile([128, 450], BF16, tag="sn_sqb")
                nc.gpsimd.tensor_tensor(sq[:], s_embT[:, dc, scn * 450:(scn + 1) * 450],
                                        s_embT[:, dc, scn * 450:(scn + 1) * 450],
                                        AT.mult)
                nc.tensor.matmul(ps[:], ones_bf[:], sq[:],
                                 start=(dc == 0), stop=(dc == DC - 1))
            nc.scalar.activation(snrow[:, scn * 450:(scn + 1) * 450], ps[:], ACTF.Copy)
        nc.vector.tensor_scalar(snh[:], snrow[:], -0.5, None, AT.mult)
        nc.sync.dma_start(snorm_dram[:, :SALL], snrow[:])
        # pnorm[p, c*6+j] = snorm[720c + 128j + p]
        for c in range(WAY):
            nc.sync.dma_start(
                pnorm[:, c * CTILES:(c + 1) * CTILES],
                _ap(snorm_dram.tensor, snorm_dram.offset + c * S,
                    [(1, 128), (128, CTILES)]))
        for it in range(ITILES):
            ps = nrmps.tile([128, 1], F32, tag="qn_ps", name="qnps")
            sqa = nrm.tile([128, DC, 128], BF16, tag="qn_sqb")
            for dc in range(DC):
                nc.scalar.activation(sqa[:, dc],
                                     q_embT[:, dc, it * 128:(it + 1) * 128],
                                     ACTF.Square)
            for dc in range(DC):
                nc.tensor.matmul(ps[:], sqa[:, dc], ones_bf[:],
                                 start=(dc == 0), stop=(dc == DC - 1))
            nc.vector.tensor_copy(qnorm[:, it:it + 1], ps[:])
        if dbg:
            nc.sync.dma_start(dbg["snorm"].ap(), snrow[:])
            nc.sync.dma_start(dbg["qnorm"].ap(), qnorm[:])

    # ====== Phases 3+4: per class, SS slab -> D tiles -> gather/rec ======
    with tc.tile_pool(name="ssst", bufs=1) as ssst, \
         tc.tile_pool(name="ssps", bufs=2, space="PSUM") as ssps, \
         tc.tile_pool(name="dph", bufs=2) as dph, \
         tc.tile_pool(name="dsm", bufs=4) as dsm, \
         tc.tile_pool(name="dps", bufs=3, space="PSUM") as dps, \
         tc.tile_pool(name="tps", bufs=2, space="PSUM") as tps, \
         tc.tile_pool(name="cdp", bufs=2) as cdp, \
         tc.tile_pool(name="rrp", bufs=1) as rrp, \
         tc.tile_pool(name="cdg", bufs=2) as cdg:
        for c in range(WAY):
            # ---- SS slab for class c ----
            staging = ssst.tile([128, CTILES, PROW2], BF16, tag="ss_stage")
            chunks = _chunks_for_class(c)
            for j in range(CTILES):
                p0 = S * c + 128 * j
                pw = min(128, S - 128 * j)
                for (dst0, pieces) in chunks:
                    ps = ssps.tile([128, 480], F32, tag="ss_ps")
                    for (src0, doff, w) in pieces:
                        for dc2 in range(NDR):
                            nc.tensor.matmul(
                                ps[:pw, doff:doff + w],
                                s_embT[:, 2 * dc2:2 * dc2 + 2, p0:p0 + pw],
                                s_embT[:, 2 * dc2:2 * dc2 + 2, src0:src0 + w],
                                start=(dc2 == 0), stop=False, perf_mode=DR)
                        nc.tensor.matmul(
                            ps[:pw, doff:doff + w], ones_row[:, :pw],
                            snh[:, src0:src0 + w], start=False, stop=True)
                    nc.scalar.activation(staging[:pw, j, dst0:dst0 + 480],
                                         ps[:pw], ACTF.Copy, scale=-2.0)
            # row-norm column (col 2880) for the gathered threshold
            nc.vector.tensor_copy(staging[:, :, OW:OW + 1],
                                  pnorm[:, c * CTILES:(c + 1) * CTILES, None])
            # write region, skipping the undefined pad rows of the last tile
            nc.sync.dma_start(
                _ap(p_dram.tensor, p_dram.offset + c * RROWS * PROW2,
                    [(PROW2, 128), (128 * PROW2, CTILES - 1), (1, PROW2)]),
                staging[:, :CTILES - 1])
            nc.sync.dma_start(
                _ap(p_dram.tensor,
                    p_dram.offset + (c * RROWS + (CTILES - 1) * 128) * PROW2,
                    [(PROW2, 80), (1, PROW2)]),
                staging[:80, CTILES - 1])
            # ---- D tiles ----
            for it in range(ITILES):
                d_bf = dph.tile([128, RROWS], BF16, tag="d_bf")
                for sc in range(2):
                    ps = dps.tile([128, 360], F32, tag="d_ps", name="dps")
                    s0 = c * S + sc * 360
                    for dc2 in range(NDR):
                        nc.tensor.matmul(
                            ps[:], q_embT[:, 2 * dc2:2 * dc2 + 2, it * 128:(it + 1) * 128],
                            s_embT[:, 2 * dc2:2 * dc2 + 2, s0:s0 + 360],
                            start=(dc2 == 0), stop=False, perf_mode=DR)
                    nc.tensor.matmul(ps[:], ones_row[:], snh[:, s0:s0 + 360],
                                     start=False, stop=True)
                    nc.scalar.activation(d_bf[:, sc * 360:(sc + 1) * 360], ps[:],
                                         ACTF.Sqrt, bias=qnorm[:, it:it + 1],
                                         scale=-2.0)
                nc.vector.memset(d_bf[:, S:RROWS], 0.0)
                # reductions
                nc.vector.tensor_reduce(
                    m16a[:, it], d_bf[:, :S].rearrange("p (a b) -> p b a", b=16),
                    X, AT.max)
                mx8 = dsm.tile([128, 8], F32, tag="mx8")
                ix8 = dsm.tile([128, 8], U32, tag="ix8")
                nc.vector.max(mx8[:], d_bf[:, :S])
                nc.vector.max_index(ix8[:], mx8[:], d_bf[:, :S])
                posf = dsm.tile([128, 1], F32, tag="posf")
                nc.vector.tensor_scalar(posf[:], ix8[:, 0:1], 0.0, None, AT.add)
                nc.vector.tensor_copy(pos16[:, it, c:c + 1], posf[:])
                # transpose into dT (fp8)
                psT = tps.tile([128, CTILES * 128], BF16, tag="psT")
                for j in range(CTILES):
                    nc.tensor.matmul(psT[:, j * 128:(j + 1) * 128],
                                     d_bf[:, j * 128:(j + 1) * 128], ident[:],
                                     start=True, stop=True, is_transpose=True)
                nc.vector.tensor_copy(
                    dT[c][:, :, it * 128:(it + 1) * 128],
                    psT[:].rearrange("p (j q) -> p j q", j=CTILES))
            nc.vector.memset(dT[c][:, :, R:RHAT], 0.0)
            # batched per-class stats: nave2 = -(asum/16)^2 ; dmax
            asum = dsm.tile([128, ITILES], F32, tag="asum")
            nc.vector.tensor_reduce(asum[:], m16a[:], X, AT.add)
            nc.vector.tensor_scalar(asum[:, ITILES - 1:ITILES],
                                    asum[:, ITILES - 1:ITILES], padv[:], None, AT.add)
            nc.vector.tensor_tensor(asum[:], asum[:], asum[:], AT.mult)
            nc.vector.tensor_scalar(nave2[:, :, c], asum[:], -1.0 / 256.0, None,
                                    AT.mult)
            nc.vector.tensor_reduce(dmax_all[:, :, c], m16a[:], X, AT.max)

            # ---- gather + rec ----
            nc.sync.dma_start(
                _ap(posw_dram.tensor, posw_dram.offset + c * 16 * ITILES * 8,
                    [(1, 8), (ITILES * 8, 16), (8, ITILES)]),
                pos16[:, :, c])
            idxs = cdp.tile([128, ITILES * 8], I16, tag="idxs")
            nc.sync.dma_start(
                idxs[:],
                _ap(posw_dram.tensor, posw_dram.offset + c * 16 * ITILES * 8,
                    [(0, 8), (ITILES * 8, 16), (1, ITILES * 8)]))
            use_sign = c in SIGN_CLASSES
            if use_sign:
                acc = cdp.tile([128, OW], BF16, tag="accb")
            else:
                acc = cdp.tile([128, OW], F32, tag="accf")
            region = _ap(p_dram.tensor, p_dram.offset + c * RROWS * PROW2,
                         [(PROW2, RROWS), (1, PROW2)])
            for g in range(ITILES):
                cd = cdg.tile([128, 1, PROW2], BF16, tag="cd")
                nc.gpsimd.dma_gather(
                    cd[:], region, idxs[:, g * 8:(g + 1) * 8],
                    128, 128, PROW2, queue_num=g % 4)
                # bias = n_i - ave^2 : sign(cd_raw + bias) = sign(d^2 - ave^2)
                nbias = cdg.tile([128, 1], F32, tag="nbias")
                nc.vector.tensor_tensor(nbias[:], cd[:, 0, OW:OW + 1],
                                        nave2[:, g, c:c + 1], AT.add)
                if use_sign:
                    cmp = cdg.tile([128, OW], BF16, tag="cmp")
                    nc.scalar.activation(cmp[:], cd[:, 0, :OW], ACTF.Sign,
                                         bias=nbias[:])
                    if g == 0:
                        nc.vector.tensor_copy(acc[:], cmp[:])
                    else:
                        nc.vector.tensor_tensor(acc[:], acc[:], cmp[:], AT.add)
                else:
                    # threshold: cd_raw > -(bias)
                    nc.vector.tensor_scalar(nbias[:], nbias[:], -1.0, None, AT.mult)
                    if g == 0:
                        nc.vector.memset(acc[:], 0.0)
                    nc.vector.scalar_tensor_tensor(
                        acc[:], cd[:, 0, :OW], nbias[:], acc[:],
                        op0=AT.is_gt, op1=AT.add)
            recrow = rrp.tile([1, OW], F32, tag="recrow")
            for k in range(6):
                ps = ssps.tile([1, 480], F32, tag="ss_ps", name="recps")
                if use_sign:
                    nc.tensor.matmul(ps[:], ones_bf[:], acc[:, k * 480:(k + 1) * 480],
                                     start=True, stop=True)
                else:
                    nc.tensor.matmul(ps[:], ones_f[:], acc[:, k * 480:(k + 1) * 480],
                                     start=True, stop=True)
                nc.scalar.activation(recrow[:, k * 480:(k + 1) * 480], ps[:],
                                     ACTF.Copy)
            nc.sync.dma_start(cc_in[c:c + 1, :], recrow[:])

    # ================= AllReduce rec =================
    if sim1:
        nc.sync.dma_start(cc_out[:, :], cc_in[:, :])
    else:
        nc.gpsimd.collective_compute(
            "AllReduce", mybir.AluOpType.add,
            replica_groups=[list(range(NCORES))],
            ins=[cc_in[:, :].opt()], outs=[cc_out[:, :].opt()])

    # ================= Phase 5: thr/mask =================
    with tc.tile_pool(name="thrp", bufs=2) as thrp, \
         tc.tile_pool(name="thrbig", bufs=1) as thrbig:
        rec = thrbig.tile([WAY, WAY - 1, S], F32, tag="rec")
        nc.sync.dma_start(rec[:], cc_out[:, :].rearrange("c (k s) -> c k s", k=WAY - 1))
        # per-class affine: sign classes hold sum(+-1); counts = (x+15360)/2
        nc.vector.tensor_scalar(rec[:], rec[:], cconst[:, 0:1], None, AT.add)
        nc.vector.tensor_scalar(rec[:], rec[:], cconst[:, 1:2], None, AT.mult)
        if dbg:
            with tc.tile_pool(name="dbgr", bufs=1) as dbgr:
                rg = dbgr.tile([WAY, OW], F32, tag="rg")
                nc.vector.tensor_copy(rg[:], rec[:].rearrange("c k s -> c (k s)"))
                nc.sync.dma_start(dbg["rec"].ap(), rg[:])
        rsum = thrp.tile([WAY, WAY - 1], F32, tag="rsum")
        nc.vector.tensor_reduce(rsum[:], rec[:], X, AT.add)
        gt0 = thrbig.tile([WAY, WAY - 1, S], F32, tag="gt0")
        nc.vector.tensor_scalar(gt0[:], rec[:], 0.5, None, AT.is_gt)
        nz = thrp.tile([WAY, WAY - 1], F32, tag="nz")
        nc.vector.tensor_reduce(nz[:], gt0[:], X, AT.add)
        nc.vector.tensor_scalar(nz[:], nz[:], 1.0, None, AT.max)
        thr = thrp.tile([WAY, WAY - 1], F32, tag="thr")
        nc.vector.reciprocal(thr[:], nz[:])
        nc.vector.tensor_tensor(thr[:], thr[:], rsum[:], AT.mult)
        mask_slots = thrbig.tile([WAY, WAY - 1, S], BF16, tag="mask_slots")
        nc.vector.tensor_tensor(
            mask_slots[:], rec[:],
            thr[:, :, None].to_broadcast((WAY, WAY - 1, S)), AT.is_lt)
        if dbg:
            with tc.tile_pool(name="dbgm", bufs=1) as dbgm:
                mg = dbgm.tile([WAY, WAY - 1, S], F32, tag="mg")
                nc.vector.tensor_copy(mg[:], mask_slots[:])
                nc.sync.dma_start(dbg["mask"].ap(), mg[:])
        msum = thrp.tile([WAY, 1], F32, tag="msum")
        nc.vector.tensor_reduce(msum[:], mask_slots[:].rearrange("c k s -> c (k s)"),
                                X, AT.add)
        nc.vector.tensor_scalar(msum[:], msum[:], 1.0, None, AT.max)
        scv = thrp.tile([WAY, 1], F32, tag="scv")
        nc.vector.reciprocal(scv[:], msum[:])
        nc.vector.tensor_scalar(scv[:], scv[:], 1.0 / (4.0 * T), None, AT.mult)
        nc.sync.dma_start(mask_dram[:, :, :S], mask_slots[:])

        # ============= Phase 6: contrast sums + finals =============
        with tc.tile_pool(name="p6", bufs=1) as p6, \
             tc.tile_pool(name="p6ps", bufs=1, space="PSUM") as p6ps:
            maskT = p6.tile([128, WAY * CTILES, WAY], BF16, tag="maskT")
            nc.vector.memset(maskT[:], 0.0)
            for c in range(WAY):
                if c > 0:
                    nc.sync.dma_start(
                        maskT[:, 0:c * CTILES, c],
                        _ap(mask_dram.tensor,
                            mask_dram.offset + c * (WAY - 1) * RROWS,
                            [(1, 128), (128, c * CTILES)]))
                if c < WAY - 1:
                    nc.sync.dma_start(
                        maskT[:, (c + 1) * CTILES:WAY * CTILES, c],
                        _ap(mask_dram.tensor,
                            mask_dram.offset + (c * (WAY - 1) + c) * RROWS,
                            [(1, 128), (128, (WAY - 1 - c) * CTILES)]))
            psC = [p6ps.tile([WAY, 480], F32, tag=f"ct_ps{qc}", name=f"ctps{qc}")
                   for qc in range(4)]
            for cr in range(WAY):
                for j in range(CTILES):
                    for qc in range(4):
                        nc.tensor.matmul(
                            psC[qc][:], maskT[:, cr * CTILES + j, :],
                            dT[cr][:, j, qc * 480:(qc + 1) * 480],
                            start=(cr == 0 and j == 0),
                            stop=(cr == WAY - 1 and j == CTILES - 1))
            ctrows = p6.tile([WAY, RHAT], F32, tag="ctrows")
            for qc in range(4):
                nc.scalar.activation(ctrows[:, qc * 480:(qc + 1) * 480], psC[qc][:],
                                     ACTF.Copy)
            if dbg:
                nc.sync.dma_start(dbg["ct"].ap(), ctrows[:])
            ctq = p6.tile([WAY, NQ], F32, tag="ctq")
            nc.vector.tensor_reduce(
                ctq[:], ctrows[:].rearrange("c (s q) -> c q s", q=NQ), X, AT.add)
            nc.vector.tensor_scalar(ctq[:], ctq[:], scv[:], None, AT.mult)

            sel_sb = p6.tile([128, ITILES, NQ], F32, tag="sel_sb")
            nc.sync.dma_start(sel_sb[:], sel_d.rearrange("t p q -> p t q"))
            dmq = p6.tile([WAY, NQ], F32, tag="dmq")
            psD = p6ps.tile([WAY, NQ], F32, tag="dm_ps")
            for it in range(ITILES):
                nc.tensor.matmul(psD[:], dmax_all[:, it, :], sel_sb[:, it],
                                 start=(it == 0), stop=(it == ITILES - 1))
            nc.scalar.activation(dmq[:], psD[:], ACTF.Copy, scale=1.0 / T)
            if dbg:
                nc.sync.dma_start(dbg["dmax"].ap(), dmax_all[:])
                nc.sync.dma_start(dbg["nave2"].ap(), nave2[:])
                with tc.tile_pool(name="dbgp", bufs=1) as dbgp:
                    pf = dbgp.tile([128, ITILES, WAY], F32, tag="pf")
                    nc.vector.tensor_copy(pf[:], pos16[:])
                    nc.sync.dma_start(dbg["pos"].ap(), pf[:])

            ssum = p6.tile([WAY, NQ], F32, tag="ssum")
            nc.vector.tensor_tensor(ssum[:], dmq[:], ctq[:], AT.add)
            rcp = p6.tile([WAY, NQ], F32, tag="rcp")
            nc.vector.reciprocal(rcp[:], ssum[:])
            lg = p6.tile([WAY, NQ], F32, tag="lg")
            nc.vector.tensor_tensor(lg[:], dmq[:], rcp[:], AT.mult)
            nc.sync.dma_start(_ap(out_d, 0, [(1, WAY), (WAY, NQ)]), dmq[:])
            nc.sync.dma_start(_ap(out_d, NQ * WAY, [(1, WAY), (WAY, NQ)]), lg[:])

    dtp.release()
    persist.release()
    dram.release()


# ---------------- host side ----------------

def _sel_host():
    sel = np.zeros((ITILES, 128, NQ), np.float32)
    for i in range(R):
        sel[i // 128, i % 128, i % NQ] = 1.0
    return sel


def _prep_inputs(support_set, queries, support_labels, W, b):
    import ml_dtypes
    f8 = ml_dtypes.float8_e4m3fn
    support_set = np.asarray(support_set, dtype=np.float32)
    queries = np.asarray(queries, dtype=np.float32)
    labels = np.asarray(support_labels).astype(np.int64)
    W = np.asarray(W, dtype=np.float32)
    b = np.asarray(b, dtype=np.float32)
    assert not np.any(b), "kernel built without bias support (reference b==0)"
    order = np.argsort(labels, kind="stable")
    support_sorted = support_set[order]

    # wT [2, 8, 128, 2, 1152]: wT[half, kc2, p, h2, d] =
    #   64*W[d, half*2048 + kc2*256 + h2*128 + p]
    w8 = (W * 64.0).astype(f8)                     # [1152, 4096]
    wT = np.ascontiguousarray(
        w8.reshape(DOUT, 2, 8, 2, 128).transpose(1, 2, 4, 3, 0))

    s8 = support_sorted.astype(f8)                 # [80, 10, 2048]
    sd = np.ascontiguousarray(
        s8.reshape(80, SEQ_LEN, 16, 128).transpose(3, 2, 1, 0)
          .reshape(128, 16, SEQ_LEN * 80))
    q8 = queries.astype(f8)                        # [320, 10, 2048]
    sel = _sel_host()
    padv = np.zeros((128, 1), np.float32)
    padv[8:] = 1.0e15
    ident = np.eye(128).astype(ml_dtypes.bfloat16)
    cconst = np.zeros((WAY, 2), np.float32)
    for c in range(WAY):
        if c in SIGN_CLASSES:
            cconst[c] = (NCORES * RHAT, 0.5)
        else:
            cconst[c] = (0.0, 1.0)
    out = []
    for k in range(NCORES):
        qk = q8[k * NQ:(k + 1) * NQ]               # [40, 10, 2048]
        qd = np.ascontiguousarray(
            qk.reshape(NQ, SEQ_LEN, 16, 128).transpose(3, 2, 1, 0)
              .reshape(128, 16, SEQ_LEN * NQ))
        out.append({
            "qd": qd,
            "sd": sd,
            "wT": wT,
            "ident": ident,
            "sel": sel,
            "padv": padv,
            "cconst": cconst,
        })
    return out


def kernel(**inputs):
    per_core = _prep_inputs(**inputs)
    if "nc" not in _CACHE:
        _CACHE["nc"] = build(debug=bool(os.environ.get("BIMACL_DEBUG")))
    nc = _CACHE["nc"]
    res = run_bass_kernel_spmd(nc, per_core, core_ids=list(range(NCORES)))
    _CACHE["last_results"] = res
    full = np.concatenate([res.results[k]["out"] for k in range(NCORES)], axis=1)
    return np.ascontiguousarray(full.astype(np.float32))


# revision 23
# speedup vs baseline: 2.7587x; 1.0048x over previous
"""Trainium2 Bass kernel for nn_CNN_BiMACL_31860067401819 (retrieval_knn).

Self-contained: hardcodes all shapes/sharding. kernel(**inputs) accepts FULL
inputs keyed as in setup_inputs(), shards queries across 8 NeuronCores
(data-parallel over the query axis), and returns the FULL [2, 320, 5] f32
output. The only collective is an AllReduce of the per-class `rec` counts.

Design (v2):
- Frame-factorized embeddings: emb(tuple t=(f1,f2)) = relu(W1^T x_f1 +
  W2^T x_f2); per-frame half-products are computed once with fp8 DoubleRow
  matmuls, tuples assembled with bf16 adds + ACT relu into fp8 embeddings.
- All distance matmuls fp8 + DoubleRow (256-deep contraction per instr).
- SS (support-support) stays in d^2 space: psum = s_i.s_j - sn_j/2 (column
  norm folded in via a 1-row matmul), scaled by -2 on the psum->SBUF copy;
  row norm rides along as an extra gathered column. rec compare is then
  cd_raw + n_i > ave^2  <=>  d^2 > ave^2 (no sqrt for SS at all).
- D (query-support): psum = q.s - sn/2; ACT Sqrt(scale=-2, bias=qnorm)
  emits bf16 distances directly.
- rec compare+accumulate: ACT Sign for SIGN_CLASSES (affine-corrected after
  the AllReduce), fused DVE scalar_tensor_tensor (is_gt,add) for the rest.
- Phase-6 masked row sums via PE transpose of D (stored fp8) + mask matmuls.
"""
import os
from itertools import combinations

import numpy as np

import concourse.bass as bass
import concourse.tile as tile
from concourse import bacc, mybir
from concourse.bass_utils import run_bass_kernel_spmd

# ---- static problem config ----
WAY, SHOT, SEQ_LEN, TSS = 5, 16, 10, 2
DIN, DOUT = 2048, 1152
N_QUERIES = 320
T = 45
S = SHOT * T                 # 720 support tuples per class
SALL = WAY * S               # 3600
NCORES = 8
NQ = N_QUERIES // NCORES     # 40
R = NQ * T                   # 1800 valid rows/core
RHAT = 1920                  # 15*128 padded rows
ITILES = RHAT // 128         # 15
TUPLES = np.array(list(combinations(range(SEQ_LEN), TSS)), dtype=np.int32)
DC = DOUT // 128             # 9
NDR = 5                      # DoubleRow matmuls per padded 1280 contraction (5*256)
OW = (WAY - 1) * S           # 2880 other-class columns
PROW2 = 2944                 # per-class region row pitch (2880 data + norm + pad)
CTILES = 6                   # 128-row tiles per class region (768 rows)
RROWS = CTILES * 128         # 768
SIGN_CLASSES = (0, 1, 3, 4)  # rec compare on ACT Sign; class 2 DVE stt

F32 = mybir.dt.float32
BF16 = mybir.dt.bfloat16
F8 = mybir.dt.float8e4
U32 = mybir.dt.uint32
I16 = mybir.dt.int16
DR = mybir.MatmulPerfMode.DoubleRow

_CACHE = {}


def _ap(tensor, offset, dims):
    return bass.AP(tensor=tensor, offset=offset, ap=[list(d) for d in dims])


def _chunks_for_class(c):
    """960-wide dst chunks over the 2880 other-class cols of class c, each
    split into <=480-wide matmul pieces (PSUM-bank limit).
    Returns list of (dst_off, [(src_col, dst_delta, width), ...])."""
    spans = []
    if c > 0:
        spans.append((0, 0, S * c))              # (dst0, src0, len)
    spans.append((S * c, S * (c + 1), OW - S * c))
    out = []
    for dst0 in range(0, OW, 960):
        pieces = []
        for sub in range(2):
            w0 = dst0 + sub * 480
            for sd, ss, ln in spans:
                lo = max(w0, sd)
                hi = min(w0 + 480, sd + ln)
                if lo < hi:
                    pieces.append((ss + (lo - sd), lo - dst0, hi - lo))
        out.append((dst0, pieces))
    return out


def build(debug=False, sim1=False):
    nc = bacc.Bacc(num_swdge_queues=4)
    qd_d = nc.dram_tensor("qd", [128, 16, SEQ_LEN * NQ], F8, kind="ExternalInput")
    sd_d = nc.dram_tensor("sd", [128, 16, SEQ_LEN * 80], F8, kind="ExternalInput")
    w_d = nc.dram_tensor("wT", [2, 8, 128, 2, DOUT], F8, kind="ExternalInput")
    id_d = nc.dram_tensor("ident", [128, 128], BF16, kind="ExternalInput")
    sel_d = nc.dram_tensor("sel", [ITILES, 128, NQ], F32, kind="ExternalInput")
    padv_d = nc.dram_tensor("padv", [128, 1], F32, kind="ExternalInput")
    cc_d = nc.dram_tensor("cconst", [WAY, 2], F32, kind="ExternalInput")
    out_d = nc.dram_tensor("out", [2, NQ, WAY], F32, kind="ExternalOutput")
    dbg = {}
    if debug:
        dbg["qemb"] = nc.dram_tensor("dbg_qemb", [128, DC, RHAT], F32, kind="ExternalOutput")
        dbg["semb"] = nc.dram_tensor("dbg_semb", [128, DC, SALL], F32, kind="ExternalOutput")
        dbg["snorm"] = nc.dram_tensor("dbg_snorm", [1, SALL], F32, kind="ExternalOutput")
        dbg["qnorm"] = nc.dram_tensor("dbg_qnorm", [128, ITILES], F32, kind="ExternalOutput")
        dbg["rec"] = nc.dram_tensor("dbg_rec", [WAY, OW], F32, kind="ExternalOutput")
        dbg["mask"] = nc.dram_tensor("dbg_mask", [WAY, WAY - 1, S], F32, kind="ExternalOutput")
        dbg["dmax"] = nc.dram_tensor("dbg_dmax", [128, ITILES, WAY], F32, kind="ExternalOutput")
        dbg["nave2"] = nc.dram_tensor("dbg_nave2", [128, ITILES, WAY], F32, kind="ExternalOutput")
        dbg["pos"] = nc.dram_tensor("dbg_pos", [128, ITILES, WAY], F32, kind="ExternalOutput")
        dbg["ct"] = nc.dram_tensor("dbg_ct", [WAY, RHAT], F32, kind="ExternalOutput")

    with tile.TileContext(nc) as tc:
        _body(nc, tc, qd_d, sd_d, w_d, id_d, sel_d, padv_d, cc_d, out_d, dbg, sim1)
    nc.finalize()
    return nc


def _body(nc, tc, qd_d, sd_d, w_d, id_d, sel_d, padv_d, cc_d, out_d, dbg, sim1):
    AT = mybir.AluOpType
    ACTF = mybir.ActivationFunctionType
    X = mybir.AxisListType.X

    persist = tc.alloc_tile_pool(name="persist", bufs=1)
    dram = tc.alloc_tile_pool(name="dram", bufs=1, space="DRAM")

    # DRAM scratch
    p_dram = dram.tile([WAY, RROWS, PROW2], BF16, tag="p_scratch")
    posw_dram = dram.tile([WAY, 16, ITILES * 8], I16, tag="posw")
    snorm_dram = dram.tile([1, 3840], F32, tag="snormd")
    mask_dram = dram.tile([WAY, WAY - 1, RROWS], BF16, tag="maskd")
    cc_in = dram.tile([WAY, OW], F32, tag="cc_in")
    cc_out = dram.tile([WAY, OW], F32, tag="cc_out")

    # persistent SBUF
    q_embT = persist.tile([128, DC + 1, RHAT], F8, tag="q_embT")
    s_embT = persist.tile([128, DC + 1, SALL], F8, tag="s_embT")
    snh = persist.tile([1, SALL], BF16, tag="snh")          # -snorm/2
    qnorm = persist.tile([128, ITILES], F32, tag="qnorm")
    pnorm = persist.tile([128, WAY * CTILES], F32, tag="pnorm")
    m16a = persist.tile([128, ITILES, 16], F32, tag="m16a")
    dmax_all = persist.tile([128, ITILES, WAY], F32, tag="dmax_all")
    nave2 = persist.tile([128, ITILES, WAY], F32, tag="nave2")
    pos16 = persist.tile([128, ITILES, WAY], I16, tag="pos16")
    ident = persist.tile([128, 128], BF16, tag="ident")
    ones_bf = persist.tile([128, 1], BF16, tag="ones_bf")
    ones_f = persist.tile([128, 1], F32, tag="ones_f")
    ones_row = persist.tile([1, 128], BF16, tag="ones_row")
    padv = persist.tile([128, 1], F32, tag="padv")
    cconst = persist.tile([WAY, 2], F32, tag="cconst")
    sel_sb = persist.tile([128, ITILES, NQ], F32, tag="sel_sb")
    dmq = persist.tile([WAY, NQ], F32, tag="dmq")

    nc.vector.memset(ones_bf[:], 1.0)
    nc.vector.memset(ones_f[:], 1.0)
    nc.vector.memset(ones_row[:], 1.0)
    nc.sync.dma_start(padv[:], padv_d[:, :])
    nc.sync.dma_start(ident[:], id_d[:, :])
    nc.sync.dma_start(cconst[:], cc_d[:, :])
    nc.sync.dma_start(sel_sb[:], sel_d.rearrange("t p q -> p t q"))
    nc.vector.memset(q_embT[:, :, R:RHAT], 0.0)
    nc.vector.memset(q_embT[:, DC], 0.0)
    nc.vector.memset(s_embT[:, DC], 0.0)

    # ================= Phase 1: per-frame half products + tuple assembly ====
    with tc.tile_pool(name="emb", bufs=1) as emb, \
         tc.tile_pool(name="embsm", bufs=3) as embsm, \
         tc.tile_pool(name="embps", bufs=4, space="PSUM") as embps:
        wT = emb.tile([128, 2, 8, 2, DOUT], F8, tag="wT")
        nc.sync.dma_start(wT[:], w_d.rearrange("a b p c d -> p a b c d"))
        qd = emb.tile([128, 16, SEQ_LEN * NQ], F8, tag="qd")
        nc.sync.dma_start(qd[:], qd_d[:, :, :])
        sd = emb.tile([128, 16, SEQ_LEN * 80], F8, tag="sd")
        nc.sync.dma_start(sd[:], sd_d[:, :, :])
        Pq = emb.tile([128, DC, 2, SEQ_LEN * NQ], BF16, tag="Pq")
        Ps = emb.tile([128, DC, 2, SEQ_LEN * 80], BF16, tag="Ps")

        # s-side first: its embeddings gate snorm -> SS -> gathers
        for half in range(2):
            for dc in range(DC):
                for ch in range(2):
                    ps2 = embps.tile([128, SEQ_LEN * NQ], F32, tag="emb_ps")
                    for kc2 in range(8):
                        nc.tensor.matmul(
                            ps2[:], wT[:, half, kc2, :, dc * 128:(dc + 1) * 128],
                            sd[:, 2 * kc2:2 * kc2 + 2, ch * 400:(ch + 1) * 400],
                            start=(kc2 == 0), stop=(kc2 == 7), perf_mode=DR)
                    nc.vector.tensor_copy(
                        Ps[:, dc, half, ch * 400:(ch + 1) * 400], ps2[:])
        for t in range(T):
            f1, f2 = int(TUPLES[t][0]), int(TUPLES[t][1])
            pres = embsm.tile([128, DC, 80], BF16, tag="pres")
            nc.vector.tensor_tensor(
                pres[:], Ps[:, :, 0, f1 * 80:(f1 + 1) * 80],
                Ps[:, :, 1, f2 * 80:(f2 + 1) * 80], AT.add)
            dst = s_embT[:, :DC].rearrange("p d (u t) -> p d t u", t=T)[:, :, t]
            nc.scalar.activation(dst, pres[:], ACTF.Relu, scale=1.0 / 64.0)
        # q side
        for half in range(2):
            for dc in range(DC):
                ps = embps.tile([128, SEQ_LEN * NQ], F32, tag="emb_ps")
                for kc2 in range(8):
                    nc.tensor.matmul(
                        ps[:], wT[:, half, kc2, :, dc * 128:(dc + 1) * 128],
                        qd[:, 2 * kc2:2 * kc2 + 2, :],
                        start=(kc2 == 0), stop=(kc2 == 7), perf_mode=DR)
                nc.vector.tensor_copy(Pq[:, dc, half], ps[:])
        for t in range(T):
            f1, f2 = int(TUPLES[t][0]), int(TUPLES[t][1])
            preq = embsm.tile([128, DC, NQ], BF16, tag="preq")
            nc.vector.tensor_tensor(
                preq[:], Pq[:, :, 0, f1 * NQ:(f1 + 1) * NQ],
                Pq[:, :, 1, f2 * NQ:(f2 + 1) * NQ], AT.add)
            nc.scalar.activation(q_embT[:, :DC, t * NQ:(t + 1) * NQ], preq[:],
                                 ACTF.Relu, scale=1.0 / 64.0)

    # dT allocated after the emb pool frees wT/Pq/Ps space
    dtp = tc.alloc_tile_pool(name="dtp", bufs=1)
    dT = [dtp.tile([128, CTILES, RHAT], F8, tag=f"dT{c}", name=f"dT{c}")
          for c in range(WAY)]

    if dbg:
        with tc.tile_pool(name="dbge", bufs=1) as dbge:
            t1 = dbge.tile([128, DC, RHAT], F32, tag="dbq")
            nc.vector.tensor_copy(t1[:], q_embT[:, :DC])
            nc.sync.dma_start(dbg["qemb"].ap(), t1[:])
            t2 = dbge.tile([128, DC, SALL], F32, tag="dbs")
            nc.vector.tensor_copy(t2[:], s_embT[:, :DC])
            nc.sync.dma_start(dbg["semb"].ap(), t2[:])

    # ================= Phase 2: norms =================
    with tc.tile_pool(name="nrm", bufs=2) as nrm, \
         tc.tile_pool(name="nrmps", bufs=2, space="PSUM") as nrmps:
        snrow = nrm.tile([1, SALL], F32, tag="snrow")
        for scn in range(8):
            ps = nrmps.tile([1, 450], F32, tag="sn_ps")
            for dc in range(DC):
                sq = nrm.tile([128, 450], BF16, tag="sn_sqb")
                nc.scalar.activation(sq[:], s_embT[:, dc, scn * 450:(scn + 1) * 450],
                                     ACTF.Square)
                nc.tensor.matmul(ps[:], ones_bf[:], sq[:],
                                 start=(dc == 0), stop=(dc == DC - 1))
            nc.scalar.activation(snrow[:, scn * 450:(scn + 1) * 450], ps[:], ACTF.Copy)
        nc.vector.tensor_scalar(snh[:], snrow[:], -0.5, None, AT.mult)
        nc.sync.dma_start(snorm_dram[:, :SALL], snrow[:])
        # pnorm[p, c*6+j] = snorm[720c + 128j + p]
        for c in range(WAY):
            nc.sync.dma_start(
                pnorm[:, c * CTILES:(c + 1) * CTILES],
                _ap(snorm_dram.tensor, snorm_dram.offset + c * S,
                    [(1, 128), (128, CTILES)]))
        for it in range(ITILES):
            ps = nrmps.tile([128, 1], F32, tag="qn_ps", name="qnps")
            sqa = nrm.tile([128, DC, 128], BF16, tag="qn_sqb")
            for dc in range(DC):
                nc.scalar.activation(sqa[:, dc],
                                     q_embT[:, dc, it * 128:(it + 1) * 128],
                                     ACTF.Square)
            for dc in range(DC):
                nc.tensor.matmul(ps[:], sqa[:, dc], ones_bf[:],
                                 start=(dc == 0), stop=(dc == DC - 1))
            nc.vector.tensor_copy(qnorm[:, it:it + 1], ps[:])
        if dbg:
            nc.sync.dma_start(dbg["snorm"].ap(), snrow[:])
            nc.sync.dma_start(dbg["qnorm"].ap(), qnorm[:])

    # ====== Phases 3+4: per class, SS slab -> D tiles -> gather/rec ======
    with tc.tile_pool(name="ssst", bufs=1) as ssst, \
         tc.tile_pool(name="ssps", bufs=2, space="PSUM") as ssps, \
         tc.tile_pool(name="dph", bufs=2) as dph, \
         tc.tile_pool(name="dsm", bufs=4) as dsm, \
         tc.tile_pool(name="dps", bufs=2, space="PSUM") as dps, \
         tc.tile_pool(name="tps", bufs=2, space="PSUM") as tps, \
         tc.tile_pool(name="cdp", bufs=2) as cdp, \
         tc.tile_pool(name="accp", bufs=2) as accp, \
         tc.tile_pool(name="rrp", bufs=2) as rrp, \
         tc.tile_pool(name="cmpp", bufs=2) as cmpp, \
         tc.tile_pool(name="cdg", bufs=3) as cdg:
        for c in range(WAY):
            # ---- SS slab for class c ----
            staging = ssst.tile([128, CTILES, PROW2], BF16, tag="ss_stage")
            chunks = _chunks_for_class(c)
            for j in range(CTILES):
                p0 = S * c + 128 * j
                pw = min(128, S - 128 * j)
                for (dst0, pieces) in chunks:
                    ps = ssps.tile([128, 960], F32, tag="ss_ps")
                    for (src0, doff, w) in pieces:
                        for dc2 in range(NDR):
                            nc.tensor.matmul(
                                ps[:pw, doff:doff + w],
                                s_embT[:, 2 * dc2:2 * dc2 + 2, p0:p0 + pw],
                                s_embT[:, 2 * dc2:2 * dc2 + 2, src0:src0 + w],
                                start=(dc2 == 0), stop=False, perf_mode=DR)
                        nc.tensor.matmul(
                            ps[:pw, doff:doff + w], ones_row[:, :pw],
                            snh[:, src0:src0 + w], start=False, stop=True)
                    nc.scalar.activation(staging[:pw, j, dst0:dst0 + 960],
                                         ps[:pw], ACTF.Copy, scale=-2.0)
            # row-norm column (col 2880) for the gathered threshold
            nc.vector.tensor_copy(staging[:, :, OW:OW + 1],
                                  pnorm[:, c * CTILES:(c + 1) * CTILES, None])
            # write region, skipping the undefined pad rows of the last tile
            nc.sync.dma_start(
                _ap(p_dram.tensor, p_dram.offset + c * RROWS * PROW2,
                    [(PROW2, 128), (128 * PROW2, CTILES - 1), (1, PROW2)]),
                staging[:, :CTILES - 1])
            nc.sync.dma_start(
                _ap(p_dram.tensor,
                    p_dram.offset + (c * RROWS + (CTILES - 1) * 128) * PROW2,
                    [(PROW2, 80), (1, PROW2)]),
                staging[:80, CTILES - 1])
            # ---- D tiles ----
            for it in range(ITILES):
                d_bf = dph.tile([128, RROWS], BF16, tag="d_bf")
                for sc in range(2):
                    ps = dps.tile([128, 360], F32, tag="d_ps", name="dps")
                    s0 = c * S + sc * 360
                    for dc2 in range(NDR):
                        nc.tensor.matmul(
                            ps[:], q_embT[:, 2 * dc2:2 * dc2 + 2, it * 128:(it + 1) * 128],
                            s_embT[:, 2 * dc2:2 * dc2 + 2, s0:s0 + 360],
                            start=(dc2 == 0), stop=False, perf_mode=DR)
                    nc.tensor.matmul(ps[:], ones_row[:], snh[:, s0:s0 + 360],
                                     start=False, stop=True)
                    nc.scalar.activation(d_bf[:, sc * 360:(sc + 1) * 360], ps[:],
                                         ACTF.Sqrt, bias=qnorm[:, it:it + 1],
                                         scale=-2.0)
                nc.vector.memset(d_bf[:, S:RROWS], 0.0)
                # reductions
                nc.vector.tensor_reduce(
                    m16a[:, it], d_bf[:, :S].rearrange("p (a b) -> p b a", b=16),
                    X, AT.max)
                nc.vector.tensor_reduce(dmax_all[:, it, c:c + 1], m16a[:, it],
                                        X, AT.max)
                mx8 = dsm.tile([128, 8], F32, tag="mx8")
                ix8 = dsm.tile([128, 8], U32, tag="ix8")
                nc.vector.tensor_copy(
                    mx8[:], dmax_all[:, it, c:c + 1].to_broadcast((128, 8)))
                nc.vector.max_index(ix8[:], mx8[:], d_bf[:, :S])
                posf = dsm.tile([128, 1], F32, tag="posf")
                nc.vector.tensor_scalar(posf[:], ix8[:, 0:1], 0.0, None, AT.add)
                nc.vector.tensor_copy(pos16[:, it, c:c + 1], posf[:])
                # transpose into dT (fp8)
                psT = tps.tile([128, CTILES * 128], BF16, tag="psT")
                for j in range(CTILES):
                    nc.tensor.matmul(psT[:, j * 128:(j + 1) * 128],
                                     d_bf[:, j * 128:(j + 1) * 128], ident[:],
                                     start=True, stop=True, is_transpose=True)
                nc.vector.tensor_copy(
                    dT[c][:, :, it * 128:(it + 1) * 128],
                    psT[:].rearrange("p (j q) -> p j q", j=CTILES))
            nc.vector.memset(dT[c][:, :, R:RHAT], 0.0)
            # batched per-class stats: nave2 = -(asum/16)^2 ; dmax
            asum = dsm.tile([128, ITILES], F32, tag="asum")
            nc.vector.tensor_reduce(asum[:], m16a[:], X, AT.add)
            nc.vector.tensor_scalar(asum[:, ITILES - 1:ITILES],
                                    asum[:, ITILES - 1:ITILES], padv[:], None, AT.add)
            nc.vector.tensor_tensor(asum[:], asum[:], asum[:], AT.mult)
            nc.vector.tensor_scalar(nave2[:, :, c], asum[:], -1.0 / 256.0, None,
                                    AT.mult)
            if c == WAY - 1:
                psD = ssps.tile([WAY, NQ], F32, tag="ss_ps", name="dmps")
                for it in range(ITILES):
                    nc.tensor.matmul(psD[:], dmax_all[:, it, :], sel_sb[:, it],
                                     start=(it == 0), stop=(it == ITILES - 1))
                nc.scalar.activation(dmq[:], psD[:], ACTF.Copy, scale=1.0 / T)

            # ---- gather + rec ----
            nc.sync.dma_start(
                _ap(posw_dram.tensor, posw_dram.offset + c * 16 * ITILES * 8,
                    [(1, 8), (ITILES * 8, 16), (8, ITILES)]),
                pos16[:, :, c])
            idxs = cdp.tile([128, ITILES * 8], I16, tag="idxs")
            nc.sync.dma_start(
                idxs[:],
                _ap(posw_dram.tensor, posw_dram.offset + c * 16 * ITILES * 8,
                    [(0, 8), (ITILES * 8, 16), (1, ITILES * 8)]))
            use_sign = c in SIGN_CLASSES
            acc = accp.tile([128, OW], BF16, tag="accb")
            if not use_sign:
                nc.vector.memset(acc[:], 0.0)
                nbneg = cdp.tile([128, ITILES], F32, tag="nbneg")
            region = _ap(p_dram.tensor, p_dram.offset + c * RROWS * PROW2,
                         [(PROW2, RROWS), (1, PROW2)])
            # software-pipelined: gather(g)/nbias(g) run one stage ahead of
            # sign(g)/add(g) so the ACT sign stream never waits on DVE.
            nball = cdp.tile([128, ITILES], F32, tag="nball")
            cds = {}

            def _compare(g):
                if use_sign:
                    cmp = cmpp.tile([128, OW], BF16, tag="cmp")
                    nc.scalar.activation(cmp[:], cds.pop(g)[:, 0, :OW], ACTF.Sign,
                                         bias=nball[:, g:g + 1])
                    if g == 0:
                        nc.vector.tensor_copy(acc[:], cmp[:])
                    else:
                        nc.vector.tensor_tensor(acc[:], acc[:], cmp[:], AT.add)
                else:
                    nc.vector.tensor_scalar(nbneg[:, g:g + 1], nball[:, g:g + 1],
                                            -1.0, None, AT.mult)
                    nc.vector.scalar_tensor_tensor(
                        acc[:], cds.pop(g)[:, 0, :OW], nbneg[:, g:g + 1], acc[:],
                        op0=AT.is_gt, op1=AT.add)

            for g in range(ITILES):
                cd = cdg.tile([128, 1, PROW2], BF16, tag="cd")
                nc.gpsimd.dma_gather(
                    cd[:], region, idxs[:, g * 8:(g + 1) * 8],
                    128, 128, PROW2, queue_num=g % 4)
                # bias = n_i - ave^2 : sign(cd_raw + bias) = sign(d^2 - ave^2)
                nc.vector.tensor_tensor(nball[:, g:g + 1], cd[:, 0, OW:OW + 1],
                                        nave2[:, g, c:c + 1], AT.add)
                cds[g] = cd
                if g >= 1:
                    _compare(g - 1)
            _compare(ITILES - 1)
            for k in range(6):
                ps = ssps.tile([1, 480], F32, tag="ss_ps", name="recps")
                nc.tensor.matmul(ps[:], ones_bf[:],
                                 acc[:, k * 480:(k + 1) * 480],
                                 start=True, stop=True)
                rc = rrp.tile([1, 480], F32, tag="recc")
                nc.scalar.activation(rc[:], ps[:], ACTF.Copy)
                nc.sync.dma_start(cc_in[c:c + 1, k * 480:(k + 1) * 480], rc[:])

    # ================= AllReduce rec =================
    if sim1:
        nc.sync.dma_start(cc_out[:, :], cc_in[:, :])
    else:
        nc.gpsimd.collective_compute(
            "AllReduce", mybir.AluOpType.add,
            replica_groups=[list(range(NCORES))],
            ins=[cc_in[:, :].opt()], outs=[cc_out[:, :].opt()])

    # ================= Phase 5: thr/mask =================
    with tc.tile_pool(name="thrp", bufs=2) as thrp, \
         tc.tile_pool(name="thrbig", bufs=1) as thrbig:
        rec = thrbig.tile([WAY, WAY - 1, S], F32, tag="rec")
        nc.sync.dma_start(rec[:], cc_out[:, :].rearrange("c (k s) -> c k s", k=WAY - 1))
        # per-class affine: sign classes hold sum(+-1); counts = (x+15360)/2
        nc.vector.tensor_scalar(rec[:], rec[:], cconst[:, 0:1], None, AT.add)
        nc.vector.tensor_scalar(rec[:], rec[:], cconst[:, 1:2], None, AT.mult)
        if dbg:
            with tc.tile_pool(name="dbgr", bufs=1) as dbgr:
                rg = dbgr.tile([WAY, OW], F32, tag="rg")
                nc.vector.tensor_copy(rg[:], rec[:].rearrange("c k s -> c (k s)"))
                nc.sync.dma_start(dbg["rec"].ap(), rg[:])
        rsum = thrp.tile([WAY, WAY - 1], F32, tag="rsum")
        nc.vector.tensor_reduce(rsum[:], rec[:], X, AT.add)
        gt0 = thrbig.tile([WAY, WAY - 1, S], F32, tag="gt0")
        nc.vector.tensor_scalar(gt0[:], rec[:], 0.5, None, AT.is_gt)
        nz = thrp.tile([WAY, WAY - 1], F32, tag="nz")
        nc.vector.tensor_reduce(nz[:], gt0[:], X, AT.add)
        nc.vector.tensor_scalar(nz[:], nz[:], 1.0, None, AT.max)
        thr = thrp.tile([WAY, WAY - 1], F32, tag="thr")
        nc.vector.reciprocal(thr[:], nz[:])
        nc.vector.tensor_tensor(thr[:], thr[:], rsum[:], AT.mult)
        mask_slots = thrbig.tile([WAY, WAY - 1, S], BF16, tag="mask_slots")
        nc.vector.tensor_tensor(
            mask_slots[:], rec[:],
            thr[:, :, None].to_broadcast((WAY, WAY - 1, S)), AT.is_lt)
        if dbg:
            with tc.tile_pool(name="dbgm", bufs=1) as dbgm:
                mg = dbgm.tile([WAY, WAY - 1, S], F32, tag="mg")
                nc.vector.tensor_copy(mg[:], mask_slots[:])
                nc.sync.dma_start(dbg["mask"].ap(), mg[:])
        msum = thrp.tile([WAY, 1], F32, tag="msum")
        nc.vector.tensor_reduce(msum[:], mask_slots[:].rearrange("c k s -> c (k s)"),
                                X, AT.add)
        nc.vector.tensor_scalar(msum[:], msum[:], 1.0, None, AT.max)
        scv = thrp.tile([WAY, 1], F32, tag="scv")
        nc.vector.reciprocal(scv[:], msum[:])
        nc.vector.tensor_scalar(scv[:], scv[:], 1.0 / (4.0 * T), None, AT.mult)
        nc.sync.dma_start(mask_dram[:, :, :S], mask_slots[:])

        # ============= Phase 6: contrast sums + finals =============
        with tc.tile_pool(name="p6", bufs=1) as p6, \
             tc.tile_pool(name="p6ps", bufs=1, space="PSUM") as p6ps:
            maskT = p6.tile([128, WAY * CTILES, WAY], BF16, tag="maskT")
            nc.vector.memset(maskT[:], 0.0)
            for c in range(WAY):
                if c > 0:
                    nc.sync.dma_start(
                        maskT[:, 0:c * CTILES, c],
                        _ap(mask_dram.tensor,
                            mask_dram.offset + c * (WAY - 1) * RROWS,
                            [(1, 128), (128, c * CTILES)]))
                if c < WAY - 1:
                    nc.sync.dma_start(
                        maskT[:, (c + 1) * CTILES:WAY * CTILES, c],
                        _ap(mask_dram.tensor,
                            mask_dram.offset + (c * (WAY - 1) + c) * RROWS,
                            [(1, 128), (128, (WAY - 1 - c) * CTILES)]))
            psC = [p6ps.tile([WAY, 480], F32, tag=f"ct_ps{qc}", name=f"ctps{qc}")
                   for qc in range(4)]
            for cr in range(WAY):
                for j in range(CTILES):
                    for qc in range(4):
                        nc.tensor.matmul(
                            psC[qc][:], maskT[:, cr * CTILES + j, :],
                            dT[cr][:, j, qc * 480:(qc + 1) * 480],
                            start=(cr == 0 and j == 0),
                            stop=(cr == WAY - 1 and j == CTILES - 1))
            ctrows = p6.tile([WAY, RHAT], F32, tag="ctrows")
            for qc in range(4):
                nc.scalar.activation(ctrows[:, qc * 480:(qc + 1) * 480], psC[qc][:],
                                     ACTF.Copy)
            if dbg:
                nc.sync.dma_start(dbg["ct"].ap(), ctrows[:])
            ctq = p6.tile([WAY, NQ], F32, tag="ctq")
            nc.vector.tensor_reduce(
                ctq[:], ctrows[:].rearrange("c (s q) -> c q s", q=NQ), X, AT.add)
            nc.vector.tensor_scalar(ctq[:], ctq[:], scv[:], None, AT.mult)

            if dbg:
                nc.sync.dma_start(dbg["dmax"].ap(), dmax_all[:])
                nc.sync.dma_start(dbg["nave2"].ap(), nave2[:])
                with tc.tile_pool(name="dbgp", bufs=1) as dbgp:
                    pf = dbgp.tile([128, ITILES, WAY], F32, tag="pf")
                    nc.vector.tensor_copy(pf[:], pos16[:])
                    nc.sync.dma_start(dbg["pos"].ap(), pf[:])

            ssum = p6.tile([WAY, NQ], F32, tag="ssum")
            nc.vector.tensor_tensor(ssum[:], dmq[:], ctq[:], AT.add)
            rcp = p6.tile([WAY, NQ], F32, tag="rcp")
            nc.vector.reciprocal(rcp[:], ssum[:])
            lg = p6.tile([WAY, NQ], F32, tag="lg")
            nc.vector.tensor_tensor(lg[:], dmq[:], rcp[:], AT.mult)
            nc.sync.dma_start(_ap(out_d, 0, [(1, WAY), (WAY, NQ)]), dmq[:])
            nc.sync.dma_start(_ap(out_d, NQ * WAY, [(1, WAY), (WAY, NQ)]), lg[:])

    dtp.release()
    persist.release()
    dram.release()


# ---------------- host side ----------------

def _sel_host():
    sel = np.zeros((ITILES, 128, NQ), np.float32)
    for i in range(R):
        sel[i // 128, i % 128, i % NQ] = 1.0
    return sel


def _prep_inputs(support_set, queries, support_labels, W, b):
    import ml_dtypes
    f8 = ml_dtypes.float8_e4m3fn
    support_set = np.asarray(support_set, dtype=np.float32)
    queries = np.asarray(queries, dtype=np.float32)
    labels = np.asarray(support_labels).astype(np.int64)
    W = np.asarray(W, dtype=np.float32)
    b = np.asarray(b, dtype=np.float32)
    assert not np.any(b), "kernel built without bias support (reference b==0)"
    order = np.argsort(labels, kind="stable")
    support_sorted = support_set[order]

    # wT [2, 8, 128, 2, 1152]: wT[half, kc2, p, h2, d] =
    #   64*W[d, half*2048 + kc2*256 + h2*128 + p]
    w8 = (W * 64.0).astype(f8)                     # [1152, 4096]
    wT = np.ascontiguousarray(
        w8.reshape(DOUT, 2, 8, 2, 128).transpose(1, 2, 4, 3, 0))

    s8 = support_sorted.astype(f8)                 # [80, 10, 2048]
    sd = np.ascontiguousarray(
        s8.reshape(80, SEQ_LEN, 16, 128).transpose(3, 2, 1, 0)
          .reshape(128, 16, SEQ_LEN * 80))
    q8 = queries.astype(f8)                        # [320, 10, 2048]
    sel = _sel_host()
    padv = np.zeros((128, 1), np.float32)
    padv[8:] = 1.0e15
    ident = np.eye(128).astype(ml_dtypes.bfloat16)
    cconst = np.zeros((WAY, 2), np.float32)
    for c in range(WAY):
        if c in SIGN_CLASSES:
            cconst[c] = (NCORES * RHAT, 0.5)
        else:
            cconst[c] = (0.0, 1.0)
    out = []
    for k in range(NCORES):
        qk = q8[k * NQ:(k + 1) * NQ]               # [40, 10, 2048]
        qd = np.ascontiguousarray(
            qk.reshape(NQ, SEQ_LEN, 16, 128).transpose(3, 2, 1, 0)
              .reshape(128, 16, SEQ_LEN * NQ))
        out.append({
            "qd": qd,
            "sd": sd,
            "wT": wT,
            "ident": ident,
            "sel": sel,
            "padv": padv,
            "cconst": cconst,
        })
    return out


def kernel(**inputs):
    per_core = _prep_inputs(**inputs)
    if "nc" not in _CACHE:
        _CACHE["nc"] = build(debug=bool(os.environ.get("BIMACL_DEBUG")))
    nc = _CACHE["nc"]
    res = run_bass_kernel_spmd(nc, per_core, core_ids=list(range(NCORES)))
    _CACHE["last_results"] = res
    full = np.concatenate([res.results[k]["out"] for k in range(NCORES)], axis=1)
    return np.ascontiguousarray(full.astype(np.float32))


# revision 27
# speedup vs baseline: 2.9341x; 1.0636x over previous
"""Trainium2 Bass kernel for nn_CNN_BiMACL_31860067401819 (retrieval_knn).

Self-contained: hardcodes all shapes/sharding. kernel(**inputs) accepts FULL
inputs keyed as in setup_inputs(), shards queries across 8 NeuronCores
(data-parallel over the query axis), and returns the FULL [2, 320, 5] f32
output. The only collective is an AllReduce of the per-class `rec` counts.

Design (v2):
- Frame-factorized embeddings: emb(tuple t=(f1,f2)) = relu(W1^T x_f1 +
  W2^T x_f2); per-frame half-products are computed once with fp8 DoubleRow
  matmuls, tuples assembled with bf16 adds + ACT relu into fp8 embeddings.
- All distance matmuls fp8 + DoubleRow (256-deep contraction per instr).
- SS (support-support) stays in d^2 space: psum = s_i.s_j - sn_j/2 (column
  norm folded in via a 1-row matmul), scaled by -2 on the psum->SBUF copy;
  row norm rides along as an extra gathered column. rec compare is then
  cd_raw + n_i > ave^2  <=>  d^2 > ave^2 (no sqrt for SS at all).
- D (query-support): psum = q.s - sn/2; ACT Sqrt(scale=-2, bias=qnorm)
  emits bf16 distances directly.
- rec compare+accumulate: ACT Sign for SIGN_CLASSES (affine-corrected after
  the AllReduce), fused DVE scalar_tensor_tensor (is_gt,add) for the rest.
- Phase-6 masked row sums via PE transpose of D (stored fp8) + mask matmuls.
"""
import os
from itertools import combinations

import numpy as np

import concourse.bass as bass
import concourse.tile as tile
from concourse import bacc, mybir
from concourse.bass_utils import run_bass_kernel_spmd

# ---- static problem config ----
WAY, SHOT, SEQ_LEN, TSS = 5, 16, 10, 2
DIN, DOUT = 2048, 1152
N_QUERIES = 320
T = 45
S = SHOT * T                 # 720 support tuples per class
SALL = WAY * S               # 3600
NCORES = 8
NQ = N_QUERIES // NCORES     # 40
R = NQ * T                   # 1800 valid rows/core
RHAT = 1920                  # 15*128 padded rows
ITILES = RHAT // 128         # 15
TUPLES = np.array(list(combinations(range(SEQ_LEN), TSS)), dtype=np.int32)
DC = DOUT // 128             # 9
NDR = 5                      # DoubleRow matmuls per padded 1280 contraction (5*256)
OW = (WAY - 1) * S           # 2880 other-class columns
PROW2 = 2944                 # per-class region row pitch (2880 data + norm + pad)
CTILES = 6                   # 128-row tiles per class region (768 rows)
RROWS = CTILES * 128         # 768
SIGN_CLASSES = (0, 1, 2, 3, 4)  # rec compare on ACT Sign

F32 = mybir.dt.float32
BF16 = mybir.dt.bfloat16
F8 = mybir.dt.float8e4
U32 = mybir.dt.uint32
I16 = mybir.dt.int16
DR = mybir.MatmulPerfMode.DoubleRow

_CACHE = {}


def _ap(tensor, offset, dims):
    return bass.AP(tensor=tensor, offset=offset, ap=[list(d) for d in dims])


def _chunks_for_class(c):
    """960-wide dst chunks over the 2880 other-class cols of class c, each
    split into <=480-wide matmul pieces (PSUM-bank limit).
    Returns list of (dst_off, [(src_col, dst_delta, width), ...])."""
    spans = []
    if c > 0:
        spans.append((0, 0, S * c))              # (dst0, src0, len)
    spans.append((S * c, S * (c + 1), OW - S * c))
    out = []
    for dst0 in range(0, OW, 960):
        pieces = []
        for sub in range(2):
            w0 = dst0 + sub * 480
            for sd, ss, ln in spans:
                lo = max(w0, sd)
                hi = min(w0 + 480, sd + ln)
                if lo < hi:
                    pieces.append((ss + (lo - sd), lo - dst0, hi - lo))
        out.append((dst0, pieces))
    return out


def build(debug=False, sim1=False):
    nc = bacc.Bacc(num_swdge_queues=4)
    qd_d = nc.dram_tensor("qd", [128, 16, SEQ_LEN * NQ], F8, kind="ExternalInput")
    sd_d = nc.dram_tensor("sd", [128, 16, SEQ_LEN * 80], F8, kind="ExternalInput")
    w_d = nc.dram_tensor("wT", [2, 8, 128, 2, DOUT], F8, kind="ExternalInput")
    id_d = nc.dram_tensor("ident", [128, 128], BF16, kind="ExternalInput")
    sel_d = nc.dram_tensor("sel", [ITILES, 128, NQ], F32, kind="ExternalInput")
    padv_d = nc.dram_tensor("padv", [128, 1], F32, kind="ExternalInput")
    cc_d = nc.dram_tensor("cconst", [WAY, 2], F32, kind="ExternalInput")
    out_d = nc.dram_tensor("out", [2, NQ, WAY], F32, kind="ExternalOutput")
    dbg = {}
    if debug:
        dbg["qemb"] = nc.dram_tensor("dbg_qemb", [128, DC, RHAT], F32, kind="ExternalOutput")
        dbg["semb"] = nc.dram_tensor("dbg_semb", [128, DC, SALL], F32, kind="ExternalOutput")
        dbg["snorm"] = nc.dram_tensor("dbg_snorm", [1, SALL], F32, kind="ExternalOutput")
        dbg["qnorm"] = nc.dram_tensor("dbg_qnorm", [128, ITILES], F32, kind="ExternalOutput")
        dbg["rec"] = nc.dram_tensor("dbg_rec", [WAY, OW], F32, kind="ExternalOutput")
        dbg["mask"] = nc.dram_tensor("dbg_mask", [WAY, WAY - 1, S], F32, kind="ExternalOutput")
        dbg["dmax"] = nc.dram_tensor("dbg_dmax", [128, ITILES, WAY], F32, kind="ExternalOutput")
        dbg["nave2"] = nc.dram_tensor("dbg_nave2", [128, ITILES, WAY], F32, kind="ExternalOutput")
        dbg["pos"] = nc.dram_tensor("dbg_pos", [128, ITILES, WAY], F32, kind="ExternalOutput")
        dbg["ct"] = nc.dram_tensor("dbg_ct", [WAY, RHAT], F32, kind="ExternalOutput")

    with tile.TileContext(nc) as tc:
        _body(nc, tc, qd_d, sd_d, w_d, id_d, sel_d, padv_d, cc_d, out_d, dbg, sim1)
    nc.finalize()
    return nc


def _body(nc, tc, qd_d, sd_d, w_d, id_d, sel_d, padv_d, cc_d, out_d, dbg, sim1):
    AT = mybir.AluOpType
    ACTF = mybir.ActivationFunctionType
    X = mybir.AxisListType.X

    persist = tc.alloc_tile_pool(name="persist", bufs=1)
    dram = tc.alloc_tile_pool(name="dram", bufs=1, space="DRAM")

    # DRAM scratch
    p_dram = dram.tile([WAY, RROWS, PROW2], BF16, tag="p_scratch")
    posw_dram = dram.tile([WAY, 16, ITILES * 8], I16, tag="posw")
    snorm_dram = dram.tile([1, 3840], F32, tag="snormd")
    mask_dram = dram.tile([WAY, WAY - 1, RROWS], BF16, tag="maskd")
    cc_in = dram.tile([WAY, OW], F32, tag="cc_in")
    cc_out = dram.tile([WAY, OW], F32, tag="cc_out")

    # persistent SBUF
    q_embT = persist.tile([128, DC + 1, RHAT], F8, tag="q_embT")
    s_embT = persist.tile([128, DC + 1, SALL], F8, tag="s_embT")
    snh = persist.tile([1, SALL], BF16, tag="snh")          # -snorm/2
    qnorm = persist.tile([128, ITILES], F32, tag="qnorm")
    pnorm = persist.tile([128, WAY * CTILES], F32, tag="pnorm")
    m16a = persist.tile([128, ITILES, 16], F32, tag="m16a")
    dmax_all = persist.tile([128, ITILES, WAY], F32, tag="dmax_all")
    nave2 = persist.tile([128, ITILES, WAY], F32, tag="nave2")
    pos16 = persist.tile([128, ITILES, WAY], I16, tag="pos16")
    ident = persist.tile([128, 128], BF16, tag="ident")
    ones_bf = persist.tile([128, 1], BF16, tag="ones_bf")
    ones_f = persist.tile([128, 1], F32, tag="ones_f")
    ones_row = persist.tile([1, 128], BF16, tag="ones_row")
    padv = persist.tile([128, 1], F32, tag="padv")
    cconst = persist.tile([WAY, 2], F32, tag="cconst")
    sel_sb = persist.tile([128, ITILES, NQ], F32, tag="sel_sb")
    dmq = persist.tile([WAY, NQ], F32, tag="dmq")

    nc.vector.memset(ones_bf[:], 1.0)
    nc.vector.memset(ones_f[:], 1.0)
    nc.vector.memset(ones_row[:], 1.0)
    nc.sync.dma_start(padv[:], padv_d[:, :])
    nc.sync.dma_start(ident[:], id_d[:, :])
    nc.sync.dma_start(cconst[:], cc_d[:, :])
    nc.sync.dma_start(sel_sb[:], sel_d.rearrange("t p q -> p t q"))
    nc.vector.memset(q_embT[:, :, R:RHAT], 0.0)
    nc.vector.memset(q_embT[:, DC], 0.0)
    nc.vector.memset(s_embT[:, DC], 0.0)

    # ================= Phase 1: per-frame half products + tuple assembly ====
    with tc.tile_pool(name="emb", bufs=1) as emb, \
         tc.tile_pool(name="embsm", bufs=3) as embsm, \
         tc.tile_pool(name="embps", bufs=4, space="PSUM") as embps:
        wT = emb.tile([128, 2, 8, 2, DOUT], F8, tag="wT")
        nc.sync.dma_start(wT[:], w_d.rearrange("a b p c d -> p a b c d"))
        qd = emb.tile([128, 16, SEQ_LEN * NQ], F8, tag="qd")
        nc.sync.dma_start(qd[:], qd_d[:, :, :])
        sd = emb.tile([128, 16, SEQ_LEN * 80], F8, tag="sd")
        nc.sync.dma_start(sd[:], sd_d[:, :, :])
        Pq = emb.tile([128, DC, 2, SEQ_LEN * NQ], BF16, tag="Pq")
        Ps = emb.tile([128, DC, 2, SEQ_LEN * 80], BF16, tag="Ps")

        # s-side first: its embeddings gate snorm -> SS -> gathers
        for half in range(2):
            for dc in range(DC):
                for ch in range(2):
                    ps2 = embps.tile([128, SEQ_LEN * NQ], F32, tag="emb_ps")
                    for kc2 in range(8):
                        nc.tensor.matmul(
                            ps2[:], wT[:, half, kc2, :, dc * 128:(dc + 1) * 128],
                            sd[:, 2 * kc2:2 * kc2 + 2, ch * 400:(ch + 1) * 400],
                            start=(kc2 == 0), stop=(kc2 == 7), perf_mode=DR)
                    nc.vector.tensor_copy(
                        Ps[:, dc, half, ch * 400:(ch + 1) * 400], ps2[:])
        for t in range(T):
            f1, f2 = int(TUPLES[t][0]), int(TUPLES[t][1])
            pres = embsm.tile([128, DC, 80], BF16, tag="pres")
            nc.vector.tensor_tensor(
                pres[:], Ps[:, :, 0, f1 * 80:(f1 + 1) * 80],
                Ps[:, :, 1, f2 * 80:(f2 + 1) * 80], AT.add)
            dst = s_embT[:, :DC].rearrange("p d (u t) -> p d t u", t=T)[:, :, t]
            if t % 2 == 0:
                nc.scalar.activation(dst, pres[:], ACTF.Relu, scale=1.0 / 64.0)
            else:
                nc.vector.tensor_scalar(dst, pres[:], 0.0, 1.0 / 64.0,
                                        AT.max, op1=AT.mult)
        # q side
        for half in range(2):
            for dc in range(DC):
                ps = embps.tile([128, SEQ_LEN * NQ], F32, tag="emb_ps")
                for kc2 in range(8):
                    nc.tensor.matmul(
                        ps[:], wT[:, half, kc2, :, dc * 128:(dc + 1) * 128],
                        qd[:, 2 * kc2:2 * kc2 + 2, :],
                        start=(kc2 == 0), stop=(kc2 == 7), perf_mode=DR)
                nc.vector.tensor_copy(Pq[:, dc, half], ps[:])
        for t in range(T):
            f1, f2 = int(TUPLES[t][0]), int(TUPLES[t][1])
            preq = embsm.tile([128, DC, NQ], BF16, tag="preq")
            nc.vector.tensor_tensor(
                preq[:], Pq[:, :, 0, f1 * NQ:(f1 + 1) * NQ],
                Pq[:, :, 1, f2 * NQ:(f2 + 1) * NQ], AT.add)
            if t % 2 == 0:
                nc.scalar.activation(q_embT[:, :DC, t * NQ:(t + 1) * NQ], preq[:],
                                     ACTF.Relu, scale=1.0 / 64.0)
            else:
                nc.vector.tensor_scalar(q_embT[:, :DC, t * NQ:(t + 1) * NQ],
                                        preq[:], 0.0, 1.0 / 64.0,
                                        AT.max, op1=AT.mult)

    # dT allocated after the emb pool frees wT/Pq/Ps space
    dtp = tc.alloc_tile_pool(name="dtp", bufs=1)
    dT = [dtp.tile([128, CTILES, RHAT], F8, tag=f"dT{c}", name=f"dT{c}")
          for c in range(WAY)]

    if dbg:
        with tc.tile_pool(name="dbge", bufs=1) as dbge:
            t1 = dbge.tile([128, DC, RHAT], F32, tag="dbq")
            nc.vector.tensor_copy(t1[:], q_embT[:, :DC])
            nc.sync.dma_start(dbg["qemb"].ap(), t1[:])
            t2 = dbge.tile([128, DC, SALL], F32, tag="dbs")
            nc.vector.tensor_copy(t2[:], s_embT[:, :DC])
            nc.sync.dma_start(dbg["semb"].ap(), t2[:])

    # ================= Phase 2: norms =================
    with tc.tile_pool(name="nrm", bufs=2) as nrm, \
         tc.tile_pool(name="nrmps", bufs=2, space="PSUM") as nrmps:
        snrow = nrm.tile([1, SALL], F32, tag="snrow")
        for scn in range(8):
            ps = nrmps.tile([1, 450], F32, tag="sn_ps")
            for dc in range(DC):
                sq = nrm.tile([128, 450], BF16, tag="sn_sqb")
                nc.scalar.activation(sq[:], s_embT[:, dc, scn * 450:(scn + 1) * 450],
                                     ACTF.Square)
                nc.tensor.matmul(ps[:], ones_bf[:], sq[:],
                                 start=(dc == 0), stop=(dc == DC - 1))
            nc.scalar.activation(snrow[:, scn * 450:(scn + 1) * 450], ps[:], ACTF.Copy)
        nc.vector.tensor_scalar(snh[:], snrow[:], -0.5, None, AT.mult)
        nc.sync.dma_start(snorm_dram[:, :SALL], snrow[:])
        # pnorm[p, c*6+j] = snorm[720c + 128j + p]
        for c in range(WAY):
            nc.sync.dma_start(
                pnorm[:, c * CTILES:(c + 1) * CTILES],
                _ap(snorm_dram.tensor, snorm_dram.offset + c * S,
                    [(1, 128), (128, CTILES)]))
        for it in range(ITILES):
            ps = nrmps.tile([128, 1], F32, tag="qn_ps", name="qnps")
            sqa = nrm.tile([128, DC, 128], BF16, tag="qn_sqb")
            qb = nrm.tile([128, DC, 128], BF16, tag="qn_qb")
            nc.vector.tensor_copy(qb[:], q_embT[:, :DC, it * 128:(it + 1) * 128])
            nc.vector.tensor_tensor(sqa[:], qb[:], qb[:], AT.mult)
            for dc in range(DC):
                nc.tensor.matmul(ps[:], sqa[:, dc], ones_bf[:],
                                 start=(dc == 0), stop=(dc == DC - 1))
            nc.vector.tensor_copy(qnorm[:, it:it + 1], ps[:])
        if dbg:
            nc.sync.dma_start(dbg["snorm"].ap(), snrow[:])
            nc.sync.dma_start(dbg["qnorm"].ap(), qnorm[:])

    # ====== Phases 3+4: per class, SS slab -> D tiles -> gather/rec ======
    with tc.tile_pool(name="ssst", bufs=1) as ssst, \
         tc.tile_pool(name="ssps", bufs=2, space="PSUM") as ssps, \
         tc.tile_pool(name="dph", bufs=3) as dph, \
         tc.tile_pool(name="dsm", bufs=4) as dsm, \
         tc.tile_pool(name="dps", bufs=2, space="PSUM") as dps, \
         tc.tile_pool(name="tps", bufs=2, space="PSUM") as tps, \
         tc.tile_pool(name="cdp", bufs=2) as cdp, \
         tc.tile_pool(name="accp", bufs=2) as accp, \
         tc.tile_pool(name="rrp", bufs=2) as rrp, \
         tc.tile_pool(name="cmpp", bufs=2) as cmpp, \
         tc.tile_pool(name="cdg", bufs=3) as cdg:
        for c in range(WAY):
            # ---- SS slab for class c ----
            staging = ssst.tile([128, CTILES, PROW2], BF16, tag="ss_stage")
            chunks = _chunks_for_class(c)
            for j in range(CTILES):
                p0 = S * c + 128 * j
                pw = min(128, S - 128 * j)
                for (dst0, pieces) in chunks:
                    ps = ssps.tile([128, 960], F32, tag="ss_ps")
                    for (src0, doff, w) in pieces:
                        for dc2 in range(NDR):
                            nc.tensor.matmul(
                                ps[:pw, doff:doff + w],
                                s_embT[:, 2 * dc2:2 * dc2 + 2, p0:p0 + pw],
                                s_embT[:, 2 * dc2:2 * dc2 + 2, src0:src0 + w],
                                start=(dc2 == 0), stop=False, perf_mode=DR)
                        nc.tensor.matmul(
                            ps[:pw, doff:doff + w], ones_row[:, :pw],
                            snh[:, src0:src0 + w], start=False, stop=True)
                    nc.scalar.activation(staging[:pw, j, dst0:dst0 + 960],
                                         ps[:pw], ACTF.Copy, scale=-2.0)
            # row-norm column (col 2880) for the gathered threshold
            nc.vector.tensor_copy(staging[:, :, OW:OW + 1],
                                  pnorm[:, c * CTILES:(c + 1) * CTILES, None])
            # write region, skipping the undefined pad rows of the last tile
            nc.sync.dma_start(
                _ap(p_dram.tensor, p_dram.offset + c * RROWS * PROW2,
                    [(PROW2, 128), (128 * PROW2, CTILES - 1), (1, PROW2)]),
                staging[:, :CTILES - 1])
            nc.sync.dma_start(
                _ap(p_dram.tensor,
                    p_dram.offset + (c * RROWS + (CTILES - 1) * 128) * PROW2,
                    [(PROW2, 80), (1, PROW2)]),
                staging[:80, CTILES - 1])
            # ---- D tiles ----
            for it in range(ITILES):
                d_bf = dph.tile([128, RROWS], BF16, tag="d_bf")
                for sc in range(2):
                    ps = dps.tile([128, 360], F32, tag="d_ps", name="dps")
                    s0 = c * S + sc * 360
                    for dc2 in range(NDR):
                        nc.tensor.matmul(
                            ps[:], q_embT[:, 2 * dc2:2 * dc2 + 2, it * 128:(it + 1) * 128],
                            s_embT[:, 2 * dc2:2 * dc2 + 2, s0:s0 + 360],
                            start=(dc2 == 0), stop=False, perf_mode=DR)
                    nc.tensor.matmul(ps[:], ones_row[:], snh[:, s0:s0 + 360],
                                     start=False, stop=True)
                    nc.scalar.activation(d_bf[:, sc * 360:(sc + 1) * 360], ps[:],
                                         ACTF.Sqrt, bias=qnorm[:, it:it + 1],
                                         scale=-2.0)
                nc.vector.memset(d_bf[:, S:RROWS], 0.0)
                # reductions
                nc.vector.tensor_reduce(
                    m16a[:, it], d_bf[:, :S].rearrange("p (a b) -> p b a", b=16),
                    X, AT.max)
                nc.vector.tensor_reduce(dmax_all[:, it, c:c + 1], m16a[:, it],
                                        X, AT.max)
                mx8 = dsm.tile([128, 8], F32, tag="mx8")
                ix8 = dsm.tile([128, 8], U32, tag="ix8")
                nc.vector.tensor_copy(
                    mx8[:], dmax_all[:, it, c:c + 1].to_broadcast((128, 8)))
                nc.vector.max_index(ix8[:], mx8[:], d_bf[:, :S])
                posf = dsm.tile([128, 1], F32, tag="posf")
                nc.vector.tensor_scalar(posf[:], ix8[:, 0:1], 0.0, None, AT.add)
                nc.vector.tensor_copy(pos16[:, it, c:c + 1], posf[:])
                # transpose into dT (fp8)
                psT = tps.tile([128, CTILES * 128], BF16, tag="psT")
                for j in range(CTILES):
                    nc.tensor.matmul(psT[:, j * 128:(j + 1) * 128],
                                     d_bf[:, j * 128:(j + 1) * 128], ident[:],
                                     start=True, stop=True, is_transpose=True)
                if it % 2 == 0:
                    nc.vector.tensor_copy(
                        dT[c][:, :, it * 128:(it + 1) * 128],
                        psT[:].rearrange("p (j q) -> p j q", j=CTILES))
                else:
                    nc.scalar.activation(
                        dT[c][:, :, it * 128:(it + 1) * 128],
                        psT[:].rearrange("p (j q) -> p j q", j=CTILES), ACTF.Copy)
            nc.vector.memset(dT[c][:, :, R:RHAT], 0.0)
            # batched per-class stats: nave2 = -(asum/16)^2 ; dmax
            asum = dsm.tile([128, ITILES], F32, tag="asum")
            nc.vector.tensor_reduce(asum[:], m16a[:], X, AT.add)
            nc.vector.tensor_scalar(asum[:, ITILES - 1:ITILES],
                                    asum[:, ITILES - 1:ITILES], padv[:], None, AT.add)
            nc.vector.tensor_tensor(asum[:], asum[:], asum[:], AT.mult)
            nc.vector.tensor_scalar(nave2[:, :, c], asum[:], -1.0 / 256.0, None,
                                    AT.mult)
            if c == WAY - 1:
                psD = ssps.tile([WAY, NQ], F32, tag="ss_ps", name="dmps")
                for it in range(ITILES):
                    nc.tensor.matmul(psD[:], dmax_all[:, it, :], sel_sb[:, it],
                                     start=(it == 0), stop=(it == ITILES - 1))
                nc.scalar.activation(dmq[:], psD[:], ACTF.Copy, scale=1.0 / T)

            # ---- gather + rec ----
            nc.sync.dma_start(
                _ap(posw_dram.tensor, posw_dram.offset + c * 16 * ITILES * 8,
                    [(1, 8), (ITILES * 8, 16), (8, ITILES)]),
                pos16[:, :, c])
            idxs = cdp.tile([128, ITILES * 8], I16, tag="idxs")
            nc.sync.dma_start(
                idxs[:],
                _ap(posw_dram.tensor, posw_dram.offset + c * 16 * ITILES * 8,
                    [(0, 8), (ITILES * 8, 16), (1, ITILES * 8)]))
            use_sign = c in SIGN_CLASSES
            acc = accp.tile([128, OW], BF16, tag="accb")
            if not use_sign:
                nc.vector.memset(acc[:], 0.0)
                nbneg = cdp.tile([128, ITILES], F32, tag="nbneg")
            region = _ap(p_dram.tensor, p_dram.offset + c * RROWS * PROW2,
                         [(PROW2, RROWS), (1, PROW2)])
            # software-pipelined: gather(g)/nbias(g) run one stage ahead of
            # sign(g)/add(g) so the ACT sign stream never waits on DVE.
            nball = cdp.tile([128, ITILES], F32, tag="nball")
            cds = {}

            def _compare(g):
                if use_sign:
                    cmp = cmpp.tile([128, OW], BF16, tag="cmp")
                    nc.scalar.activation(cmp[:], cds.pop(g)[:, 0, :OW], ACTF.Sign,
                                         bias=nball[:, g:g + 1])
                    if g == 0:
                        nc.vector.tensor_copy(acc[:], cmp[:])
                    else:
                        nc.vector.tensor_tensor(acc[:], acc[:], cmp[:], AT.add)
                else:
                    nc.vector.tensor_scalar(nbneg[:, g:g + 1], nball[:, g:g + 1],
                                            -1.0, None, AT.mult)
                    nc.vector.scalar_tensor_tensor(
                        acc[:], cds.pop(g)[:, 0, :OW], nbneg[:, g:g + 1], acc[:],
                        op0=AT.is_gt, op1=AT.add)

            for g in range(ITILES):
                cd = cdg.tile([128, 1, PROW2], BF16, tag="cd")
                nc.gpsimd.dma_gather(
                    cd[:], region, idxs[:, g * 8:(g + 1) * 8],
                    128, 128, PROW2, queue_num=g % 4)
                # bias = n_i - ave^2 : sign(cd_raw + bias) = sign(d^2 - ave^2)
                nc.vector.tensor_tensor(nball[:, g:g + 1], cd[:, 0, OW:OW + 1],
                                        nave2[:, g, c:c + 1], AT.add)
                cds[g] = cd
                if g >= 1:
                    _compare(g - 1)
            _compare(ITILES - 1)
            for k in range(6):
                ps = ssps.tile([1, 480], F32, tag="ss_ps", name="recps")
                nc.tensor.matmul(ps[:], ones_bf[:],
                                 acc[:, k * 480:(k + 1) * 480],
                                 start=True, stop=True)
                rc = rrp.tile([1, 480], F32, tag="recc")
                nc.scalar.activation(rc[:], ps[:], ACTF.Copy)
                nc.sync.dma_start(cc_in[c:c + 1, k * 480:(k + 1) * 480], rc[:])

    # ================= AllReduce rec =================
    if sim1:
        nc.sync.dma_start(cc_out[:, :], cc_in[:, :])
    else:
        nc.gpsimd.collective_compute(
            "AllReduce", mybir.AluOpType.add,
            replica_groups=[list(range(NCORES))],
            ins=[cc_in[:, :].opt()], outs=[cc_out[:, :].opt()])

    # ================= Phase 5: thr/mask =================
    with tc.tile_pool(name="thrp", bufs=2) as thrp, \
         tc.tile_pool(name="thrbig", bufs=1) as thrbig:
        rec = thrbig.tile([WAY, WAY - 1, S], F32, tag="rec")
        nc.sync.dma_start(rec[:], cc_out[:, :].rearrange("c (k s) -> c k s", k=WAY - 1))
        # per-class affine: sign classes hold sum(+-1); counts = (x+15360)/2
        nc.vector.tensor_scalar(rec[:], rec[:], cconst[:, 0:1], None, AT.add)
        nc.vector.tensor_scalar(rec[:], rec[:], cconst[:, 1:2], None, AT.mult)
        if dbg:
            with tc.tile_pool(name="dbgr", bufs=1) as dbgr:
                rg = dbgr.tile([WAY, OW], F32, tag="rg")
                nc.vector.tensor_copy(rg[:], rec[:].rearrange("c k s -> c (k s)"))
                nc.sync.dma_start(dbg["rec"].ap(), rg[:])
        rsum = thrp.tile([WAY, WAY - 1], F32, tag="rsum")
        nc.vector.tensor_reduce(rsum[:], rec[:], X, AT.add)
        gt0 = thrbig.tile([WAY, WAY - 1, S], F32, tag="gt0")
        nc.vector.tensor_scalar(gt0[:], rec[:], 0.5, None, AT.is_gt)
        nz = thrp.tile([WAY, WAY - 1], F32, tag="nz")
        nc.vector.tensor_reduce(nz[:], gt0[:], X, AT.add)
        nc.vector.tensor_scalar(nz[:], nz[:], 1.0, None, AT.max)
        thr = thrp.tile([WAY, WAY - 1], F32, tag="thr")
        nc.vector.reciprocal(thr[:], nz[:])
        nc.vector.tensor_tensor(thr[:], thr[:], rsum[:], AT.mult)
        mask_slots = thrbig.tile([WAY, WAY - 1, S], BF16, tag="mask_slots")
        nc.vector.tensor_tensor(
            mask_slots[:], rec[:],
            thr[:, :, None].to_broadcast((WAY, WAY - 1, S)), AT.is_lt)
        if dbg:
            with tc.tile_pool(name="dbgm", bufs=1) as dbgm:
                mg = dbgm.tile([WAY, WAY - 1, S], F32, tag="mg")
                nc.vector.tensor_copy(mg[:], mask_slots[:])
                nc.sync.dma_start(dbg["mask"].ap(), mg[:])
        msum = thrp.tile([WAY, 1], F32, tag="msum")
        nc.vector.tensor_reduce(msum[:], mask_slots[:].rearrange("c k s -> c (k s)"),
                                X, AT.add)
        nc.vector.tensor_scalar(msum[:], msum[:], 1.0, None, AT.max)
        scv = thrp.tile([WAY, 1], F32, tag="scv")
        nc.vector.reciprocal(scv[:], msum[:])
        nc.vector.tensor_scalar(scv[:], scv[:], 1.0 / (4.0 * T), None, AT.mult)
        nc.sync.dma_start(mask_dram[:, :, :S], mask_slots[:])

        # ============= Phase 6: contrast sums + finals =============
        with tc.tile_pool(name="p6", bufs=1) as p6, \
             tc.tile_pool(name="p6ps", bufs=1, space="PSUM") as p6ps:
            maskT = p6.tile([128, WAY * CTILES, WAY], BF16, tag="maskT")
            nc.vector.memset(maskT[:], 0.0)
            for c in range(WAY):
                if c > 0:
                    nc.sync.dma_start(
                        maskT[:, 0:c * CTILES, c],
                        _ap(mask_dram.tensor,
                            mask_dram.offset + c * (WAY - 1) * RROWS,
                            [(1, 128), (128, c * CTILES)]))
                if c < WAY - 1:
                    nc.sync.dma_start(
                        maskT[:, (c + 1) * CTILES:WAY * CTILES, c],
                        _ap(mask_dram.tensor,
                            mask_dram.offset + (c * (WAY - 1) + c) * RROWS,
                            [(1, 128), (128, (WAY - 1 - c) * CTILES)]))
            psC = [p6ps.tile([WAY, 480], F32, tag=f"ct_ps{qc}", name=f"ctps{qc}")
                   for qc in range(4)]
            for cr in range(WAY):
                for j in range(CTILES):
                    for qc in range(4):
                        nc.tensor.matmul(
                            psC[qc][:], maskT[:, cr * CTILES + j, :],
                            dT[cr][:, j, qc * 480:(qc + 1) * 480],
                            start=(cr == 0 and j == 0),
                            stop=(cr == WAY - 1 and j == CTILES - 1))
            ctrows = p6.tile([WAY, RHAT], F32, tag="ctrows")
            for qc in range(4):
                nc.scalar.activation(ctrows[:, qc * 480:(qc + 1) * 480], psC[qc][:],
                                     ACTF.Copy)
            if dbg:
                nc.sync.dma_start(dbg["ct"].ap(), ctrows[:])
            ctq = p6.tile([WAY, NQ], F32, tag="ctq")
            nc.vector.tensor_reduce(
                ctq[:], ctrows[:].rearrange("c (s q) -> c q s", q=NQ), X, AT.add)
            nc.vector.tensor_scalar(ctq[:], ctq[:], scv[:], None, AT.mult)

            if dbg:
                nc.sync.dma_start(dbg["dmax"].ap(), dmax_all[:])
                nc.sync.dma_start(dbg["nave2"].ap(), nave2[:])
                with tc.tile_pool(name="dbgp", bufs=1) as dbgp:
                    pf = dbgp.tile([128, ITILES, WAY], F32, tag="pf")
                    nc.vector.tensor_copy(pf[:], pos16[:])
                    nc.sync.dma_start(dbg["pos"].ap(), pf[:])

            ssum = p6.tile([WAY, NQ], F32, tag="ssum")
            nc.vector.tensor_tensor(ssum[:], dmq[:], ctq[:], AT.add)
            rcp = p6.tile([WAY, NQ], F32, tag="rcp")
            nc.vector.reciprocal(rcp[:], ssum[:])
            lg = p6.tile([WAY, NQ], F32, tag="lg")
            nc.vector.tensor_tensor(lg[:], dmq[:], rcp[:], AT.mult)
            nc.sync.dma_start(_ap(out_d, 0, [(1, WAY), (WAY, NQ)]), dmq[:])
            nc.sync.dma_start(_ap(out_d, NQ * WAY, [(1, WAY), (WAY, NQ)]), lg[:])

    dtp.release()
    persist.release()
    dram.release()


# ---------------- host side ----------------

def _sel_host():
    sel = np.zeros((ITILES, 128, NQ), np.float32)
    for i in range(R):
        sel[i // 128, i % 128, i % NQ] = 1.0
    return sel


def _prep_inputs(support_set, queries, support_labels, W, b):
    import ml_dtypes
    f8 = ml_dtypes.float8_e4m3fn
    support_set = np.asarray(support_set, dtype=np.float32)
    queries = np.asarray(queries, dtype=np.float32)
    labels = np.asarray(support_labels).astype(np.int64)
    W = np.asarray(W, dtype=np.float32)
    b = np.asarray(b, dtype=np.float32)
    assert not np.any(b), "kernel built without bias support (reference b==0)"
    order = np.argsort(labels, kind="stable")
    support_sorted = support_set[order]

    # wT [2, 8, 128, 2, 1152]: wT[half, kc2, p, h2, d] =
    #   64*W[d, half*2048 + kc2*256 + h2*128 + p]
    w8 = (W * 64.0).astype(f8)                     # [1152, 4096]
    wT = np.ascontiguousarray(
        w8.reshape(DOUT, 2, 8, 2, 128).transpose(1, 2, 4, 3, 0))

    s8 = support_sorted.astype(f8)                 # [80, 10, 2048]
    sd = np.ascontiguousarray(
        s8.reshape(80, SEQ_LEN, 16, 128).transpose(3, 2, 1, 0)
          .reshape(128, 16, SEQ_LEN * 80))
    q8 = queries.astype(f8)                        # [320, 10, 2048]
    sel = _sel_host()
    padv = np.zeros((128, 1), np.float32)
    padv[8:] = 1.0e15
    ident = np.eye(128).astype(ml_dtypes.bfloat16)
    cconst = np.zeros((WAY, 2), np.float32)
    for c in range(WAY):
        if c in SIGN_CLASSES:
            cconst[c] = (NCORES * RHAT, 0.5)
        else:
            cconst[c] = (0.0, 1.0)
    out = []
    for k in range(NCORES):
        qk = q8[k * NQ:(k + 1) * NQ]               # [40, 10, 2048]
        qd = np.ascontiguousarray(
            qk.reshape(NQ, SEQ_LEN, 16, 128).transpose(3, 2, 1, 0)
              .reshape(128, 16, SEQ_LEN * NQ))
        out.append({
            "qd": qd,
            "sd": sd,
            "wT": wT,
            "ident": ident,
            "sel": sel,
            "padv": padv,
            "cconst": cconst,
        })
    return out


def kernel(**inputs):
    per_core = _prep_inputs(**inputs)
    if "nc" not in _CACHE:
        _CACHE["nc"] = build(debug=bool(os.environ.get("BIMACL_DEBUG")))
    nc = _CACHE["nc"]
    res = run_bass_kernel_spmd(nc, per_core, core_ids=list(range(NCORES)))
    _CACHE["last_results"] = res
    full = np.concatenate([res.results[k]["out"] for k in range(NCORES)], axis=1)
    return np.ascontiguousarray(full.astype(np.float32))


# revision 31
# speedup vs baseline: 3.0039x; 1.0238x over previous
"""Trainium2 Bass kernel for nn_CNN_BiMACL_31860067401819 (retrieval_knn).

Self-contained: hardcodes all shapes/sharding. kernel(**inputs) accepts FULL
inputs keyed as in setup_inputs(), shards queries across 8 NeuronCores
(data-parallel over the query axis), and returns the FULL [2, 320, 5] f32
output. The only collective is an AllReduce of the per-class `rec` counts.

Design (v2):
- Frame-factorized embeddings: emb(tuple t=(f1,f2)) = relu(W1^T x_f1 +
  W2^T x_f2); per-frame half-products are computed once with fp8 DoubleRow
  matmuls, tuples assembled with bf16 adds + ACT relu into fp8 embeddings.
- All distance matmuls fp8 + DoubleRow (256-deep contraction per instr).
- SS (support-support) stays in d^2 space: psum = s_i.s_j - sn_j/2 (column
  norm folded in via a 1-row matmul), scaled by -2 on the psum->SBUF copy;
  row norm rides along as an extra gathered column. rec compare is then
  cd_raw + n_i > ave^2  <=>  d^2 > ave^2 (no sqrt for SS at all).
- D (query-support): psum = q.s - sn/2; ACT Sqrt(scale=-2, bias=qnorm)
  emits bf16 distances directly.
- rec compare+accumulate: ACT Sign for SIGN_CLASSES (affine-corrected after
  the AllReduce), fused DVE scalar_tensor_tensor (is_gt,add) for the rest.
- Phase-6 masked row sums via PE transpose of D (stored fp8) + mask matmuls.
"""
import os
from itertools import combinations

import numpy as np

import concourse.bass as bass
import concourse.tile as tile
from concourse import bacc, mybir
from concourse.bass_utils import run_bass_kernel_spmd

# ---- static problem config ----
WAY, SHOT, SEQ_LEN, TSS = 5, 16, 10, 2
DIN, DOUT = 2048, 1152
N_QUERIES = 320
T = 45
S = SHOT * T                 # 720 support tuples per class
SALL = WAY * S               # 3600
NCORES = 8
NQ = N_QUERIES // NCORES     # 40
R = NQ * T                   # 1800 valid rows/core
RHAT = 1920                  # 15*128 padded rows
ITILES = RHAT // 128         # 15
TUPLES = np.array(list(combinations(range(SEQ_LEN), TSS)), dtype=np.int32)
DC = DOUT // 128             # 9
NDR = 5                      # DoubleRow matmuls per padded 1280 contraction (5*256)
OW = (WAY - 1) * S           # 2880 other-class columns
PROW2 = 2944                 # per-class region row pitch (2880 data + norm + pad)
CTILES = 6                   # 128-row tiles per class region (768 rows)
RROWS = CTILES * 128         # 768
SIGN_CLASSES = (0, 1, 2, 3, 4)  # rec compare on ACT Sign

F32 = mybir.dt.float32
BF16 = mybir.dt.bfloat16
F8 = mybir.dt.float8e4
U32 = mybir.dt.uint32
I16 = mybir.dt.int16
DR = mybir.MatmulPerfMode.DoubleRow

_CACHE = {}


def _ap(tensor, offset, dims):
    return bass.AP(tensor=tensor, offset=offset, ap=[list(d) for d in dims])


def _chunks_for_class(c):
    """960-wide dst chunks over the 2880 other-class cols of class c, each
    split into <=480-wide matmul pieces (PSUM-bank limit).
    Returns list of (dst_off, [(src_col, dst_delta, width), ...])."""
    spans = []
    if c > 0:
        spans.append((0, 0, S * c))              # (dst0, src0, len)
    spans.append((S * c, S * (c + 1), OW - S * c))
    out = []
    for dst0 in range(0, OW, 960):
        pieces = []
        for sub in range(2):
            w0 = dst0 + sub * 480
            for sd, ss, ln in spans:
                lo = max(w0, sd)
                hi = min(w0 + 480, sd + ln)
                if lo < hi:
                    pieces.append((ss + (lo - sd), lo - dst0, hi - lo))
        out.append((dst0, pieces))
    return out


def build(debug=False, sim1=False):
    nc = bacc.Bacc(num_swdge_queues=4)
    qd_d = nc.dram_tensor("qd", [128, 16, SEQ_LEN * NQ], F8, kind="ExternalInput")
    sd_d = nc.dram_tensor("sd", [128, 16, SEQ_LEN * 80], F8, kind="ExternalInput")
    w_d = nc.dram_tensor("wT", [2, 8, 128, 2, DOUT], F8, kind="ExternalInput")
    id_d = nc.dram_tensor("ident", [128, 128], BF16, kind="ExternalInput")
    sel_d = nc.dram_tensor("sel", [ITILES, 128, NQ], F32, kind="ExternalInput")
    padv_d = nc.dram_tensor("padv", [128, 1], F32, kind="ExternalInput")
    cc_d = nc.dram_tensor("cconst", [WAY, 2], F32, kind="ExternalInput")
    out_d = nc.dram_tensor("out", [2, NQ, WAY], F32, kind="ExternalOutput")
    dbg = {}
    if debug:
        dbg["qemb"] = nc.dram_tensor("dbg_qemb", [128, DC, RHAT], F32, kind="ExternalOutput")
        dbg["semb"] = nc.dram_tensor("dbg_semb", [128, DC, SALL], F32, kind="ExternalOutput")
        dbg["snorm"] = nc.dram_tensor("dbg_snorm", [1, SALL], F32, kind="ExternalOutput")
        dbg["qnorm"] = nc.dram_tensor("dbg_qnorm", [128, ITILES], F32, kind="ExternalOutput")
        dbg["rec"] = nc.dram_tensor("dbg_rec", [WAY, OW], F32, kind="ExternalOutput")
        dbg["mask"] = nc.dram_tensor("dbg_mask", [WAY, WAY - 1, S], F32, kind="ExternalOutput")
        dbg["dmax"] = nc.dram_tensor("dbg_dmax", [128, ITILES, WAY], F32, kind="ExternalOutput")
        dbg["nave2"] = nc.dram_tensor("dbg_nave2", [128, ITILES, WAY], F32, kind="ExternalOutput")
        dbg["pos"] = nc.dram_tensor("dbg_pos", [128, ITILES, WAY], F32, kind="ExternalOutput")
        dbg["ct"] = nc.dram_tensor("dbg_ct", [WAY, RHAT], F32, kind="ExternalOutput")

    with tile.TileContext(nc) as tc:
        _body(nc, tc, qd_d, sd_d, w_d, id_d, sel_d, padv_d, cc_d, out_d, dbg, sim1)
    nc.finalize()
    return nc


def _body(nc, tc, qd_d, sd_d, w_d, id_d, sel_d, padv_d, cc_d, out_d, dbg, sim1):
    AT = mybir.AluOpType
    ACTF = mybir.ActivationFunctionType
    X = mybir.AxisListType.X

    persist = tc.alloc_tile_pool(name="persist", bufs=1)
    dram = tc.alloc_tile_pool(name="dram", bufs=1, space="DRAM")

    # DRAM scratch
    p_dram = dram.tile([WAY, RROWS, PROW2], BF16, tag="p_scratch")
    posw_dram = dram.tile([WAY, 16, ITILES * 8], I16, tag="posw")
    snorm_dram = dram.tile([1, 3840], F32, tag="snormd")
    mask_dram = dram.tile([WAY, WAY - 1, RROWS], BF16, tag="maskd")
    cc_in = dram.tile([WAY, OW], F32, tag="cc_in")
    cc_out = dram.tile([WAY, OW], F32, tag="cc_out")

    # persistent SBUF
    q_embT = persist.tile([128, DC + 1, RHAT], F8, tag="q_embT")
    s_embT = persist.tile([128, DC + 1, SALL], F8, tag="s_embT")
    snh = persist.tile([1, SALL], BF16, tag="snh")          # -snorm/2
    qnorm = persist.tile([128, ITILES], F32, tag="qnorm")
    pnorm = persist.tile([128, WAY * CTILES], F32, tag="pnorm")
    m16a = persist.tile([128, ITILES, 16], F32, tag="m16a")
    dmax_all = persist.tile([128, ITILES, WAY], F32, tag="dmax_all")
    nave2 = persist.tile([128, ITILES, WAY], F32, tag="nave2")
    pos16 = persist.tile([128, ITILES, WAY], I16, tag="pos16")
    ident = persist.tile([128, 128], BF16, tag="ident")
    ones_bf = persist.tile([128, 1], BF16, tag="ones_bf")
    ones_f = persist.tile([128, 1], F32, tag="ones_f")
    ones_row = persist.tile([1, 128], BF16, tag="ones_row")
    padv = persist.tile([128, 1], F32, tag="padv")
    cconst = persist.tile([WAY, 2], F32, tag="cconst")
    sel_sb = persist.tile([128, ITILES, NQ], F32, tag="sel_sb")
    dmq = persist.tile([WAY, NQ], F32, tag="dmq")

    nc.vector.memset(ones_bf[:], 1.0)
    nc.vector.memset(ones_f[:], 1.0)
    nc.vector.memset(ones_row[:], 1.0)
    nc.sync.dma_start(padv[:], padv_d[:, :])
    nc.sync.dma_start(ident[:], id_d[:, :])
    nc.sync.dma_start(cconst[:], cc_d[:, :])
    nc.sync.dma_start(sel_sb[:], sel_d.rearrange("t p q -> p t q"))
    nc.vector.memset(q_embT[:, :, R:RHAT], 0.0)
    nc.vector.memset(q_embT[:, DC], 0.0)
    nc.vector.memset(s_embT[:, DC], 0.0)

    # ================= Phase 1: per-frame half products + tuple assembly ====
    with tc.tile_pool(name="emb", bufs=1) as emb, \
         tc.tile_pool(name="embsm", bufs=3) as embsm, \
         tc.tile_pool(name="embps", bufs=4, space="PSUM") as embps:
        wT = emb.tile([128, 2, 8, 2, DOUT], F8, tag="wT")
        nc.sync.dma_start(wT[:], w_d.rearrange("a b p c d -> p a b c d"))
        qd = emb.tile([128, 16, SEQ_LEN * NQ], F8, tag="qd")
        nc.sync.dma_start(qd[:], qd_d[:, :, :])
        sd = emb.tile([128, 16, SEQ_LEN * 80], F8, tag="sd")
        nc.sync.dma_start(sd[:], sd_d[:, :, :])
        Pq = emb.tile([128, DC, 2, SEQ_LEN * NQ], BF16, tag="Pq")
        Ps = emb.tile([128, DC, 2, SEQ_LEN * 80], BF16, tag="Ps")

        # s-side first: its embeddings gate snorm -> SS -> gathers
        for half in range(2):
            for dc in range(DC):
                for ch in range(2):
                    ps2 = embps.tile([128, SEQ_LEN * NQ], F32, tag="emb_ps")
                    for kc2 in range(8):
                        nc.tensor.matmul(
                            ps2[:], wT[:, half, kc2, :, dc * 128:(dc + 1) * 128],
                            sd[:, 2 * kc2:2 * kc2 + 2, ch * 400:(ch + 1) * 400],
                            start=(kc2 == 0), stop=(kc2 == 7), perf_mode=DR)
                    nc.vector.tensor_copy(
                        Ps[:, dc, half, ch * 400:(ch + 1) * 400], ps2[:])
        for t in range(T):
            f1, f2 = int(TUPLES[t][0]), int(TUPLES[t][1])
            pres = embsm.tile([128, DC, 80], BF16, tag="pres")
            nc.vector.tensor_tensor(
                pres[:], Ps[:, :, 0, f1 * 80:(f1 + 1) * 80],
                Ps[:, :, 1, f2 * 80:(f2 + 1) * 80], AT.add)
            dst = s_embT[:, :DC].rearrange("p d (u t) -> p d t u", t=T)[:, :, t]
            if t % 2 == 0:
                nc.scalar.activation(dst, pres[:], ACTF.Relu, scale=1.0 / 64.0)
            else:
                nc.vector.tensor_scalar(dst, pres[:], 0.0, 1.0 / 64.0,
                                        AT.max, op1=AT.mult)
        # q side
        for half in range(2):
            for dc in range(DC):
                ps = embps.tile([128, SEQ_LEN * NQ], F32, tag="emb_ps")
                for kc2 in range(8):
                    nc.tensor.matmul(
                        ps[:], wT[:, half, kc2, :, dc * 128:(dc + 1) * 128],
                        qd[:, 2 * kc2:2 * kc2 + 2, :],
                        start=(kc2 == 0), stop=(kc2 == 7), perf_mode=DR)
                nc.vector.tensor_copy(Pq[:, dc, half], ps[:])
        for t in range(T):
            f1, f2 = int(TUPLES[t][0]), int(TUPLES[t][1])
            preq = embsm.tile([128, DC, NQ], BF16, tag="preq")
            nc.vector.tensor_tensor(
                preq[:], Pq[:, :, 0, f1 * NQ:(f1 + 1) * NQ],
                Pq[:, :, 1, f2 * NQ:(f2 + 1) * NQ], AT.add)
            if t % 2 == 0:
                nc.scalar.activation(q_embT[:, :DC, t * NQ:(t + 1) * NQ], preq[:],
                                     ACTF.Relu, scale=1.0 / 64.0)
            else:
                nc.vector.tensor_scalar(q_embT[:, :DC, t * NQ:(t + 1) * NQ],
                                        preq[:], 0.0, 1.0 / 64.0,
                                        AT.max, op1=AT.mult)

    # dT allocated after the emb pool frees wT/Pq/Ps space
    dtp = tc.alloc_tile_pool(name="dtp", bufs=1)
    dT = [dtp.tile([128, CTILES, RHAT], F8, tag=f"dT{c}", name=f"dT{c}")
          for c in range(WAY)]

    if dbg:
        with tc.tile_pool(name="dbge", bufs=1) as dbge:
            t1 = dbge.tile([128, DC, RHAT], F32, tag="dbq")
            nc.vector.tensor_copy(t1[:], q_embT[:, :DC])
            nc.sync.dma_start(dbg["qemb"].ap(), t1[:])
            t2 = dbge.tile([128, DC, SALL], F32, tag="dbs")
            nc.vector.tensor_copy(t2[:], s_embT[:, :DC])
            nc.sync.dma_start(dbg["semb"].ap(), t2[:])

    # ================= Phase 2: norms =================
    with tc.tile_pool(name="nrm", bufs=2) as nrm, \
         tc.tile_pool(name="nrmps", bufs=2, space="PSUM") as nrmps:
        snrow = nrm.tile([1, SALL], F32, tag="snrow")
        for scn in range(8):
            ps = nrmps.tile([1, 450], F32, tag="sn_ps")
            for dc in range(DC):
                sq = nrm.tile([128, 450], BF16, tag="sn_sqb")
                nc.scalar.activation(sq[:], s_embT[:, dc, scn * 450:(scn + 1) * 450],
                                     ACTF.Square)
                nc.tensor.matmul(ps[:], ones_bf[:], sq[:],
                                 start=(dc == 0), stop=(dc == DC - 1))
            nc.scalar.activation(snrow[:, scn * 450:(scn + 1) * 450], ps[:], ACTF.Copy)
        nc.vector.tensor_scalar(snh[:], snrow[:], -0.5, None, AT.mult)
        nc.sync.dma_start(snorm_dram[:, :SALL], snrow[:])
        # pnorm[p, c*6+j] = snorm[720c + 128j + p]
        for c in range(WAY):
            nc.sync.dma_start(
                pnorm[:, c * CTILES:(c + 1) * CTILES],
                _ap(snorm_dram.tensor, snorm_dram.offset + c * S,
                    [(1, 128), (128, CTILES)]))
        for it in range(ITILES):
            ps = nrmps.tile([128, 1], F32, tag="qn_ps", name="qnps")
            sqa = nrm.tile([128, DC, 128], BF16, tag="qn_sqb")
            qb = nrm.tile([128, DC, 128], BF16, tag="qn_qb")
            nc.vector.tensor_copy(qb[:], q_embT[:, :DC, it * 128:(it + 1) * 128])
            nc.vector.tensor_tensor(sqa[:], qb[:], qb[:], AT.mult)
            for dc in range(DC):
                nc.tensor.matmul(ps[:], sqa[:, dc], ones_bf[:],
                                 start=(dc == 0), stop=(dc == DC - 1))
            nc.vector.tensor_copy(qnorm[:, it:it + 1], ps[:])
        if dbg:
            nc.sync.dma_start(dbg["snorm"].ap(), snrow[:])
            nc.sync.dma_start(dbg["qnorm"].ap(), qnorm[:])

    # ====== Phases 3+4: per class, SS slab -> D tiles -> gather/rec ======
    with tc.tile_pool(name="ssst", bufs=1) as ssst, \
         tc.tile_pool(name="ssps", bufs=2, space="PSUM") as ssps, \
         tc.tile_pool(name="dph", bufs=3) as dph, \
         tc.tile_pool(name="dsm", bufs=4) as dsm, \
         tc.tile_pool(name="dps", bufs=2, space="PSUM") as dps, \
         tc.tile_pool(name="tps", bufs=2, space="PSUM") as tps, \
         tc.tile_pool(name="cdp", bufs=2) as cdp, \
         tc.tile_pool(name="accp", bufs=2) as accp, \
         tc.tile_pool(name="rrp", bufs=2) as rrp, \
         tc.tile_pool(name="cmpp", bufs=2) as cmpp, \
         tc.tile_pool(name="cdg", bufs=3) as cdg:
        for c in range(WAY):
            # ---- SS slab for class c ----
            staging = ssst.tile([128, CTILES, PROW2], BF16, tag="ss_stage")
            chunks = _chunks_for_class(c)
            for j in range(CTILES):
                p0 = S * c + 128 * j
                pw = min(128, S - 128 * j)
                for (dst0, pieces) in chunks:
                    ps = ssps.tile([128, 960], F32, tag="ss_ps")
                    for (src0, doff, w) in pieces:
                        for dc2 in range(NDR):
                            nc.tensor.matmul(
                                ps[:pw, doff:doff + w],
                                s_embT[:, 2 * dc2:2 * dc2 + 2, p0:p0 + pw],
                                s_embT[:, 2 * dc2:2 * dc2 + 2, src0:src0 + w],
                                start=(dc2 == 0), stop=False, perf_mode=DR)
                        nc.tensor.matmul(
                            ps[:pw, doff:doff + w], ones_row[:, :pw],
                            snh[:, src0:src0 + w], start=False, stop=True)
                    nc.scalar.activation(staging[:pw, j, dst0:dst0 + 960],
                                         ps[:pw], ACTF.Copy, scale=-2.0)
            # row-norm column (col 2880) for the gathered threshold
            nc.vector.tensor_copy(staging[:, :, OW:OW + 1],
                                  pnorm[:, c * CTILES:(c + 1) * CTILES, None])
            # write region, skipping the undefined pad rows of the last tile
            nc.sync.dma_start(
                _ap(p_dram.tensor, p_dram.offset + c * RROWS * PROW2,
                    [(PROW2, 128), (128 * PROW2, CTILES - 1), (1, PROW2)]),
                staging[:, :CTILES - 1])
            nc.sync.dma_start(
                _ap(p_dram.tensor,
                    p_dram.offset + (c * RROWS + (CTILES - 1) * 128) * PROW2,
                    [(PROW2, 80), (1, PROW2)]),
                staging[:80, CTILES - 1])
            # ---- D tiles ----
            for it in range(ITILES):
                d_bf = dph.tile([128, RROWS], BF16, tag="d_bf")
                for sc in range(2):
                    ps = dps.tile([128, 360], F32, tag="d_ps", name="dps")
                    s0 = c * S + sc * 360
                    for dc2 in range(NDR):
                        nc.tensor.matmul(
                            ps[:], q_embT[:, 2 * dc2:2 * dc2 + 2, it * 128:(it + 1) * 128],
                            s_embT[:, 2 * dc2:2 * dc2 + 2, s0:s0 + 360],
                            start=(dc2 == 0), stop=False, perf_mode=DR)
                    nc.tensor.matmul(ps[:], ones_row[:], snh[:, s0:s0 + 360],
                                     start=False, stop=True)
                    nc.scalar.activation(d_bf[:, sc * 360:(sc + 1) * 360], ps[:],
                                         ACTF.Sqrt, bias=qnorm[:, it:it + 1],
                                         scale=-2.0)
                nc.vector.memset(d_bf[:, S:RROWS], 0.0)
                # reductions
                nc.vector.tensor_reduce(
                    m16a[:, it], d_bf[:, :S].rearrange("p (a b) -> p b a", b=16),
                    X, AT.max)
                nc.vector.tensor_reduce(dmax_all[:, it, c:c + 1], m16a[:, it],
                                        X, AT.max)
                mx8 = dsm.tile([128, 8], F32, tag="mx8")
                ix8 = dsm.tile([128, 8], U32, tag="ix8")
                nc.vector.tensor_copy(
                    mx8[:], dmax_all[:, it, c:c + 1].to_broadcast((128, 8)))
                nc.vector.max_index(ix8[:], mx8[:], d_bf[:, :S])
                posf = dsm.tile([128, 1], F32, tag="posf")
                nc.vector.tensor_scalar(posf[:], ix8[:, 0:1], 0.0, None, AT.add)
                nc.vector.tensor_copy(pos16[:, it, c:c + 1], posf[:])
                # transpose into dT (fp8)
                psT = tps.tile([128, CTILES * 128], BF16, tag="psT")
                for j in range(CTILES):
                    nc.tensor.matmul(psT[:, j * 128:(j + 1) * 128],
                                     d_bf[:, j * 128:(j + 1) * 128], ident[:],
                                     start=True, stop=True, is_transpose=True)
                if it % 2 == 0:
                    nc.vector.tensor_copy(
                        dT[c][:, :, it * 128:(it + 1) * 128],
                        psT[:].rearrange("p (j q) -> p j q", j=CTILES))
                else:
                    nc.scalar.activation(
                        dT[c][:, :, it * 128:(it + 1) * 128],
                        psT[:].rearrange("p (j q) -> p j q", j=CTILES), ACTF.Copy)
            nc.vector.memset(dT[c][:, :, R:RHAT], 0.0)
            # batched per-class stats: nave2 = -(asum/16)^2 ; dmax
            asum = dsm.tile([128, ITILES], F32, tag="asum")
            nc.vector.tensor_reduce(asum[:], m16a[:], X, AT.add)
            nc.vector.tensor_scalar(asum[:, ITILES - 1:ITILES],
                                    asum[:, ITILES - 1:ITILES], padv[:], None, AT.add)
            nc.vector.tensor_tensor(asum[:], asum[:], asum[:], AT.mult)
            nc.vector.tensor_scalar(nave2[:, :, c], asum[:], -1.0 / 256.0, None,
                                    AT.mult)
            if c == WAY - 1:
                psD = ssps.tile([WAY, NQ], F32, tag="ss_ps", name="dmps")
                for it in range(ITILES):
                    nc.tensor.matmul(psD[:], dmax_all[:, it, :], sel_sb[:, it],
                                     start=(it == 0), stop=(it == ITILES - 1))
                nc.scalar.activation(dmq[:], psD[:], ACTF.Copy, scale=1.0 / T)

            # ---- gather + rec ----
            nc.sync.dma_start(
                _ap(posw_dram.tensor, posw_dram.offset + c * 16 * ITILES * 8,
                    [(1, 8), (ITILES * 8, 16), (8, ITILES)]),
                pos16[:, :, c])
            idxs = cdp.tile([128, ITILES * 8], I16, tag="idxs")
            nc.sync.dma_start(
                idxs[:],
                _ap(posw_dram.tensor, posw_dram.offset + c * 16 * ITILES * 8,
                    [(0, 8), (ITILES * 8, 16), (1, ITILES * 8)]))
            use_sign = c in SIGN_CLASSES
            acc = accp.tile([128, OW], BF16, tag="accb")
            if not use_sign:
                nc.vector.memset(acc[:], 0.0)
                nbneg = cdp.tile([128, ITILES], F32, tag="nbneg")
            region = _ap(p_dram.tensor, p_dram.offset + c * RROWS * PROW2,
                         [(PROW2, RROWS), (1, PROW2)])
            # software-pipelined: gather(g)/nbias(g) run one stage ahead of
            # sign(g)/add(g) so the ACT sign stream never waits on DVE.
            nball = cdp.tile([128, ITILES], F32, tag="nball")
            cds = {}

            def _compare(g):
                if use_sign:
                    cmp = cmpp.tile([128, OW], BF16, tag="cmp")
                    nc.scalar.activation(cmp[:], cds.pop(g)[:, 0, :OW], ACTF.Sign,
                                         bias=nball[:, g:g + 1])
                    if g == 0:
                        nc.vector.tensor_copy(acc[:], cmp[:])
                    else:
                        nc.vector.tensor_tensor(acc[:], acc[:], cmp[:], AT.add)
                else:
                    nc.vector.tensor_scalar(nbneg[:, g:g + 1], nball[:, g:g + 1],
                                            -1.0, None, AT.mult)
                    nc.vector.scalar_tensor_tensor(
                        acc[:], cds.pop(g)[:, 0, :OW], nbneg[:, g:g + 1], acc[:],
                        op0=AT.is_gt, op1=AT.add)

            for g in range(ITILES):
                cd = cdg.tile([128, 1, PROW2], BF16, tag="cd")
                nc.gpsimd.dma_gather(
                    cd[:], region, idxs[:, g * 8:(g + 1) * 8],
                    128, 128, PROW2, queue_num=g % 4)
                # bias = n_i - ave^2 : sign(cd_raw + bias) = sign(d^2 - ave^2)
                nc.vector.tensor_tensor(nball[:, g:g + 1], cd[:, 0, OW:OW + 1],
                                        nave2[:, g, c:c + 1], AT.add)
                cds[g] = cd
                if g >= 1:
                    _compare(g - 1)
            _compare(ITILES - 1)
            for k in range(6):
                ps = ssps.tile([1, 480], F32, tag="ss_ps", name="recps")
                nc.tensor.matmul(ps[:], ones_bf[:],
                                 acc[:, k * 480:(k + 1) * 480],
                                 start=True, stop=True)
                rc = rrp.tile([1, 480], F32, tag="recc")
                nc.scalar.activation(rc[:], ps[:], ACTF.Copy)
                nc.sync.dma_start(cc_in[c:c + 1, k * 480:(k + 1) * 480], rc[:])

    # ================= AllReduce rec =================
    if sim1:
        nc.sync.dma_start(cc_out[:, :], cc_in[:, :])
    else:
        nc.gpsimd.collective_compute(
            "AllReduce", mybir.AluOpType.add,
            replica_groups=[list(range(NCORES))],
            ins=[cc_in[:, :].opt()], outs=[cc_out[:, :].opt()])

    # ================= Phase 5: thr/mask =================
    with tc.tile_pool(name="thrp", bufs=2) as thrp, \
         tc.tile_pool(name="thrbig", bufs=1) as thrbig:
        rec = thrbig.tile([WAY, WAY - 1, S], F32, tag="rec")
        nc.sync.dma_start(rec[:], cc_out[:, :].rearrange("c (k s) -> c k s", k=WAY - 1))
        # per-class affine: sign classes hold sum(+-1); counts = (x+15360)/2
        nc.vector.tensor_scalar(rec[:], rec[:], cconst[:, 0:1], None, AT.add)
        nc.vector.tensor_scalar(rec[:], rec[:], cconst[:, 1:2], None, AT.mult)
        if dbg:
            with tc.tile_pool(name="dbgr", bufs=1) as dbgr:
                rg = dbgr.tile([WAY, OW], F32, tag="rg")
                nc.vector.tensor_copy(rg[:], rec[:].rearrange("c k s -> c (k s)"))
                nc.sync.dma_start(dbg["rec"].ap(), rg[:])
        rsum = thrp.tile([WAY, WAY - 1], F32, tag="rsum")
        nc.vector.tensor_reduce(rsum[:], rec[:], X, AT.add)
        gt0 = thrbig.tile([WAY, WAY - 1, S], F32, tag="gt0")
        nc.vector.tensor_scalar(gt0[:], rec[:], 0.5, None, AT.is_gt)
        nz = thrp.tile([WAY, WAY - 1], F32, tag="nz")
        nc.vector.tensor_reduce(nz[:], gt0[:], X, AT.add)
        nc.vector.tensor_scalar(nz[:], nz[:], 1.0, None, AT.max)
        thr = thrp.tile([WAY, WAY - 1], F32, tag="thr")
        nc.vector.reciprocal(thr[:], nz[:])
        nc.vector.tensor_tensor(thr[:], thr[:], rsum[:], AT.mult)
        mask_slots = thrbig.tile([WAY, WAY - 1, RROWS], BF16, tag="mask_slots")
        nc.vector.memset(mask_slots[:, :, S:RROWS], 0.0)
        nc.vector.tensor_tensor(
            mask_slots[:, :, :S], rec[:],
            thr[:, :, None].to_broadcast((WAY, WAY - 1, S)), AT.is_lt)
        if dbg:
            with tc.tile_pool(name="dbgm", bufs=1) as dbgm:
                mg = dbgm.tile([WAY, WAY - 1, S], F32, tag="mg")
                nc.vector.tensor_copy(mg[:], mask_slots[:, :, :S])
                nc.sync.dma_start(dbg["mask"].ap(), mg[:])
        msum = thrp.tile([WAY, 1], F32, tag="msum")
        nc.vector.tensor_reduce(
            msum[:], mask_slots[:].rearrange("c k s -> c (k s)"), X, AT.add)
        nc.vector.tensor_scalar(msum[:], msum[:], 1.0, None, AT.max)
        scv = thrp.tile([WAY, 1], F32, tag="scv")
        nc.vector.reciprocal(scv[:], msum[:])
        nc.vector.tensor_scalar(scv[:], scv[:], 1.0 / (4.0 * T), None, AT.mult)
        nc.sync.dma_start(mask_dram[:, :, :], mask_slots[:])

        # ============= Phase 6: contrast sums + finals =============
        with tc.tile_pool(name="p6", bufs=1) as p6, \
             tc.tile_pool(name="p6ps", bufs=1, space="PSUM") as p6ps:
            maskT = p6.tile([128, WAY * CTILES, WAY], BF16, tag="maskT")
            nc.vector.memset(maskT[:], 0.0)
            for c in range(WAY):
                if c > 0:
                    nc.sync.dma_start(
                        maskT[:, 0:c * CTILES, c],
                        _ap(mask_dram.tensor,
                            mask_dram.offset + c * (WAY - 1) * RROWS,
                            [(1, 128), (128, c * CTILES)]))
                if c < WAY - 1:
                    nc.sync.dma_start(
                        maskT[:, (c + 1) * CTILES:WAY * CTILES, c],
                        _ap(mask_dram.tensor,
                            mask_dram.offset + (c * (WAY - 1) + c) * RROWS,
                            [(1, 128), (128, (WAY - 1 - c) * CTILES)]))
            psC = [p6ps.tile([WAY, 480], F32, tag=f"ct_ps{qc}", name=f"ctps{qc}")
                   for qc in range(4)]
            for cr in range(WAY):
                for j in range(CTILES):
                    for qc in range(4):
                        nc.tensor.matmul(
                            psC[qc][:], maskT[:, cr * CTILES + j, :],
                            dT[cr][:, j, qc * 480:(qc + 1) * 480],
                            start=(cr == 0 and j == 0),
                            stop=(cr == WAY - 1 and j == CTILES - 1))
            ctrows = p6.tile([WAY, RHAT], F32, tag="ctrows")
            for qc in range(4):
                nc.scalar.activation(ctrows[:, qc * 480:(qc + 1) * 480], psC[qc][:],
                                     ACTF.Copy)
            if dbg:
                nc.sync.dma_start(dbg["ct"].ap(), ctrows[:])
            ctq = p6.tile([WAY, NQ], F32, tag="ctq")
            nc.vector.tensor_reduce(
                ctq[:], ctrows[:].rearrange("c (s q) -> c q s", q=NQ), X, AT.add)
            nc.vector.tensor_scalar(ctq[:], ctq[:], scv[:], None, AT.mult)

            if dbg:
                nc.sync.dma_start(dbg["dmax"].ap(), dmax_all[:])
                nc.sync.dma_start(dbg["nave2"].ap(), nave2[:])
                with tc.tile_pool(name="dbgp", bufs=1) as dbgp:
                    pf = dbgp.tile([128, ITILES, WAY], F32, tag="pf")
                    nc.vector.tensor_copy(pf[:], pos16[:])
                    nc.sync.dma_start(dbg["pos"].ap(), pf[:])

            ssum = p6.tile([WAY, NQ], F32, tag="ssum")
            nc.vector.tensor_tensor(ssum[:], dmq[:], ctq[:], AT.add)
            rcp = p6.tile([WAY, NQ], F32, tag="rcp")
            nc.vector.reciprocal(rcp[:], ssum[:])
            lg = p6.tile([WAY, NQ], F32, tag="lg")
            nc.vector.tensor_tensor(lg[:], dmq[:], rcp[:], AT.mult)
            nc.sync.dma_start(_ap(out_d, 0, [(1, WAY), (WAY, NQ)]), dmq[:])
            nc.sync.dma_start(_ap(out_d, NQ * WAY, [(1, WAY), (WAY, NQ)]), lg[:])

    dtp.release()
    persist.release()
    dram.release()


# ---------------- host side ----------------

def _sel_host():
    sel = np.zeros((ITILES, 128, NQ), np.float32)
    for i in range(R):
        sel[i // 128, i % 128, i % NQ] = 1.0
    return sel


def _prep_inputs(support_set, queries, support_labels, W, b):
    import ml_dtypes
    f8 = ml_dtypes.float8_e4m3fn
    support_set = np.asarray(support_set, dtype=np.float32)
    queries = np.asarray(queries, dtype=np.float32)
    labels = np.asarray(support_labels).astype(np.int64)
    W = np.asarray(W, dtype=np.float32)
    b = np.asarray(b, dtype=np.float32)
    assert not np.any(b), "kernel built without bias support (reference b==0)"
    order = np.argsort(labels, kind="stable")
    support_sorted = support_set[order]

    # wT [2, 8, 128, 2, 1152]: wT[half, kc2, p, h2, d] =
    #   64*W[d, half*2048 + kc2*256 + h2*128 + p]
    w8 = (W * 64.0).astype(f8)                     # [1152, 4096]
    wT = np.ascontiguousarray(
        w8.reshape(DOUT, 2, 8, 2, 128).transpose(1, 2, 4, 3, 0))

    s8 = support_sorted.astype(f8)                 # [80, 10, 2048]
    sd = np.ascontiguousarray(
        s8.reshape(80, SEQ_LEN, 16, 128).transpose(3, 2, 1, 0)
          .reshape(128, 16, SEQ_LEN * 80))
    q8 = queries.astype(f8)                        # [320, 10, 2048]
    sel = _sel_host()
    padv = np.zeros((128, 1), np.float32)
    padv[8:] = 1.0e15
    ident = np.eye(128).astype(ml_dtypes.bfloat16)
    cconst = np.zeros((WAY, 2), np.float32)
    for c in range(WAY):
        if c in SIGN_CLASSES:
            cconst[c] = (NCORES * RHAT, 0.5)
        else:
            cconst[c] = (0.0, 1.0)
    out = []
    for k in range(NCORES):
        qk = q8[k * NQ:(k + 1) * NQ]               # [40, 10, 2048]
        qd = np.ascontiguousarray(
            qk.reshape(NQ, SEQ_LEN, 16, 128).transpose(3, 2, 1, 0)
              .reshape(128, 16, SEQ_LEN * NQ))
        out.append({
            "qd": qd,
            "sd": sd,
            "wT": wT,
            "ident": ident,
            "sel": sel,
            "padv": padv,
            "cconst": cconst,
        })
    return out


def kernel(**inputs):
    per_core = _prep_inputs(**inputs)
    if "nc" not in _CACHE:
        _CACHE["nc"] = build(debug=bool(os.environ.get("BIMACL_DEBUG")))
    nc = _CACHE["nc"]
    res = run_bass_kernel_spmd(nc, per_core, core_ids=list(range(NCORES)))
    _CACHE["last_results"] = res
    full = np.concatenate([res.results[k]["out"] for k in range(NCORES)], axis=1)
    return np.ascontiguousarray(full.astype(np.float32))


# revision 32
# speedup vs baseline: 3.1919x; 1.0626x over previous
"""Trainium2 Bass kernel for nn_CNN_BiMACL_31860067401819 (retrieval_knn).

Self-contained: hardcodes all shapes/sharding. kernel(**inputs) accepts FULL
inputs keyed as in setup_inputs(), shards queries across 8 NeuronCores
(data-parallel over the query axis), and returns the FULL [2, 320, 5] f32
output. The only collective is an AllReduce of the per-class `rec` counts.

Design (v2):
- Frame-factorized embeddings: emb(tuple t=(f1,f2)) = relu(W1^T x_f1 +
  W2^T x_f2); per-frame half-products are computed once with fp8 DoubleRow
  matmuls, tuples assembled with bf16 adds + ACT relu into fp8 embeddings.
- All distance matmuls fp8 + DoubleRow (256-deep contraction per instr).
- SS (support-support) stays in d^2 space: psum = s_i.s_j - sn_j/2 (column
  norm folded in via a 1-row matmul), scaled by -2 on the psum->SBUF copy;
  row norm rides along as an extra gathered column. rec compare is then
  cd_raw + n_i > ave^2  <=>  d^2 > ave^2 (no sqrt for SS at all).
- D (query-support): psum = q.s - sn/2; ACT Sqrt(scale=-2, bias=qnorm)
  emits bf16 distances directly.
- rec compare+accumulate: ACT Sign for SIGN_CLASSES (affine-corrected after
  the AllReduce), fused DVE scalar_tensor_tensor (is_gt,add) for the rest.
- Phase-6 masked row sums via PE transpose of D (stored fp8) + mask matmuls.
"""
import os
from itertools import combinations

import numpy as np

import concourse.bass as bass
import concourse.tile as tile
from concourse import bacc, mybir
from concourse.bass_utils import run_bass_kernel_spmd

# ---- static problem config ----
WAY, SHOT, SEQ_LEN, TSS = 5, 16, 10, 2
DIN, DOUT = 2048, 1152
N_QUERIES = 320
T = 45
S = SHOT * T                 # 720 support tuples per class
SALL = WAY * S               # 3600
NCORES = 8
NQ = N_QUERIES // NCORES     # 40
R = NQ * T                   # 1800 valid rows/core
RHAT = 1920                  # 15*128 padded rows
ITILES = RHAT // 128         # 15
TUPLES = np.array(list(combinations(range(SEQ_LEN), TSS)), dtype=np.int32)
DC = DOUT // 128             # 9
NDR = 5                      # DoubleRow matmuls per padded 1280 contraction (5*256)
OW = (WAY - 1) * S           # 2880 other-class columns
PROW2 = 2944                 # per-class region row pitch (2880 data + norm + pad)
CTILES = 6                   # 128-row tiles per class region (768 rows)
RROWS = CTILES * 128         # 768
SIGN_CLASSES = (0, 1, 2, 3, 4)  # rec compare on ACT Sign
WA = 2240                    # compare width on ACT; rest on DVE stt

F32 = mybir.dt.float32
BF16 = mybir.dt.bfloat16
F8 = mybir.dt.float8e4
U32 = mybir.dt.uint32
I16 = mybir.dt.int16
DR = mybir.MatmulPerfMode.DoubleRow

_CACHE = {}


def _ap(tensor, offset, dims):
    return bass.AP(tensor=tensor, offset=offset, ap=[list(d) for d in dims])


def _chunks_for_class(c):
    """960-wide dst chunks over the 2880 other-class cols of class c, each
    split into <=480-wide matmul pieces (PSUM-bank limit).
    Returns list of (dst_off, [(src_col, dst_delta, width), ...])."""
    spans = []
    if c > 0:
        spans.append((0, 0, S * c))              # (dst0, src0, len)
    spans.append((S * c, S * (c + 1), OW - S * c))
    out = []
    for dst0 in range(0, OW, 960):
        pieces = []
        for sub in range(2):
            w0 = dst0 + sub * 480
            for sd, ss, ln in spans:
                lo = max(w0, sd)
                hi = min(w0 + 480, sd + ln)
                if lo < hi:
                    pieces.append((ss + (lo - sd), lo - dst0, hi - lo))
        out.append((dst0, pieces))
    return out


def build(debug=False, sim1=False):
    nc = bacc.Bacc(num_swdge_queues=4)
    qd_d = nc.dram_tensor("qd", [128, 16, SEQ_LEN * NQ], F8, kind="ExternalInput")
    sd_d = nc.dram_tensor("sd", [128, 16, SEQ_LEN * 80], F8, kind="ExternalInput")
    w_d = nc.dram_tensor("wT", [2, 8, 128, 2, DOUT], F8, kind="ExternalInput")
    id_d = nc.dram_tensor("ident", [128, 128], BF16, kind="ExternalInput")
    sel_d = nc.dram_tensor("sel", [ITILES, 128, NQ], F32, kind="ExternalInput")
    padv_d = nc.dram_tensor("padv", [128, 1], F32, kind="ExternalInput")
    cc_d = nc.dram_tensor("cconst", [WAY, 2], F32, kind="ExternalInput")
    out_d = nc.dram_tensor("out", [2, NQ, WAY], F32, kind="ExternalOutput")
    dbg = {}
    if debug:
        dbg["qemb"] = nc.dram_tensor("dbg_qemb", [128, DC, RHAT], F32, kind="ExternalOutput")
        dbg["semb"] = nc.dram_tensor("dbg_semb", [128, DC, SALL], F32, kind="ExternalOutput")
        dbg["snorm"] = nc.dram_tensor("dbg_snorm", [1, SALL], F32, kind="ExternalOutput")
        dbg["qnorm"] = nc.dram_tensor("dbg_qnorm", [128, ITILES], F32, kind="ExternalOutput")
        dbg["rec"] = nc.dram_tensor("dbg_rec", [WAY, OW], F32, kind="ExternalOutput")
        dbg["mask"] = nc.dram_tensor("dbg_mask", [WAY, WAY - 1, S], F32, kind="ExternalOutput")
        dbg["dmax"] = nc.dram_tensor("dbg_dmax", [128, ITILES, WAY], F32, kind="ExternalOutput")
        dbg["nave2"] = nc.dram_tensor("dbg_nave2", [128, ITILES, WAY], F32, kind="ExternalOutput")
        dbg["pos"] = nc.dram_tensor("dbg_pos", [128, ITILES, WAY], F32, kind="ExternalOutput")
        dbg["ct"] = nc.dram_tensor("dbg_ct", [WAY, RHAT], F32, kind="ExternalOutput")

    with tile.TileContext(nc) as tc:
        _body(nc, tc, qd_d, sd_d, w_d, id_d, sel_d, padv_d, cc_d, out_d, dbg, sim1)
    nc.finalize()
    return nc


def _body(nc, tc, qd_d, sd_d, w_d, id_d, sel_d, padv_d, cc_d, out_d, dbg, sim1):
    AT = mybir.AluOpType
    ACTF = mybir.ActivationFunctionType
    X = mybir.AxisListType.X

    persist = tc.alloc_tile_pool(name="persist", bufs=1)
    dram = tc.alloc_tile_pool(name="dram", bufs=1, space="DRAM")

    # DRAM scratch
    p_dram = dram.tile([WAY, RROWS, PROW2], BF16, tag="p_scratch")
    posw_dram = dram.tile([WAY, 16, ITILES * 8], I16, tag="posw")
    snorm_dram = dram.tile([1, 3840], F32, tag="snormd")
    mask_dram = dram.tile([WAY, WAY - 1, RROWS], BF16, tag="maskd")
    cc_in = dram.tile([WAY, OW], F32, tag="cc_in")
    cc_out = dram.tile([WAY, OW], F32, tag="cc_out")

    # persistent SBUF
    q_embT = persist.tile([128, DC + 1, RHAT], F8, tag="q_embT")
    s_embT = persist.tile([128, DC + 1, SALL], F8, tag="s_embT")
    snh = persist.tile([1, SALL], BF16, tag="snh")          # -snorm/2
    qnorm = persist.tile([128, ITILES], F32, tag="qnorm")
    pnorm = persist.tile([128, WAY * CTILES], F32, tag="pnorm")
    m16a = persist.tile([128, ITILES, 16], F32, tag="m16a")
    dmax_all = persist.tile([128, ITILES, WAY], F32, tag="dmax_all")
    nave2 = persist.tile([128, ITILES, WAY], F32, tag="nave2")
    pos16 = persist.tile([128, ITILES, WAY], I16, tag="pos16")
    ident = persist.tile([128, 128], BF16, tag="ident")
    ones_bf = persist.tile([128, 1], BF16, tag="ones_bf")
    ones_f = persist.tile([128, 1], F32, tag="ones_f")
    ones_row = persist.tile([1, 128], BF16, tag="ones_row")
    padv = persist.tile([128, 1], F32, tag="padv")
    cconst = persist.tile([WAY, 2], F32, tag="cconst")
    sel_sb = persist.tile([128, ITILES, NQ], F32, tag="sel_sb")
    dmq = persist.tile([WAY, NQ], F32, tag="dmq")

    nc.vector.memset(ones_bf[:], 1.0)
    nc.vector.memset(ones_f[:], 1.0)
    nc.vector.memset(ones_row[:], 1.0)
    nc.sync.dma_start(padv[:], padv_d[:, :])
    nc.sync.dma_start(ident[:], id_d[:, :])
    nc.sync.dma_start(cconst[:], cc_d[:, :])
    nc.sync.dma_start(sel_sb[:], sel_d.rearrange("t p q -> p t q"))
    nc.vector.memset(q_embT[:, :, R:RHAT], 0.0)
    nc.vector.memset(q_embT[:, DC], 0.0)
    nc.vector.memset(s_embT[:, DC], 0.0)

    # ================= Phase 1: per-frame half products + tuple assembly ====
    with tc.tile_pool(name="emb", bufs=1) as emb, \
         tc.tile_pool(name="embsm", bufs=3) as embsm, \
         tc.tile_pool(name="embps", bufs=4, space="PSUM") as embps:
        wT = emb.tile([128, 2, 8, 2, DOUT], F8, tag="wT")
        nc.sync.dma_start(wT[:], w_d.rearrange("a b p c d -> p a b c d"))
        qd = emb.tile([128, 16, SEQ_LEN * NQ], F8, tag="qd")
        nc.sync.dma_start(qd[:], qd_d[:, :, :])
        sd = emb.tile([128, 16, SEQ_LEN * 80], F8, tag="sd")
        nc.sync.dma_start(sd[:], sd_d[:, :, :])
        Pq = emb.tile([128, DC, 2, SEQ_LEN * NQ], BF16, tag="Pq")
        Ps = emb.tile([128, DC, 2, SEQ_LEN * 80], BF16, tag="Ps")

        # s-side first: its embeddings gate snorm -> SS -> gathers
        for half in range(2):
            for dc in range(DC):
                for ch in range(2):
                    ps2 = embps.tile([128, SEQ_LEN * NQ], F32, tag="emb_ps")
                    for kc2 in range(8):
                        nc.tensor.matmul(
                            ps2[:], wT[:, half, kc2, :, dc * 128:(dc + 1) * 128],
                            sd[:, 2 * kc2:2 * kc2 + 2, ch * 400:(ch + 1) * 400],
                            start=(kc2 == 0), stop=(kc2 == 7), perf_mode=DR)
                    nc.vector.tensor_copy(
                        Ps[:, dc, half, ch * 400:(ch + 1) * 400], ps2[:])
        for t in range(T):
            f1, f2 = int(TUPLES[t][0]), int(TUPLES[t][1])
            pres = embsm.tile([128, DC, 80], BF16, tag="pres")
            nc.vector.tensor_tensor(
                pres[:], Ps[:, :, 0, f1 * 80:(f1 + 1) * 80],
                Ps[:, :, 1, f2 * 80:(f2 + 1) * 80], AT.add)
            dst = s_embT[:, :DC].rearrange("p d (u t) -> p d t u", t=T)[:, :, t]
            if t % 2 == 0:
                nc.scalar.activation(dst, pres[:], ACTF.Relu, scale=1.0 / 64.0)
            else:
                nc.vector.tensor_scalar(dst, pres[:], 0.0, 1.0 / 64.0,
                                        AT.max, op1=AT.mult)
        # q side
        for half in range(2):
            for dc in range(DC):
                ps = embps.tile([128, SEQ_LEN * NQ], F32, tag="emb_ps")
                for kc2 in range(8):
                    nc.tensor.matmul(
                        ps[:], wT[:, half, kc2, :, dc * 128:(dc + 1) * 128],
                        qd[:, 2 * kc2:2 * kc2 + 2, :],
                        start=(kc2 == 0), stop=(kc2 == 7), perf_mode=DR)
                nc.vector.tensor_copy(Pq[:, dc, half], ps[:])
        for t in range(T):
            f1, f2 = int(TUPLES[t][0]), int(TUPLES[t][1])
            preq = embsm.tile([128, DC, NQ], BF16, tag="preq")
            nc.vector.tensor_tensor(
                preq[:], Pq[:, :, 0, f1 * NQ:(f1 + 1) * NQ],
                Pq[:, :, 1, f2 * NQ:(f2 + 1) * NQ], AT.add)
            if t % 2 == 0:
                nc.scalar.activation(q_embT[:, :DC, t * NQ:(t + 1) * NQ], preq[:],
                                     ACTF.Relu, scale=1.0 / 64.0)
            else:
                nc.vector.tensor_scalar(q_embT[:, :DC, t * NQ:(t + 1) * NQ],
                                        preq[:], 0.0, 1.0 / 64.0,
                                        AT.max, op1=AT.mult)

    # dT allocated after the emb pool frees wT/Pq/Ps space
    dtp = tc.alloc_tile_pool(name="dtp", bufs=1)
    dT = [dtp.tile([128, CTILES, RHAT], F8, tag=f"dT{c}", name=f"dT{c}")
          for c in range(WAY)]

    if dbg:
        with tc.tile_pool(name="dbge", bufs=1) as dbge:
            t1 = dbge.tile([128, DC, RHAT], F32, tag="dbq")
            nc.vector.tensor_copy(t1[:], q_embT[:, :DC])
            nc.sync.dma_start(dbg["qemb"].ap(), t1[:])
            t2 = dbge.tile([128, DC, SALL], F32, tag="dbs")
            nc.vector.tensor_copy(t2[:], s_embT[:, :DC])
            nc.sync.dma_start(dbg["semb"].ap(), t2[:])

    # ================= Phase 2: norms =================
    with tc.tile_pool(name="nrm", bufs=2) as nrm, \
         tc.tile_pool(name="nrmps", bufs=2, space="PSUM") as nrmps:
        snrow = nrm.tile([1, SALL], F32, tag="snrow")
        for scn in range(8):
            ps = nrmps.tile([1, 450], F32, tag="sn_ps")
            for dc in range(DC):
                sq = nrm.tile([128, 450], BF16, tag="sn_sqb")
                nc.scalar.activation(sq[:], s_embT[:, dc, scn * 450:(scn + 1) * 450],
                                     ACTF.Square)
                nc.tensor.matmul(ps[:], ones_bf[:], sq[:],
                                 start=(dc == 0), stop=(dc == DC - 1))
            nc.scalar.activation(snrow[:, scn * 450:(scn + 1) * 450], ps[:], ACTF.Copy)
        nc.vector.tensor_scalar(snh[:], snrow[:], -0.5, None, AT.mult)
        nc.sync.dma_start(snorm_dram[:, :SALL], snrow[:])
        # pnorm[p, c*6+j] = snorm[720c + 128j + p]
        for c in range(WAY):
            nc.sync.dma_start(
                pnorm[:, c * CTILES:(c + 1) * CTILES],
                _ap(snorm_dram.tensor, snorm_dram.offset + c * S,
                    [(1, 128), (128, CTILES)]))
        for it in range(ITILES):
            ps = nrmps.tile([128, 1], F32, tag="qn_ps", name="qnps")
            sqa = nrm.tile([128, DC, 128], BF16, tag="qn_sqb")
            qb = nrm.tile([128, DC, 128], BF16, tag="qn_qb")
            nc.vector.tensor_copy(qb[:], q_embT[:, :DC, it * 128:(it + 1) * 128])
            nc.vector.tensor_tensor(sqa[:], qb[:], qb[:], AT.mult)
            for dc in range(DC):
                nc.tensor.matmul(ps[:], sqa[:, dc], ones_bf[:],
                                 start=(dc == 0), stop=(dc == DC - 1))
            nc.vector.tensor_copy(qnorm[:, it:it + 1], ps[:])
        if dbg:
            nc.sync.dma_start(dbg["snorm"].ap(), snrow[:])
            nc.sync.dma_start(dbg["qnorm"].ap(), qnorm[:])

    # ====== Phases 3+4: per class, SS slab -> D tiles -> gather/rec ======
    with tc.tile_pool(name="ssst", bufs=1) as ssst, \
         tc.tile_pool(name="ssps", bufs=2, space="PSUM") as ssps, \
         tc.tile_pool(name="dph", bufs=3) as dph, \
         tc.tile_pool(name="dsm", bufs=4) as dsm, \
         tc.tile_pool(name="dps", bufs=2, space="PSUM") as dps, \
         tc.tile_pool(name="tps", bufs=2, space="PSUM") as tps, \
         tc.tile_pool(name="cdp", bufs=2) as cdp, \
         tc.tile_pool(name="accp", bufs=2) as accp, \
         tc.tile_pool(name="rrp", bufs=2) as rrp, \
         tc.tile_pool(name="cmpp", bufs=2) as cmpp, \
         tc.tile_pool(name="cdg", bufs=3) as cdg:
        for c in range(WAY):
            # ---- SS slab for class c ----
            staging = ssst.tile([128, CTILES, PROW2], BF16, tag="ss_stage")
            chunks = _chunks_for_class(c)
            for j in range(CTILES):
                p0 = S * c + 128 * j
                pw = min(128, S - 128 * j)
                for (dst0, pieces) in chunks:
                    ps = ssps.tile([128, 960], F32, tag="ss_ps")
                    for (src0, doff, w) in pieces:
                        for dc2 in range(NDR):
                            nc.tensor.matmul(
                                ps[:pw, doff:doff + w],
                                s_embT[:, 2 * dc2:2 * dc2 + 2, p0:p0 + pw],
                                s_embT[:, 2 * dc2:2 * dc2 + 2, src0:src0 + w],
                                start=(dc2 == 0), stop=False, perf_mode=DR)
                        nc.tensor.matmul(
                            ps[:pw, doff:doff + w], ones_row[:, :pw],
                            snh[:, src0:src0 + w], start=False, stop=True)
                    nc.scalar.activation(staging[:pw, j, dst0:dst0 + 960],
                                         ps[:pw], ACTF.Copy, scale=-2.0)
            # row-norm column (col 2880) for the gathered threshold
            nc.vector.tensor_copy(staging[:, :, OW:OW + 1],
                                  pnorm[:, c * CTILES:(c + 1) * CTILES, None])
            # write region, skipping the undefined pad rows of the last tile
            nc.sync.dma_start(
                _ap(p_dram.tensor, p_dram.offset + c * RROWS * PROW2,
                    [(PROW2, 128), (128 * PROW2, CTILES - 1), (1, PROW2)]),
                staging[:, :CTILES - 1])
            nc.sync.dma_start(
                _ap(p_dram.tensor,
                    p_dram.offset + (c * RROWS + (CTILES - 1) * 128) * PROW2,
                    [(PROW2, 80), (1, PROW2)]),
                staging[:80, CTILES - 1])
            # ---- D tiles ----
            for it in range(ITILES):
                d_bf = dph.tile([128, RROWS], BF16, tag="d_bf")
                for sc in range(2):
                    ps = dps.tile([128, 360], F32, tag="d_ps", name="dps")
                    s0 = c * S + sc * 360
                    for dc2 in range(NDR):
                        nc.tensor.matmul(
                            ps[:], q_embT[:, 2 * dc2:2 * dc2 + 2, it * 128:(it + 1) * 128],
                            s_embT[:, 2 * dc2:2 * dc2 + 2, s0:s0 + 360],
                            start=(dc2 == 0), stop=False, perf_mode=DR)
                    nc.tensor.matmul(ps[:], ones_row[:], snh[:, s0:s0 + 360],
                                     start=False, stop=True)
                    nc.scalar.activation(d_bf[:, sc * 360:(sc + 1) * 360], ps[:],
                                         ACTF.Sqrt, bias=qnorm[:, it:it + 1],
                                         scale=-2.0)
                nc.vector.memset(d_bf[:, S:RROWS], 0.0)
                # reductions
                nc.vector.tensor_reduce(
                    m16a[:, it], d_bf[:, :S].rearrange("p (a b) -> p b a", b=16),
                    X, AT.max)
                nc.vector.tensor_reduce(dmax_all[:, it, c:c + 1], m16a[:, it],
                                        X, AT.max)
                mx8 = dsm.tile([128, 8], F32, tag="mx8")
                ix8 = dsm.tile([128, 8], U32, tag="ix8")
                nc.vector.tensor_copy(
                    mx8[:], dmax_all[:, it, c:c + 1].to_broadcast((128, 8)))
                nc.vector.max_index(ix8[:], mx8[:], d_bf[:, :S])
                posf = dsm.tile([128, 1], F32, tag="posf")
                nc.vector.tensor_scalar(posf[:], ix8[:, 0:1], 0.0, None, AT.add)
                nc.vector.tensor_copy(pos16[:, it, c:c + 1], posf[:])
                # transpose into dT (fp8)
                psT = tps.tile([128, CTILES * 128], BF16, tag="psT")
                for j in range(CTILES):
                    nc.tensor.matmul(psT[:, j * 128:(j + 1) * 128],
                                     d_bf[:, j * 128:(j + 1) * 128], ident[:],
                                     start=True, stop=True, is_transpose=True)
                if it % 2 == 0:
                    nc.vector.tensor_copy(
                        dT[c][:, :, it * 128:(it + 1) * 128],
                        psT[:].rearrange("p (j q) -> p j q", j=CTILES))
                else:
                    nc.scalar.activation(
                        dT[c][:, :, it * 128:(it + 1) * 128],
                        psT[:].rearrange("p (j q) -> p j q", j=CTILES), ACTF.Copy)
            nc.vector.memset(dT[c][:, :, R:RHAT], 0.0)
            # batched per-class stats: nave2 = -(asum/16)^2 ; dmax
            asum = dsm.tile([128, ITILES], F32, tag="asum")
            nc.vector.tensor_reduce(asum[:], m16a[:], X, AT.add)
            nc.vector.tensor_scalar(asum[:, ITILES - 1:ITILES],
                                    asum[:, ITILES - 1:ITILES], padv[:], None, AT.add)
            nc.vector.tensor_tensor(asum[:], asum[:], asum[:], AT.mult)
            nc.vector.tensor_scalar(nave2[:, :, c], asum[:], -1.0 / 256.0, None,
                                    AT.mult)
            if c == WAY - 1:
                psD = ssps.tile([WAY, NQ], F32, tag="ss_ps", name="dmps")
                for it in range(ITILES):
                    nc.tensor.matmul(psD[:], dmax_all[:, it, :], sel_sb[:, it],
                                     start=(it == 0), stop=(it == ITILES - 1))
                nc.scalar.activation(dmq[:], psD[:], ACTF.Copy, scale=1.0 / T)

            # ---- gather + rec ----
            nc.sync.dma_start(
                _ap(posw_dram.tensor, posw_dram.offset + c * 16 * ITILES * 8,
                    [(1, 8), (ITILES * 8, 16), (8, ITILES)]),
                pos16[:, :, c])
            idxs = cdp.tile([128, ITILES * 8], I16, tag="idxs")
            nc.sync.dma_start(
                idxs[:],
                _ap(posw_dram.tensor, posw_dram.offset + c * 16 * ITILES * 8,
                    [(0, 8), (ITILES * 8, 16), (1, ITILES * 8)]))
            acc = accp.tile([128, OW], BF16, tag="accb")
            nc.vector.memset(acc[:, WA:], 0.0)
            nbneg = cdp.tile([128, ITILES], F32, tag="nbneg")
            region = _ap(p_dram.tensor, p_dram.offset + c * RROWS * PROW2,
                         [(PROW2, RROWS), (1, PROW2)])
            # software-pipelined: gather(g)/nbias(g) run one stage ahead of
            # sign(g)/add(g) so the ACT sign stream never waits on DVE.
            nball = cdp.tile([128, ITILES], F32, tag="nball")
            cds = {}

            def _compare(g):
                cd = cds.pop(g)
                cmp = cmpp.tile([128, WA], BF16, tag="cmp")
                nc.scalar.activation(cmp[:], cd[:, 0, :WA], ACTF.Sign,
                                     bias=nball[:, g:g + 1])
                nc.vector.scalar_tensor_tensor(
                    acc[:, WA:], cd[:, 0, WA:OW], nbneg[:, g:g + 1], acc[:, WA:],
                    op0=AT.is_gt, op1=AT.add)
                if g == 0:
                    nc.vector.tensor_copy(acc[:, :WA], cmp[:])
                else:
                    nc.vector.tensor_tensor(acc[:, :WA], acc[:, :WA], cmp[:],
                                            AT.add)

            for g in range(ITILES):
                cd = cdg.tile([128, 1, PROW2], BF16, tag="cd")
                nc.gpsimd.dma_gather(
                    cd[:], region, idxs[:, g * 8:(g + 1) * 8],
                    128, 128, PROW2, queue_num=g % 4)
                # bias = n_i - ave^2 : sign(cd_raw + bias) = sign(d^2 - ave^2)
                nc.vector.tensor_tensor(nball[:, g:g + 1], cd[:, 0, OW:OW + 1],
                                        nave2[:, g, c:c + 1], AT.add)
                nc.vector.tensor_scalar(nbneg[:, g:g + 1], nball[:, g:g + 1],
                                        -1.0, None, AT.mult)
                cds[g] = cd
                if g >= 1:
                    _compare(g - 1)
            _compare(ITILES - 1)
            for k in range(6):
                ps = ssps.tile([1, 480], F32, tag="ss_ps", name="recps")
                nc.tensor.matmul(ps[:], ones_bf[:],
                                 acc[:, k * 480:(k + 1) * 480],
                                 start=True, stop=True)
                rc = rrp.tile([1, 480], F32, tag="recc")
                nc.scalar.activation(rc[:], ps[:], ACTF.Copy)
                nc.sync.dma_start(cc_in[c:c + 1, k * 480:(k + 1) * 480], rc[:])

    # ================= AllReduce rec =================
    if sim1:
        nc.sync.dma_start(cc_out[:, :], cc_in[:, :])
    else:
        nc.gpsimd.collective_compute(
            "AllReduce", mybir.AluOpType.add,
            replica_groups=[list(range(NCORES))],
            ins=[cc_in[:, :].opt()], outs=[cc_out[:, :].opt()])

    # ================= Phase 5: thr/mask =================
    with tc.tile_pool(name="thrp", bufs=2) as thrp, \
         tc.tile_pool(name="thrbig", bufs=1) as thrbig:
        recf = thrbig.tile([WAY, OW], F32, tag="recf")
        nc.sync.dma_start(recf[:], cc_out[:, :])
        # sign region holds sum(+-1) over 8*1920 rows; counts = (x+15360)/2
        nc.vector.tensor_scalar(recf[:, :WA], recf[:, :WA],
                                float(NCORES * RHAT), 0.5, AT.add, op1=AT.mult)
        rec = recf[:].rearrange("c (k s) -> c k s", k=WAY - 1)
        if dbg:
            with tc.tile_pool(name="dbgr", bufs=1) as dbgr:
                rg = dbgr.tile([WAY, OW], F32, tag="rg")
                nc.vector.tensor_copy(rg[:], recf[:])
                nc.sync.dma_start(dbg["rec"].ap(), rg[:])
        rsum = thrp.tile([WAY, WAY - 1], F32, tag="rsum")
        nc.vector.tensor_reduce(rsum[:], rec[:], X, AT.add)
        gt0 = thrbig.tile([WAY, WAY - 1, S], F32, tag="gt0")
        nc.vector.tensor_scalar(gt0[:], rec[:], 0.5, None, AT.is_gt)
        nz = thrp.tile([WAY, WAY - 1], F32, tag="nz")
        nc.vector.tensor_reduce(nz[:], gt0[:], X, AT.add)
        nc.vector.tensor_scalar(nz[:], nz[:], 1.0, None, AT.max)
        thr = thrp.tile([WAY, WAY - 1], F32, tag="thr")
        nc.vector.reciprocal(thr[:], nz[:])
        nc.vector.tensor_tensor(thr[:], thr[:], rsum[:], AT.mult)
        mask_slots = thrbig.tile([WAY, WAY - 1, RROWS], BF16, tag="mask_slots")
        nc.vector.memset(mask_slots[:, :, S:RROWS], 0.0)
        nc.vector.tensor_tensor(
            mask_slots[:, :, :S], rec[:],
            thr[:, :, None].to_broadcast((WAY, WAY - 1, S)), AT.is_lt)
        if dbg:
            with tc.tile_pool(name="dbgm", bufs=1) as dbgm:
                mg = dbgm.tile([WAY, WAY - 1, S], F32, tag="mg")
                nc.vector.tensor_copy(mg[:], mask_slots[:, :, :S])
                nc.sync.dma_start(dbg["mask"].ap(), mg[:])
        msum = thrp.tile([WAY, 1], F32, tag="msum")
        nc.vector.tensor_reduce(
            msum[:], mask_slots[:].rearrange("c k s -> c (k s)"), X, AT.add)
        nc.vector.tensor_scalar(msum[:], msum[:], 1.0, None, AT.max)
        scv = thrp.tile([WAY, 1], F32, tag="scv")
        nc.vector.reciprocal(scv[:], msum[:])
        nc.vector.tensor_scalar(scv[:], scv[:], 1.0 / (4.0 * T), None, AT.mult)
        nc.sync.dma_start(mask_dram[:, :, :], mask_slots[:])

        # ============= Phase 6: contrast sums + finals =============
        with tc.tile_pool(name="p6", bufs=1) as p6, \
             tc.tile_pool(name="p6ps", bufs=1, space="PSUM") as p6ps:
            maskT = p6.tile([128, WAY * CTILES, WAY], BF16, tag="maskT")
            nc.vector.memset(maskT[:], 0.0)
            for c in range(WAY):
                if c > 0:
                    nc.sync.dma_start(
                        maskT[:, 0:c * CTILES, c],
                        _ap(mask_dram.tensor,
                            mask_dram.offset + c * (WAY - 1) * RROWS,
                            [(1, 128), (128, c * CTILES)]))
                if c < WAY - 1:
                    nc.sync.dma_start(
                        maskT[:, (c + 1) * CTILES:WAY * CTILES, c],
                        _ap(mask_dram.tensor,
                            mask_dram.offset + (c * (WAY - 1) + c) * RROWS,
                            [(1, 128), (128, (WAY - 1 - c) * CTILES)]))
            psC = [p6ps.tile([WAY, 480], F32, tag=f"ct_ps{qc}", name=f"ctps{qc}")
                   for qc in range(4)]
            for cr in range(WAY):
                for j in range(CTILES):
                    for qc in range(4):
                        nc.tensor.matmul(
                            psC[qc][:], maskT[:, cr * CTILES + j, :],
                            dT[cr][:, j, qc * 480:(qc + 1) * 480],
                            start=(cr == 0 and j == 0),
                            stop=(cr == WAY - 1 and j == CTILES - 1))
            ctrows = p6.tile([WAY, RHAT], F32, tag="ctrows")
            for qc in range(4):
                nc.scalar.activation(ctrows[:, qc * 480:(qc + 1) * 480], psC[qc][:],
                                     ACTF.Copy)
            if dbg:
                nc.sync.dma_start(dbg["ct"].ap(), ctrows[:])
            ctq = p6.tile([WAY, NQ], F32, tag="ctq")
            nc.vector.tensor_reduce(
                ctq[:], ctrows[:].rearrange("c (s q) -> c q s", q=NQ), X, AT.add)
            nc.vector.tensor_scalar(ctq[:], ctq[:], scv[:], None, AT.mult)

            if dbg:
                nc.sync.dma_start(dbg["dmax"].ap(), dmax_all[:])
                nc.sync.dma_start(dbg["nave2"].ap(), nave2[:])
                with tc.tile_pool(name="dbgp", bufs=1) as dbgp:
                    pf = dbgp.tile([128, ITILES, WAY], F32, tag="pf")
                    nc.vector.tensor_copy(pf[:], pos16[:])
                    nc.sync.dma_start(dbg["pos"].ap(), pf[:])

            ssum = p6.tile([WAY, NQ], F32, tag="ssum")
            nc.vector.tensor_tensor(ssum[:], dmq[:], ctq[:], AT.add)
            rcp = p6.tile([WAY, NQ], F32, tag="rcp")
            nc.vector.reciprocal(rcp[:], ssum[:])
            lg = p6.tile([WAY, NQ], F32, tag="lg")
            nc.vector.tensor_tensor(lg[:], dmq[:], rcp[:], AT.mult)
            nc.sync.dma_start(_ap(out_d, 0, [(1, WAY), (WAY, NQ)]), dmq[:])
            nc.sync.dma_start(_ap(out_d, NQ * WAY, [(1, WAY), (WAY, NQ)]), lg[:])

    dtp.release()
    persist.release()
    dram.release()


# ---------------- host side ----------------

def _sel_host():
    sel = np.zeros((ITILES, 128, NQ), np.float32)
    for i in range(R):
        sel[i // 128, i % 128, i % NQ] = 1.0
    return sel


def _prep_inputs(support_set, queries, support_labels, W, b):
    import ml_dtypes
    f8 = ml_dtypes.float8_e4m3fn
    support_set = np.asarray(support_set, dtype=np.float32)
    queries = np.asarray(queries, dtype=np.float32)
    labels = np.asarray(support_labels).astype(np.int64)
    W = np.asarray(W, dtype=np.float32)
    b = np.asarray(b, dtype=np.float32)
    assert not np.any(b), "kernel built without bias support (reference b==0)"
    order = np.argsort(labels, kind="stable")
    support_sorted = support_set[order]

    # wT [2, 8, 128, 2, 1152]: wT[half, kc2, p, h2, d] =
    #   64*W[d, half*2048 + kc2*256 + h2*128 + p]
    w8 = (W * 64.0).astype(f8)                     # [1152, 4096]
    wT = np.ascontiguousarray(
        w8.reshape(DOUT, 2, 8, 2, 128).transpose(1, 2, 4, 3, 0))

    s8 = support_sorted.astype(f8)                 # [80, 10, 2048]
    sd = np.ascontiguousarray(
        s8.reshape(80, SEQ_LEN, 16, 128).transpose(3, 2, 1, 0)
          .reshape(128, 16, SEQ_LEN * 80))
    q8 = queries.astype(f8)                        # [320, 10, 2048]
    sel = _sel_host()
    padv = np.zeros((128, 1), np.float32)
    padv[8:] = 1.0e15
    ident = np.eye(128).astype(ml_dtypes.bfloat16)
    cconst = np.zeros((WAY, 2), np.float32)
    for c in range(WAY):
        if c in SIGN_CLASSES:
            cconst[c] = (NCORES * RHAT, 0.5)
        else:
            cconst[c] = (0.0, 1.0)
    out = []
    for k in range(NCORES):
        qk = q8[k * NQ:(k + 1) * NQ]               # [40, 10, 2048]
        qd = np.ascontiguousarray(
            qk.reshape(NQ, SEQ_LEN, 16, 128).transpose(3, 2, 1, 0)
              .reshape(128, 16, SEQ_LEN * NQ))
        out.append({
            "qd": qd,
            "sd": sd,
            "wT": wT,
            "ident": ident,
            "sel": sel,
            "padv": padv,
            "cconst": cconst,
        })
    return out


def kernel(**inputs):
    per_core = _prep_inputs(**inputs)
    if "nc" not in _CACHE:
        _CACHE["nc"] = build(debug=bool(os.environ.get("BIMACL_DEBUG")))
    nc = _CACHE["nc"]
    res = run_bass_kernel_spmd(nc, per_core, core_ids=list(range(NCORES)))
    _CACHE["last_results"] = res
    full = np.concatenate([res.results[k]["out"] for k in range(NCORES)], axis=1)
    return np.ascontiguousarray(full.astype(np.float32))


# revision 37
# speedup vs baseline: 3.1990x; 1.0022x over previous
"""Trainium2 Bass kernel for nn_CNN_BiMACL_31860067401819 (retrieval_knn).

Self-contained: hardcodes all shapes/sharding. kernel(**inputs) accepts FULL
inputs keyed as in setup_inputs(), shards queries across 8 NeuronCores
(data-parallel over the query axis), and returns the FULL [2, 320, 5] f32
output. The only collective is an AllReduce of the per-class `rec` counts.

Design (v2):
- Frame-factorized embeddings: emb(tuple t=(f1,f2)) = relu(W1^T x_f1 +
  W2^T x_f2); per-frame half-products are computed once with fp8 DoubleRow
  matmuls, tuples assembled with bf16 adds + ACT relu into fp8 embeddings.
- All distance matmuls fp8 + DoubleRow (256-deep contraction per instr).
- SS (support-support) stays in d^2 space: psum = s_i.s_j - sn_j/2 (column
  norm folded in via a 1-row matmul), scaled by -2 on the psum->SBUF copy;
  row norm rides along as an extra gathered column. rec compare is then
  cd_raw + n_i > ave^2  <=>  d^2 > ave^2 (no sqrt for SS at all).
- D (query-support): psum = q.s - sn/2; ACT Sqrt(scale=-2, bias=qnorm)
  emits bf16 distances directly.
- rec compare+accumulate: ACT Sign for SIGN_CLASSES (affine-corrected after
  the AllReduce), fused DVE scalar_tensor_tensor (is_gt,add) for the rest.
- Phase-6 masked row sums via PE transpose of D (stored fp8) + mask matmuls.
"""
import os
from itertools import combinations

import numpy as np

import concourse.bass as bass
import concourse.tile as tile
from concourse import bacc, mybir
from concourse.bass_utils import run_bass_kernel_spmd

# ---- static problem config ----
WAY, SHOT, SEQ_LEN, TSS = 5, 16, 10, 2
DIN, DOUT = 2048, 1152
N_QUERIES = 320
T = 45
S = SHOT * T                 # 720 support tuples per class
SALL = WAY * S               # 3600
NCORES = 8
NQ = N_QUERIES // NCORES     # 40
R = NQ * T                   # 1800 valid rows/core
RHAT = 1920                  # 15*128 padded rows
ITILES = RHAT // 128         # 15
TUPLES = np.array(list(combinations(range(SEQ_LEN), TSS)), dtype=np.int32)
DC = DOUT // 128             # 9
NDR = 5                      # DoubleRow matmuls per padded 1280 contraction (5*256)
OW = (WAY - 1) * S           # 2880 other-class columns
PROW2 = 2944                 # per-class region row pitch (2880 data + norm + pad)
CTILES = 6                   # 128-row tiles per class region (768 rows)
RROWS = CTILES * 128         # 768
SIGN_CLASSES = (0, 1, 2, 3, 4)  # rec compare on ACT Sign
WA = 2240                    # compare width on ACT; rest on DVE stt

F32 = mybir.dt.float32
BF16 = mybir.dt.bfloat16
F8 = mybir.dt.float8e4
U32 = mybir.dt.uint32
I16 = mybir.dt.int16
DR = mybir.MatmulPerfMode.DoubleRow

_CACHE = {}


def _ap(tensor, offset, dims):
    return bass.AP(tensor=tensor, offset=offset, ap=[list(d) for d in dims])


def _chunks_for_class(c):
    """960-wide dst chunks over the 2880 other-class cols of class c, each
    split into <=480-wide matmul pieces (PSUM-bank limit).
    Returns list of (dst_off, [(src_col, dst_delta, width), ...])."""
    spans = []
    if c > 0:
        spans.append((0, 0, S * c))              # (dst0, src0, len)
    spans.append((S * c, S * (c + 1), OW - S * c))
    out = []
    for dst0 in range(0, OW, 960):
        pieces = []
        for sub in range(2):
            w0 = dst0 + sub * 480
            for sd, ss, ln in spans:
                lo = max(w0, sd)
                hi = min(w0 + 480, sd + ln)
                if lo < hi:
                    pieces.append((ss + (lo - sd), lo - dst0, hi - lo))
        out.append((dst0, pieces))
    return out


def build(debug=False, sim1=False):
    nc = bacc.Bacc(num_swdge_queues=4)
    qd_d = nc.dram_tensor("qd", [128, 16, SEQ_LEN * NQ], F8, kind="ExternalInput")
    sd_d = nc.dram_tensor("sd", [128, 16, SEQ_LEN * 80], F8, kind="ExternalInput")
    w_d = nc.dram_tensor("wT", [2, 8, 128, 2, DOUT], F8, kind="ExternalInput")
    id_d = nc.dram_tensor("ident", [128, 128], BF16, kind="ExternalInput")
    sel_d = nc.dram_tensor("sel", [ITILES, 128, NQ], F32, kind="ExternalInput")
    padv_d = nc.dram_tensor("padv", [128, 1], F32, kind="ExternalInput")
    cc_d = nc.dram_tensor("cconst", [WAY, 2], F32, kind="ExternalInput")
    out_d = nc.dram_tensor("out", [2, NQ, WAY], F32, kind="ExternalOutput")
    dbg = {}
    if debug:
        dbg["qemb"] = nc.dram_tensor("dbg_qemb", [128, DC, RHAT], F32, kind="ExternalOutput")
        dbg["semb"] = nc.dram_tensor("dbg_semb", [128, DC, SALL], F32, kind="ExternalOutput")
        dbg["snorm"] = nc.dram_tensor("dbg_snorm", [1, SALL], F32, kind="ExternalOutput")
        dbg["qnorm"] = nc.dram_tensor("dbg_qnorm", [128, ITILES], F32, kind="ExternalOutput")
        dbg["rec"] = nc.dram_tensor("dbg_rec", [WAY, OW], F32, kind="ExternalOutput")
        dbg["mask"] = nc.dram_tensor("dbg_mask", [WAY, WAY - 1, S], F32, kind="ExternalOutput")
        dbg["dmax"] = nc.dram_tensor("dbg_dmax", [128, ITILES, WAY], F32, kind="ExternalOutput")
        dbg["nave2"] = nc.dram_tensor("dbg_nave2", [128, ITILES, WAY], F32, kind="ExternalOutput")
        dbg["pos"] = nc.dram_tensor("dbg_pos", [128, ITILES, WAY], F32, kind="ExternalOutput")
        dbg["ct"] = nc.dram_tensor("dbg_ct", [WAY, RHAT], F32, kind="ExternalOutput")

    with tile.TileContext(nc) as tc:
        _body(nc, tc, qd_d, sd_d, w_d, id_d, sel_d, padv_d, cc_d, out_d, dbg, sim1)
    nc.finalize()
    return nc


def _body(nc, tc, qd_d, sd_d, w_d, id_d, sel_d, padv_d, cc_d, out_d, dbg, sim1):
    AT = mybir.AluOpType
    ACTF = mybir.ActivationFunctionType
    X = mybir.AxisListType.X

    persist = tc.alloc_tile_pool(name="persist", bufs=1)
    dram = tc.alloc_tile_pool(name="dram", bufs=1, space="DRAM")

    # DRAM scratch
    p_dram = dram.tile([WAY, RROWS, PROW2], BF16, tag="p_scratch")
    posw_dram = dram.tile([WAY, 16, ITILES * 8], I16, tag="posw")
    snorm_dram = dram.tile([1, 3840], F32, tag="snormd")
    mask_dram = dram.tile([WAY, WAY - 1, RROWS], BF16, tag="maskd")
    cc_in = dram.tile([WAY, OW], F32, tag="cc_in")
    cc_out = dram.tile([WAY, OW], F32, tag="cc_out")

    # persistent SBUF
    q_embT = persist.tile([128, DC + 1, RHAT], F8, tag="q_embT")
    s_embT = persist.tile([128, DC + 1, SALL], F8, tag="s_embT")
    snh = persist.tile([1, SALL], BF16, tag="snh")          # -snorm/2
    qnorm = persist.tile([128, ITILES], F32, tag="qnorm")
    pnorm = persist.tile([128, WAY * CTILES], F32, tag="pnorm")
    m16a = persist.tile([128, ITILES, 16], F32, tag="m16a")
    dmax_all = persist.tile([128, ITILES, WAY], F32, tag="dmax_all")
    nave2 = persist.tile([128, ITILES, WAY], F32, tag="nave2")
    pos16 = persist.tile([128, ITILES, WAY], I16, tag="pos16")
    ident = persist.tile([128, 128], BF16, tag="ident")
    ones_bf = persist.tile([128, 1], BF16, tag="ones_bf")
    ones_f = persist.tile([128, 1], F32, tag="ones_f")
    ones_row = persist.tile([1, 128], BF16, tag="ones_row")
    padv = persist.tile([128, 1], F32, tag="padv")
    cconst = persist.tile([WAY, 2], F32, tag="cconst")
    sel_sb = persist.tile([128, ITILES, NQ], F32, tag="sel_sb")
    dmq = persist.tile([WAY, NQ], F32, tag="dmq")

    nc.vector.memset(ones_bf[:], 1.0)
    nc.vector.memset(ones_f[:], 1.0)
    nc.vector.memset(ones_row[:], 1.0)
    nc.sync.dma_start(padv[:], padv_d[:, :])
    nc.sync.dma_start(ident[:], id_d[:, :])
    nc.sync.dma_start(cconst[:], cc_d[:, :])
    nc.sync.dma_start(sel_sb[:], sel_d.rearrange("t p q -> p t q"))
    nc.vector.memset(q_embT[:, :, R:RHAT], 0.0)
    nc.vector.memset(q_embT[:, DC], 0.0)
    nc.vector.memset(s_embT[:, DC], 0.0)

    # ================= Phase 1: per-frame half products + tuple assembly ====
    with tc.tile_pool(name="emb", bufs=1) as emb, \
         tc.tile_pool(name="embsm", bufs=4) as embsm, \
         tc.tile_pool(name="embps", bufs=6, space="PSUM") as embps:
        wT = emb.tile([128, 2, 8, 2, DOUT], F8, tag="wT")
        nc.sync.dma_start(wT[:], w_d.rearrange("a b p c d -> p a b c d"))
        qd = emb.tile([128, 16, SEQ_LEN * NQ], F8, tag="qd")
        nc.sync.dma_start(qd[:], qd_d[:, :, :])
        sd = emb.tile([128, 16, SEQ_LEN * 80], F8, tag="sd")
        nc.sync.dma_start(sd[:], sd_d[:, :, :])
        Pq = emb.tile([128, DC, 2, SEQ_LEN * NQ], BF16, tag="Pq")
        Ps = emb.tile([128, DC, 2, SEQ_LEN * 80], BF16, tag="Ps")

        # s-side first: its embeddings gate snorm -> SS -> gathers
        for half in range(2):
            for dc in range(DC):
                for ch in range(2):
                    ps2 = embps.tile([128, SEQ_LEN * NQ], F32, tag="emb_ps")
                    for kc2 in range(8):
                        nc.tensor.matmul(
                            ps2[:], wT[:, half, kc2, :, dc * 128:(dc + 1) * 128],
                            sd[:, 2 * kc2:2 * kc2 + 2, ch * 400:(ch + 1) * 400],
                            start=(kc2 == 0), stop=(kc2 == 7), perf_mode=DR)
                    nc.vector.tensor_copy(
                        Ps[:, dc, half, ch * 400:(ch + 1) * 400], ps2[:])
        for t in range(T):
            f1, f2 = int(TUPLES[t][0]), int(TUPLES[t][1])
            pres = embsm.tile([128, DC, 80], BF16, tag="pres")
            nc.vector.tensor_tensor(
                pres[:], Ps[:, :, 0, f1 * 80:(f1 + 1) * 80],
                Ps[:, :, 1, f2 * 80:(f2 + 1) * 80], AT.add)
            dst = s_embT[:, :DC].rearrange("p d (u t) -> p d t u", t=T)[:, :, t]
            if t % 2 == 0:
                nc.scalar.activation(dst, pres[:], ACTF.Relu, scale=1.0 / 64.0)
            else:
                nc.vector.tensor_scalar(dst, pres[:], 0.0, 1.0 / 64.0,
                                        AT.max, op1=AT.mult)
        # q side
        for half in range(2):
            for dc in range(DC):
                ps = embps.tile([128, SEQ_LEN * NQ], F32, tag="emb_ps")
                for kc2 in range(8):
                    nc.tensor.matmul(
                        ps[:], wT[:, half, kc2, :, dc * 128:(dc + 1) * 128],
                        qd[:, 2 * kc2:2 * kc2 + 2, :],
                        start=(kc2 == 0), stop=(kc2 == 7), perf_mode=DR)
                nc.vector.tensor_copy(Pq[:, dc, half], ps[:])
        for t in range(T):
            f1, f2 = int(TUPLES[t][0]), int(TUPLES[t][1])
            preq = embsm.tile([128, DC, NQ], BF16, tag="preq")
            nc.vector.tensor_tensor(
                preq[:], Pq[:, :, 0, f1 * NQ:(f1 + 1) * NQ],
                Pq[:, :, 1, f2 * NQ:(f2 + 1) * NQ], AT.add)
            if t % 2 == 0:
                nc.scalar.activation(q_embT[:, :DC, t * NQ:(t + 1) * NQ], preq[:],
                                     ACTF.Relu, scale=1.0 / 64.0)
            else:
                nc.vector.tensor_scalar(q_embT[:, :DC, t * NQ:(t + 1) * NQ],
                                        preq[:], 0.0, 1.0 / 64.0,
                                        AT.max, op1=AT.mult)

    # dT allocated after the emb pool frees wT/Pq/Ps space
    dtp = tc.alloc_tile_pool(name="dtp", bufs=1)
    dT = [dtp.tile([128, CTILES, RHAT], F8, tag=f"dT{c}", name=f"dT{c}")
          for c in range(WAY)]

    if dbg:
        with tc.tile_pool(name="dbge", bufs=1) as dbge:
            t1 = dbge.tile([128, DC, RHAT], F32, tag="dbq")
            nc.vector.tensor_copy(t1[:], q_embT[:, :DC])
            nc.sync.dma_start(dbg["qemb"].ap(), t1[:])
            t2 = dbge.tile([128, DC, SALL], F32, tag="dbs")
            nc.vector.tensor_copy(t2[:], s_embT[:, :DC])
            nc.sync.dma_start(dbg["semb"].ap(), t2[:])

    # ================= Phase 2: norms =================
    with tc.tile_pool(name="nrm", bufs=2) as nrm, \
         tc.tile_pool(name="nrmps", bufs=2, space="PSUM") as nrmps:
        snrow = nrm.tile([1, SALL], F32, tag="snrow")
        for scn in range(8):
            ps = nrmps.tile([1, 450], F32, tag="sn_ps")
            for dc in range(DC):
                sq = nrm.tile([128, 450], BF16, tag="sn_sqb")
                nc.scalar.activation(sq[:], s_embT[:, dc, scn * 450:(scn + 1) * 450],
                                     ACTF.Square)
                nc.tensor.matmul(ps[:], ones_bf[:], sq[:],
                                 start=(dc == 0), stop=(dc == DC - 1))
            nc.scalar.activation(snrow[:, scn * 450:(scn + 1) * 450], ps[:], ACTF.Copy)
        nc.vector.tensor_scalar(snh[:], snrow[:], -0.5, None, AT.mult)
        nc.sync.dma_start(snorm_dram[:, :SALL], snrow[:])
        # pnorm[p, c*6+j] = snorm[720c + 128j + p]
        for c in range(WAY):
            nc.sync.dma_start(
                pnorm[:, c * CTILES:(c + 1) * CTILES],
                _ap(snorm_dram.tensor, snorm_dram.offset + c * S,
                    [(1, 128), (128, CTILES)]))
        for it in range(ITILES):
            ps = nrmps.tile([128, 1], F32, tag="qn_ps", name="qnps")
            sqa = nrm.tile([128, DC, 128], BF16, tag="qn_sqb")
            qb = nrm.tile([128, DC, 128], BF16, tag="qn_qb")
            nc.vector.tensor_copy(qb[:], q_embT[:, :DC, it * 128:(it + 1) * 128])
            nc.vector.tensor_tensor(sqa[:], qb[:], qb[:], AT.mult)
            for dc in range(DC):
                nc.tensor.matmul(ps[:], sqa[:, dc], ones_bf[:],
                                 start=(dc == 0), stop=(dc == DC - 1))
            nc.vector.tensor_copy(qnorm[:, it:it + 1], ps[:])
        if dbg:
            nc.sync.dma_start(dbg["snorm"].ap(), snrow[:])
            nc.sync.dma_start(dbg["qnorm"].ap(), qnorm[:])

    # ====== Phases 3+4: per class, SS slab -> D tiles -> gather/rec ======
    with tc.tile_pool(name="ssst", bufs=1) as ssst, \
         tc.tile_pool(name="ssps", bufs=2, space="PSUM") as ssps, \
         tc.tile_pool(name="dph", bufs=3) as dph, \
         tc.tile_pool(name="dsm", bufs=6) as dsm, \
         tc.tile_pool(name="dps", bufs=2, space="PSUM") as dps, \
         tc.tile_pool(name="tps", bufs=2, space="PSUM") as tps, \
         tc.tile_pool(name="cdp", bufs=2) as cdp, \
         tc.tile_pool(name="accp", bufs=2) as accp, \
         tc.tile_pool(name="rrp", bufs=2) as rrp, \
         tc.tile_pool(name="cmpp", bufs=2) as cmpp, \
         tc.tile_pool(name="cdg", bufs=3) as cdg:
        for c in range(WAY):
            # ---- SS slab for class c ----
            staging = ssst.tile([128, CTILES, PROW2], BF16, tag="ss_stage")
            chunks = _chunks_for_class(c)
            for j in range(CTILES):
                p0 = S * c + 128 * j
                pw = min(128, S - 128 * j)
                for (dst0, pieces) in chunks:
                    ps = ssps.tile([128, 960], F32, tag="ss_ps")
                    for (src0, doff, w) in pieces:
                        for dc2 in range(NDR):
                            nc.tensor.matmul(
                                ps[:pw, doff:doff + w],
                                s_embT[:, 2 * dc2:2 * dc2 + 2, p0:p0 + pw],
                                s_embT[:, 2 * dc2:2 * dc2 + 2, src0:src0 + w],
                                start=(dc2 == 0), stop=False, perf_mode=DR)
                        nc.tensor.matmul(
                            ps[:pw, doff:doff + w], ones_row[:, :pw],
                            snh[:, src0:src0 + w], start=False, stop=True)
                    nc.scalar.activation(staging[:pw, j, dst0:dst0 + 960],
                                         ps[:pw], ACTF.Copy, scale=-2.0)
            # row-norm column (col 2880) for the gathered threshold
            nc.vector.tensor_copy(staging[:, :, OW:OW + 1],
                                  pnorm[:, c * CTILES:(c + 1) * CTILES, None])
            # write region, skipping the undefined pad rows of the last tile
            nc.sync.dma_start(
                _ap(p_dram.tensor, p_dram.offset + c * RROWS * PROW2,
                    [(PROW2, 128), (128 * PROW2, CTILES - 1), (1, PROW2)]),
                staging[:, :CTILES - 1])
            nc.sync.dma_start(
                _ap(p_dram.tensor,
                    p_dram.offset + (c * RROWS + (CTILES - 1) * 128) * PROW2,
                    [(PROW2, 80), (1, PROW2)]),
                staging[:80, CTILES - 1])
            # ---- D tiles ----
            for it in range(ITILES):
                d_bf = dph.tile([128, RROWS], BF16, tag="d_bf")
                for sc in range(2):
                    ps = dps.tile([128, 360], F32, tag="d_ps", name="dps")
                    s0 = c * S + sc * 360
                    for dc2 in range(NDR):
                        nc.tensor.matmul(
                            ps[:], q_embT[:, 2 * dc2:2 * dc2 + 2, it * 128:(it + 1) * 128],
                            s_embT[:, 2 * dc2:2 * dc2 + 2, s0:s0 + 360],
                            start=(dc2 == 0), stop=False, perf_mode=DR)
                    nc.tensor.matmul(ps[:], ones_row[:], snh[:, s0:s0 + 360],
                                     start=False, stop=True)
                    nc.scalar.activation(d_bf[:, sc * 360:(sc + 1) * 360], ps[:],
                                         ACTF.Sqrt, bias=qnorm[:, it:it + 1],
                                         scale=-2.0)
                nc.vector.memset(d_bf[:, S:RROWS], 0.0)
                # reductions
                nc.vector.tensor_reduce(
                    m16a[:, it], d_bf[:, :S].rearrange("p (a b) -> p b a", b=16),
                    X, AT.max)
                nc.vector.tensor_reduce(dmax_all[:, it, c:c + 1], m16a[:, it],
                                        X, AT.max)
                mx8 = dsm.tile([128, 8], F32, tag="mx8")
                ix8 = dsm.tile([128, 8], U32, tag="ix8")
                nc.vector.tensor_copy(
                    mx8[:], dmax_all[:, it, c:c + 1].to_broadcast((128, 8)))
                nc.vector.max_index(ix8[:], mx8[:], d_bf[:, :S])
                posf = dsm.tile([128, 1], F32, tag="posf")
                nc.vector.tensor_scalar(posf[:], ix8[:, 0:1], 0.0, None, AT.add)
                nc.vector.tensor_copy(pos16[:, it, c:c + 1], posf[:])
                # transpose into dT (fp8)
                psT = tps.tile([128, CTILES * 128], BF16, tag="psT")
                for j in range(CTILES):
                    nc.tensor.matmul(psT[:, j * 128:(j + 1) * 128],
                                     d_bf[:, j * 128:(j + 1) * 128], ident[:],
                                     start=True, stop=True, is_transpose=True)
                if it % 2 == 0:
                    nc.vector.tensor_copy(
                        dT[c][:, :, it * 128:(it + 1) * 128],
                        psT[:].rearrange("p (j q) -> p j q", j=CTILES))
                else:
                    nc.scalar.activation(
                        dT[c][:, :, it * 128:(it + 1) * 128],
                        psT[:].rearrange("p (j q) -> p j q", j=CTILES), ACTF.Copy)
            nc.vector.memset(dT[c][:, :, R:RHAT], 0.0)
            # batched per-class stats: nave2 = -(asum/16)^2 ; dmax
            asum = dsm.tile([128, ITILES], F32, tag="asum")
            nc.vector.tensor_reduce(asum[:], m16a[:], X, AT.add)
            nc.vector.tensor_scalar(asum[:, ITILES - 1:ITILES],
                                    asum[:, ITILES - 1:ITILES], padv[:], None, AT.add)
            nc.vector.tensor_tensor(asum[:], asum[:], asum[:], AT.mult)
            nc.vector.tensor_scalar(nave2[:, :, c], asum[:], -1.0 / 256.0, None,
                                    AT.mult)
            if c == WAY - 1:
                psD = ssps.tile([WAY, NQ], F32, tag="ss_ps", name="dmps")
                for it in range(ITILES):
                    nc.tensor.matmul(psD[:], dmax_all[:, it, :], sel_sb[:, it],
                                     start=(it == 0), stop=(it == ITILES - 1))
                nc.scalar.activation(dmq[:], psD[:], ACTF.Copy, scale=1.0 / T)

            # ---- gather + rec ----
            nc.sync.dma_start(
                _ap(posw_dram.tensor, posw_dram.offset + c * 16 * ITILES * 8,
                    [(1, 8), (ITILES * 8, 16), (8, ITILES)]),
                pos16[:, :, c])
            idxs = cdp.tile([128, ITILES * 8], I16, tag="idxs")
            nc.sync.dma_start(
                idxs[:],
                _ap(posw_dram.tensor, posw_dram.offset + c * 16 * ITILES * 8,
                    [(0, 8), (ITILES * 8, 16), (1, ITILES * 8)]))
            acc = accp.tile([128, OW], BF16, tag="accb")
            nc.vector.memset(acc[:, WA:], 0.0)
            nbneg = cdp.tile([128, ITILES], F32, tag="nbneg")
            region = _ap(p_dram.tensor, p_dram.offset + c * RROWS * PROW2,
                         [(PROW2, RROWS), (1, PROW2)])
            # software-pipelined: gather(g)/nbias(g) run one stage ahead of
            # sign(g)/add(g) so the ACT sign stream never waits on DVE.
            nball = cdp.tile([128, ITILES], F32, tag="nball")
            cds = {}

            def _compare(g):
                cd = cds.pop(g)
                cmp = cmpp.tile([128, WA], BF16, tag="cmp")
                nc.scalar.activation(cmp[:], cd[:, 0, :WA], ACTF.Sign,
                                     bias=nball[:, g:g + 1])
                nc.vector.scalar_tensor_tensor(
                    acc[:, WA:], cd[:, 0, WA:OW], nbneg[:, g:g + 1], acc[:, WA:],
                    op0=AT.is_gt, op1=AT.add)
                if g == 0:
                    nc.vector.tensor_copy(acc[:, :WA], cmp[:])
                else:
                    nc.vector.tensor_tensor(acc[:, :WA], acc[:, :WA], cmp[:],
                                            AT.add)

            for g in range(ITILES):
                cd = cdg.tile([128, 1, PROW2], BF16, tag="cd")
                nc.gpsimd.dma_gather(
                    cd[:], region, idxs[:, g * 8:(g + 1) * 8],
                    128, 128, PROW2, queue_num=g % 4)
                # bias = n_i - ave^2 : sign(cd_raw + bias) = sign(d^2 - ave^2)
                nc.vector.tensor_tensor(nball[:, g:g + 1], cd[:, 0, OW:OW + 1],
                                        nave2[:, g, c:c + 1], AT.add)
                nc.vector.tensor_scalar(nbneg[:, g:g + 1], nball[:, g:g + 1],
                                        -1.0, None, AT.mult)
                cds[g] = cd
                if g >= 1:
                    _compare(g - 1)
            _compare(ITILES - 1)
            for k in range(6):
                ps = ssps.tile([1, 480], F32, tag="ss_ps", name="recps")
                nc.tensor.matmul(ps[:], ones_bf[:],
                                 acc[:, k * 480:(k + 1) * 480],
                                 start=True, stop=True)
                rc = rrp.tile([1, 480], F32, tag="recc")
                nc.scalar.activation(rc[:], ps[:], ACTF.Copy)
                nc.sync.dma_start(cc_in[c:c + 1, k * 480:(k + 1) * 480], rc[:])

    # ================= AllReduce rec =================
    if sim1:
        nc.sync.dma_start(cc_out[:, :], cc_in[:, :])
    else:
        nc.gpsimd.collective_compute(
            "AllReduce", mybir.AluOpType.add,
            replica_groups=[list(range(NCORES))],
            ins=[cc_in[:, :].opt()], outs=[cc_out[:, :].opt()])

    # ================= Phase 5: thr/mask =================
    with tc.tile_pool(name="thrp", bufs=2) as thrp, \
         tc.tile_pool(name="thrbig", bufs=1) as thrbig:
        recf = thrbig.tile([WAY, OW], F32, tag="recf")
        nc.sync.dma_start(recf[:], cc_out[:, :])
        # sign region holds sum(+-1) over 8*1920 rows; counts = (x+15360)/2
        nc.vector.tensor_scalar(recf[:, :WA], recf[:, :WA],
                                float(NCORES * RHAT), 0.5, AT.add, op1=AT.mult)
        rec = recf[:].rearrange("c (k s) -> c k s", k=WAY - 1)
        if dbg:
            with tc.tile_pool(name="dbgr", bufs=1) as dbgr:
                rg = dbgr.tile([WAY, OW], F32, tag="rg")
                nc.vector.tensor_copy(rg[:], recf[:])
                nc.sync.dma_start(dbg["rec"].ap(), rg[:])
        rsum = thrp.tile([WAY, WAY - 1], F32, tag="rsum")
        nc.vector.tensor_reduce(rsum[:], rec[:], X, AT.add)
        gt0 = thrbig.tile([WAY, WAY - 1, S], F32, tag="gt0")
        nc.vector.tensor_scalar(gt0[:], rec[:], 0.5, None, AT.is_gt)
        nz = thrp.tile([WAY, WAY - 1], F32, tag="nz")
        nc.vector.tensor_reduce(nz[:], gt0[:], X, AT.add)
        nc.vector.tensor_scalar(nz[:], nz[:], 1.0, None, AT.max)
        thr = thrp.tile([WAY, WAY - 1], F32, tag="thr")
        nc.vector.reciprocal(thr[:], nz[:])
        nc.vector.tensor_tensor(thr[:], thr[:], rsum[:], AT.mult)
        mask_slots = thrbig.tile([WAY, WAY - 1, RROWS], BF16, tag="mask_slots")
        nc.vector.memset(mask_slots[:, :, S:RROWS], 0.0)
        nc.vector.tensor_tensor(
            mask_slots[:, :, :S], rec[:],
            thr[:, :, None].to_broadcast((WAY, WAY - 1, S)), AT.is_lt)
        if dbg:
            with tc.tile_pool(name="dbgm", bufs=1) as dbgm:
                mg = dbgm.tile([WAY, WAY - 1, S], F32, tag="mg")
                nc.vector.tensor_copy(mg[:], mask_slots[:, :, :S])
                nc.sync.dma_start(dbg["mask"].ap(), mg[:])
        msum = thrp.tile([WAY, 1], F32, tag="msum")
        nc.vector.tensor_reduce(
            msum[:], mask_slots[:].rearrange("c k s -> c (k s)"), X, AT.add)
        nc.vector.tensor_scalar(msum[:], msum[:], 1.0, None, AT.max)
        scv = thrp.tile([WAY, 1], F32, tag="scv")
        nc.vector.reciprocal(scv[:], msum[:])
        nc.vector.tensor_scalar(scv[:], scv[:], 1.0 / (4.0 * T), None, AT.mult)
        nc.sync.dma_start(mask_dram[:, :, :], mask_slots[:])

        # ============= Phase 6: contrast sums + finals =============
        with tc.tile_pool(name="p6", bufs=1) as p6, \
             tc.tile_pool(name="p6ps", bufs=1, space="PSUM") as p6ps:
            maskT = p6.tile([128, WAY * CTILES, WAY], BF16, tag="maskT")
            nc.vector.memset(maskT[:], 0.0)
            for c in range(WAY):
                if c > 0:
                    nc.sync.dma_start(
                        maskT[:, 0:c * CTILES, c],
                        _ap(mask_dram.tensor,
                            mask_dram.offset + c * (WAY - 1) * RROWS,
                            [(1, 128), (128, c * CTILES)]))
                if c < WAY - 1:
                    nc.sync.dma_start(
                        maskT[:, (c + 1) * CTILES:WAY * CTILES, c],
                        _ap(mask_dram.tensor,
                            mask_dram.offset + (c * (WAY - 1) + c) * RROWS,
                            [(1, 128), (128, (WAY - 1 - c) * CTILES)]))
            psC = [p6ps.tile([WAY, 480], F32, tag=f"ct_ps{qc}", name=f"ctps{qc}")
                   for qc in range(4)]
            for cr in range(WAY):
                for j in range(CTILES):
                    for qc in range(4):
                        nc.tensor.matmul(
                            psC[qc][:], maskT[:, cr * CTILES + j, :],
                            dT[cr][:, j, qc * 480:(qc + 1) * 480],
                            start=(cr == 0 and j == 0),
                            stop=(cr == WAY - 1 and j == CTILES - 1))
            ctrows = p6.tile([WAY, RHAT], F32, tag="ctrows")
            for qc in range(4):
                nc.scalar.activation(ctrows[:, qc * 480:(qc + 1) * 480], psC[qc][:],
                                     ACTF.Copy)
            if dbg:
                nc.sync.dma_start(dbg["ct"].ap(), ctrows[:])
            ctq = p6.tile([WAY, NQ], F32, tag="ctq")
            nc.vector.tensor_reduce(
                ctq[:], ctrows[:].rearrange("c (s q) -> c q s", q=NQ), X, AT.add)
            nc.vector.tensor_scalar(ctq[:], ctq[:], scv[:], None, AT.mult)

            if dbg:
                nc.sync.dma_start(dbg["dmax"].ap(), dmax_all[:])
                nc.sync.dma_start(dbg["nave2"].ap(), nave2[:])
                with tc.tile_pool(name="dbgp", bufs=1) as dbgp:
                    pf = dbgp.tile([128, ITILES, WAY], F32, tag="pf")
                    nc.vector.tensor_copy(pf[:], pos16[:])
                    nc.sync.dma_start(dbg["pos"].ap(), pf[:])

            ssum = p6.tile([WAY, NQ], F32, tag="ssum")
            nc.vector.tensor_tensor(ssum[:], dmq[:], ctq[:], AT.add)
            rcp = p6.tile([WAY, NQ], F32, tag="rcp")
            nc.vector.reciprocal(rcp[:], ssum[:])
            lg = p6.tile([WAY, NQ], F32, tag="lg")
            nc.vector.tensor_tensor(lg[:], dmq[:], rcp[:], AT.mult)
            nc.sync.dma_start(_ap(out_d, 0, [(1, WAY), (WAY, NQ)]), dmq[:])
            nc.sync.dma_start(_ap(out_d, NQ * WAY, [(1, WAY), (WAY, NQ)]), lg[:])

    dtp.release()
    persist.release()
    dram.release()


# ---------------- host side ----------------

def _sel_host():
    sel = np.zeros((ITILES, 128, NQ), np.float32)
    for i in range(R):
        sel[i // 128, i % 128, i % NQ] = 1.0
    return sel


def _prep_inputs(support_set, queries, support_labels, W, b):
    import ml_dtypes
    f8 = ml_dtypes.float8_e4m3fn
    support_set = np.asarray(support_set, dtype=np.float32)
    queries = np.asarray(queries, dtype=np.float32)
    labels = np.asarray(support_labels).astype(np.int64)
    W = np.asarray(W, dtype=np.float32)
    b = np.asarray(b, dtype=np.float32)
    assert not np.any(b), "kernel built without bias support (reference b==0)"
    order = np.argsort(labels, kind="stable")
    support_sorted = support_set[order]

    # wT [2, 8, 128, 2, 1152]: wT[half, kc2, p, h2, d] =
    #   64*W[d, half*2048 + kc2*256 + h2*128 + p]
    w8 = (W * 64.0).astype(f8)                     # [1152, 4096]
    wT = np.ascontiguousarray(
        w8.reshape(DOUT, 2, 8, 2, 128).transpose(1, 2, 4, 3, 0))

    s8 = support_sorted.astype(f8)                 # [80, 10, 2048]
    sd = np.ascontiguousarray(
        s8.reshape(80, SEQ_LEN, 16, 128).transpose(3, 2, 1, 0)
          .reshape(128, 16, SEQ_LEN * 80))
    q8 = queries.astype(f8)                        # [320, 10, 2048]
    sel = _sel_host()
    padv = np.zeros((128, 1), np.float32)
    padv[8:] = 1.0e15
    ident = np.eye(128).astype(ml_dtypes.bfloat16)
    cconst = np.zeros((WAY, 2), np.float32)
    for c in range(WAY):
        if c in SIGN_CLASSES:
            cconst[c] = (NCORES * RHAT, 0.5)
        else:
            cconst[c] = (0.0, 1.0)
    out = []
    for k in range(NCORES):
        qk = q8[k * NQ:(k + 1) * NQ]               # [40, 10, 2048]
        qd = np.ascontiguousarray(
            qk.reshape(NQ, SEQ_LEN, 16, 128).transpose(3, 2, 1, 0)
              .reshape(128, 16, SEQ_LEN * NQ))
        out.append({
            "qd": qd,
            "sd": sd,
            "wT": wT,
            "ident": ident,
            "sel": sel,
            "padv": padv,
            "cconst": cconst,
        })
    return out


def kernel(**inputs):
    per_core = _prep_inputs(**inputs)
    if "nc" not in _CACHE:
        _CACHE["nc"] = build(debug=bool(os.environ.get("BIMACL_DEBUG")))
    nc = _CACHE["nc"]
    res = run_bass_kernel_spmd(nc, per_core, core_ids=list(range(NCORES)))
    _CACHE["last_results"] = res
    full = np.concatenate([res.results[k]["out"] for k in range(NCORES)], axis=1)
    return np.ascontiguousarray(full.astype(np.float32))


# revision 40
# speedup vs baseline: 3.2022x; 1.0010x over previous
"""Trainium2 Bass kernel for nn_CNN_BiMACL_31860067401819 (retrieval_knn).

Self-contained: hardcodes all shapes/sharding. kernel(**inputs) accepts FULL
inputs keyed as in setup_inputs(), shards queries across 8 NeuronCores
(data-parallel over the query axis), and returns the FULL [2, 320, 5] f32
output. The only collective is an AllReduce of the per-class `rec` counts.

Design (v2):
- Frame-factorized embeddings: emb(tuple t=(f1,f2)) = relu(W1^T x_f1 +
  W2^T x_f2); per-frame half-products are computed once with fp8 DoubleRow
  matmuls, tuples assembled with bf16 adds + ACT relu into fp8 embeddings.
- All distance matmuls fp8 + DoubleRow (256-deep contraction per instr).
- SS (support-support) stays in d^2 space: psum = s_i.s_j - sn_j/2 (column
  norm folded in via a 1-row matmul), scaled by -2 on the psum->SBUF copy;
  row norm rides along as an extra gathered column. rec compare is then
  cd_raw + n_i > ave^2  <=>  d^2 > ave^2 (no sqrt for SS at all).
- D (query-support): psum = q.s - sn/2; ACT Sqrt(scale=-2, bias=qnorm)
  emits bf16 distances directly.
- rec compare+accumulate: ACT Sign for SIGN_CLASSES (affine-corrected after
  the AllReduce), fused DVE scalar_tensor_tensor (is_gt,add) for the rest.
- Phase-6 masked row sums via PE transpose of D (stored fp8) + mask matmuls.
"""
import os
from itertools import combinations

import numpy as np

import concourse.bass as bass
import concourse.tile as tile
from concourse import bacc, mybir
from concourse.bass_utils import run_bass_kernel_spmd

# ---- static problem config ----
WAY, SHOT, SEQ_LEN, TSS = 5, 16, 10, 2
DIN, DOUT = 2048, 1152
N_QUERIES = 320
T = 45
S = SHOT * T                 # 720 support tuples per class
SALL = WAY * S               # 3600
NCORES = 8
NQ = N_QUERIES // NCORES     # 40
R = NQ * T                   # 1800 valid rows/core
RHAT = 1920                  # 15*128 padded rows
ITILES = RHAT // 128         # 15
TUPLES = np.array(list(combinations(range(SEQ_LEN), TSS)), dtype=np.int32)
DC = DOUT // 128             # 9
NDR = 5                      # DoubleRow matmuls per padded 1280 contraction (5*256)
OW = (WAY - 1) * S           # 2880 other-class columns
PROW2 = 2944                 # per-class region row pitch (2880 data + norm + pad)
CTILES = 6                   # 128-row tiles per class region (768 rows)
RROWS = CTILES * 128         # 768
SIGN_CLASSES = (0, 1, 2, 3, 4)  # rec compare on ACT Sign
WA = 2304                    # compare width on ACT; rest on DVE stt

F32 = mybir.dt.float32
BF16 = mybir.dt.bfloat16
F8 = mybir.dt.float8e4
U32 = mybir.dt.uint32
I16 = mybir.dt.int16
DR = mybir.MatmulPerfMode.DoubleRow

_CACHE = {}


def _ap(tensor, offset, dims):
    return bass.AP(tensor=tensor, offset=offset, ap=[list(d) for d in dims])


def _chunks_for_class(c):
    """960-wide dst chunks over the 2880 other-class cols of class c, each
    split into <=480-wide matmul pieces (PSUM-bank limit).
    Returns list of (dst_off, [(src_col, dst_delta, width), ...])."""
    spans = []
    if c > 0:
        spans.append((0, 0, S * c))              # (dst0, src0, len)
    spans.append((S * c, S * (c + 1), OW - S * c))
    out = []
    for dst0 in range(0, OW, 960):
        pieces = []
        for sub in range(2):
            w0 = dst0 + sub * 480
            for sd, ss, ln in spans:
                lo = max(w0, sd)
                hi = min(w0 + 480, sd + ln)
                if lo < hi:
                    pieces.append((ss + (lo - sd), lo - dst0, hi - lo))
        out.append((dst0, pieces))
    return out


def build(debug=False, sim1=False):
    nc = bacc.Bacc(num_swdge_queues=4)
    qd_d = nc.dram_tensor("qd", [128, 16, SEQ_LEN * NQ], F8, kind="ExternalInput")
    sd_d = nc.dram_tensor("sd", [128, 16, SEQ_LEN * 80], F8, kind="ExternalInput")
    w_d = nc.dram_tensor("wT", [2, 8, 128, 2, DOUT], F8, kind="ExternalInput")
    id_d = nc.dram_tensor("ident", [128, 128], BF16, kind="ExternalInput")
    sel_d = nc.dram_tensor("sel", [ITILES, 128, NQ], F32, kind="ExternalInput")
    padv_d = nc.dram_tensor("padv", [128, 1], F32, kind="ExternalInput")
    cc_d = nc.dram_tensor("cconst", [WAY, 2], F32, kind="ExternalInput")
    out_d = nc.dram_tensor("out", [2, NQ, WAY], F32, kind="ExternalOutput")
    dbg = {}
    if debug:
        dbg["qemb"] = nc.dram_tensor("dbg_qemb", [128, DC, RHAT], F32, kind="ExternalOutput")
        dbg["semb"] = nc.dram_tensor("dbg_semb", [128, DC, SALL], F32, kind="ExternalOutput")
        dbg["snorm"] = nc.dram_tensor("dbg_snorm", [1, SALL], F32, kind="ExternalOutput")
        dbg["qnorm"] = nc.dram_tensor("dbg_qnorm", [128, ITILES], F32, kind="ExternalOutput")
        dbg["rec"] = nc.dram_tensor("dbg_rec", [WAY, OW], F32, kind="ExternalOutput")
        dbg["mask"] = nc.dram_tensor("dbg_mask", [WAY, WAY - 1, S], F32, kind="ExternalOutput")
        dbg["dmax"] = nc.dram_tensor("dbg_dmax", [128, ITILES, WAY], F32, kind="ExternalOutput")
        dbg["nave2"] = nc.dram_tensor("dbg_nave2", [128, ITILES, WAY], F32, kind="ExternalOutput")
        dbg["pos"] = nc.dram_tensor("dbg_pos", [128, ITILES, WAY], F32, kind="ExternalOutput")
        dbg["ct"] = nc.dram_tensor("dbg_ct", [WAY, RHAT], F32, kind="ExternalOutput")

    with tile.TileContext(nc) as tc:
        _body(nc, tc, qd_d, sd_d, w_d, id_d, sel_d, padv_d, cc_d, out_d, dbg, sim1)
    nc.finalize()
    return nc


def _body(nc, tc, qd_d, sd_d, w_d, id_d, sel_d, padv_d, cc_d, out_d, dbg, sim1):
    AT = mybir.AluOpType
    ACTF = mybir.ActivationFunctionType
    X = mybir.AxisListType.X

    persist = tc.alloc_tile_pool(name="persist", bufs=1)
    dram = tc.alloc_tile_pool(name="dram", bufs=1, space="DRAM")

    # DRAM scratch
    p_dram = dram.tile([WAY, RROWS, PROW2], BF16, tag="p_scratch")
    posw_dram = dram.tile([WAY, 16, ITILES * 8], I16, tag="posw")
    snorm_dram = dram.tile([1, 3840], F32, tag="snormd")
    mask_dram = dram.tile([WAY, WAY - 1, RROWS], BF16, tag="maskd")
    cc_in = dram.tile([WAY, OW], F32, tag="cc_in")
    cc_out = dram.tile([WAY, OW], F32, tag="cc_out")

    # persistent SBUF
    q_embT = persist.tile([128, DC + 1, RHAT], F8, tag="q_embT")
    s_embT = persist.tile([128, DC + 1, SALL], F8, tag="s_embT")
    snh = persist.tile([1, SALL], BF16, tag="snh")          # -snorm/2
    qnorm = persist.tile([128, ITILES], F32, tag="qnorm")
    pnorm = persist.tile([128, WAY * CTILES], F32, tag="pnorm")
    m16a = persist.tile([128, ITILES, 16], F32, tag="m16a")
    dmax_all = persist.tile([128, ITILES, WAY], F32, tag="dmax_all")
    nave2 = persist.tile([128, ITILES, WAY], F32, tag="nave2")
    pos16 = persist.tile([128, ITILES, WAY], I16, tag="pos16")
    ident = persist.tile([128, 128], BF16, tag="ident")
    ones_bf = persist.tile([128, 1], BF16, tag="ones_bf")
    ones_f = persist.tile([128, 1], F32, tag="ones_f")
    ones_row = persist.tile([1, 128], BF16, tag="ones_row")
    padv = persist.tile([128, 1], F32, tag="padv")
    cconst = persist.tile([WAY, 2], F32, tag="cconst")
    sel_sb = persist.tile([128, ITILES, NQ], F32, tag="sel_sb")
    dmq = persist.tile([WAY, NQ], F32, tag="dmq")

    nc.vector.memset(ones_bf[:], 1.0)
    nc.vector.memset(ones_f[:], 1.0)
    nc.vector.memset(ones_row[:], 1.0)
    nc.sync.dma_start(padv[:], padv_d[:, :])
    nc.sync.dma_start(ident[:], id_d[:, :])
    nc.sync.dma_start(cconst[:], cc_d[:, :])
    nc.sync.dma_start(sel_sb[:], sel_d.rearrange("t p q -> p t q"))
    nc.vector.memset(q_embT[:, :, R:RHAT], 0.0)
    nc.vector.memset(q_embT[:, DC], 0.0)
    nc.vector.memset(s_embT[:, DC], 0.0)

    # ================= Phase 1: per-frame half products + tuple assembly ====
    with tc.tile_pool(name="emb", bufs=1) as emb, \
         tc.tile_pool(name="embsm", bufs=4) as embsm, \
         tc.tile_pool(name="embps", bufs=6, space="PSUM") as embps:
        wT = emb.tile([128, 2, 8, 2, DOUT], F8, tag="wT")
        nc.sync.dma_start(wT[:], w_d.rearrange("a b p c d -> p a b c d"))
        qd = emb.tile([128, 16, SEQ_LEN * NQ], F8, tag="qd")
        nc.sync.dma_start(qd[:], qd_d[:, :, :])
        sd = emb.tile([128, 16, SEQ_LEN * 80], F8, tag="sd")
        nc.sync.dma_start(sd[:], sd_d[:, :, :])
        Pq = emb.tile([128, DC, 2, SEQ_LEN * NQ], BF16, tag="Pq")
        Ps = emb.tile([128, DC, 2, SEQ_LEN * 80], BF16, tag="Ps")

        # s-side first: its embeddings gate snorm -> SS -> gathers
        for half in range(2):
            for dc in range(DC):
                for ch in range(2):
                    ps2 = embps.tile([128, SEQ_LEN * NQ], F32, tag="emb_ps")
                    for kc2 in range(8):
                        nc.tensor.matmul(
                            ps2[:], wT[:, half, kc2, :, dc * 128:(dc + 1) * 128],
                            sd[:, 2 * kc2:2 * kc2 + 2, ch * 400:(ch + 1) * 400],
                            start=(kc2 == 0), stop=(kc2 == 7), perf_mode=DR)
                    nc.vector.tensor_copy(
                        Ps[:, dc, half, ch * 400:(ch + 1) * 400], ps2[:])
        for t in range(T):
            f1, f2 = int(TUPLES[t][0]), int(TUPLES[t][1])
            pres = embsm.tile([128, DC, 80], BF16, tag="pres")
            nc.vector.tensor_tensor(
                pres[:], Ps[:, :, 0, f1 * 80:(f1 + 1) * 80],
                Ps[:, :, 1, f2 * 80:(f2 + 1) * 80], AT.add)
            dst = s_embT[:, :DC].rearrange("p d (u t) -> p d t u", t=T)[:, :, t]
            if t % 2 == 0:
                nc.scalar.activation(dst, pres[:], ACTF.Relu, scale=1.0 / 64.0)
            else:
                nc.vector.tensor_scalar(dst, pres[:], 0.0, 1.0 / 64.0,
                                        AT.max, op1=AT.mult)
        # q side
        for half in range(2):
            for dc in range(DC):
                ps = embps.tile([128, SEQ_LEN * NQ], F32, tag="emb_ps")
                for kc2 in range(8):
                    nc.tensor.matmul(
                        ps[:], wT[:, half, kc2, :, dc * 128:(dc + 1) * 128],
                        qd[:, 2 * kc2:2 * kc2 + 2, :],
                        start=(kc2 == 0), stop=(kc2 == 7), perf_mode=DR)
                nc.vector.tensor_copy(Pq[:, dc, half], ps[:])
        for t in range(T):
            f1, f2 = int(TUPLES[t][0]), int(TUPLES[t][1])
            preq = embsm.tile([128, DC, NQ], BF16, tag="preq")
            nc.vector.tensor_tensor(
                preq[:], Pq[:, :, 0, f1 * NQ:(f1 + 1) * NQ],
                Pq[:, :, 1, f2 * NQ:(f2 + 1) * NQ], AT.add)
            if t % 2 == 0:
                nc.scalar.activation(q_embT[:, :DC, t * NQ:(t + 1) * NQ], preq[:],
                                     ACTF.Relu, scale=1.0 / 64.0)
            else:
                nc.vector.tensor_scalar(q_embT[:, :DC, t * NQ:(t + 1) * NQ],
                                        preq[:], 0.0, 1.0 / 64.0,
                                        AT.max, op1=AT.mult)

    # dT allocated after the emb pool frees wT/Pq/Ps space
    dtp = tc.alloc_tile_pool(name="dtp", bufs=1)
    dT = [dtp.tile([128, CTILES, RHAT], F8, tag=f"dT{c}", name=f"dT{c}")
          for c in range(WAY)]

    if dbg:
        with tc.tile_pool(name="dbge", bufs=1) as dbge:
            t1 = dbge.tile([128, DC, RHAT], F32, tag="dbq")
            nc.vector.tensor_copy(t1[:], q_embT[:, :DC])
            nc.sync.dma_start(dbg["qemb"].ap(), t1[:])
            t2 = dbge.tile([128, DC, SALL], F32, tag="dbs")
            nc.vector.tensor_copy(t2[:], s_embT[:, :DC])
            nc.sync.dma_start(dbg["semb"].ap(), t2[:])

    # ================= Phase 2: norms =================
    with tc.tile_pool(name="nrm", bufs=2) as nrm, \
         tc.tile_pool(name="nrmps", bufs=2, space="PSUM") as nrmps:
        snrow = nrm.tile([1, SALL], F32, tag="snrow")
        for scn in range(8):
            ps = nrmps.tile([1, 450], F32, tag="sn_ps")
            for dc in range(DC):
                sq = nrm.tile([128, 450], BF16, tag="sn_sqb")
                nc.scalar.activation(sq[:], s_embT[:, dc, scn * 450:(scn + 1) * 450],
                                     ACTF.Square)
                nc.tensor.matmul(ps[:], ones_bf[:], sq[:],
                                 start=(dc == 0), stop=(dc == DC - 1))
            nc.scalar.activation(snrow[:, scn * 450:(scn + 1) * 450], ps[:], ACTF.Copy)
        nc.vector.tensor_scalar(snh[:], snrow[:], -0.5, None, AT.mult)
        nc.sync.dma_start(snorm_dram[:, :SALL], snrow[:])
        # pnorm[p, c*6+j] = snorm[720c + 128j + p]
        for c in range(WAY):
            nc.sync.dma_start(
                pnorm[:, c * CTILES:(c + 1) * CTILES],
                _ap(snorm_dram.tensor, snorm_dram.offset + c * S,
                    [(1, 128), (128, CTILES)]))
        for it in range(ITILES):
            ps = nrmps.tile([128, 1], F32, tag="qn_ps", name="qnps")
            sqa = nrm.tile([128, DC, 128], BF16, tag="qn_sqb")
            qb = nrm.tile([128, DC, 128], BF16, tag="qn_qb")
            nc.vector.tensor_copy(qb[:], q_embT[:, :DC, it * 128:(it + 1) * 128])
            nc.vector.tensor_tensor(sqa[:], qb[:], qb[:], AT.mult)
            for dc in range(DC):
                nc.tensor.matmul(ps[:], sqa[:, dc], ones_bf[:],
                                 start=(dc == 0), stop=(dc == DC - 1))
            nc.vector.tensor_copy(qnorm[:, it:it + 1], ps[:])
        if dbg:
            nc.sync.dma_start(dbg["snorm"].ap(), snrow[:])
            nc.sync.dma_start(dbg["qnorm"].ap(), qnorm[:])

    # ====== Phases 3+4: per class, SS slab -> D tiles -> gather/rec ======
    with tc.tile_pool(name="ssst", bufs=1) as ssst, \
         tc.tile_pool(name="ssps", bufs=2, space="PSUM") as ssps, \
         tc.tile_pool(name="dph", bufs=3) as dph, \
         tc.tile_pool(name="dsm", bufs=6) as dsm, \
         tc.tile_pool(name="dps", bufs=2, space="PSUM") as dps, \
         tc.tile_pool(name="tps", bufs=2, space="PSUM") as tps, \
         tc.tile_pool(name="cdp", bufs=2) as cdp, \
         tc.tile_pool(name="accp", bufs=2) as accp, \
         tc.tile_pool(name="rrp", bufs=2) as rrp, \
         tc.tile_pool(name="cmpp", bufs=2) as cmpp, \
         tc.tile_pool(name="cdg", bufs=3) as cdg:
        for c in range(WAY):
            # ---- SS slab for class c ----
            staging = ssst.tile([128, CTILES, PROW2], BF16, tag="ss_stage")
            chunks = _chunks_for_class(c)
            for j in range(CTILES):
                p0 = S * c + 128 * j
                pw = min(128, S - 128 * j)
                for (dst0, pieces) in chunks:
                    ps = ssps.tile([128, 960], F32, tag="ss_ps")
                    for (src0, doff, w) in pieces:
                        for dc2 in range(NDR):
                            nc.tensor.matmul(
                                ps[:pw, doff:doff + w],
                                s_embT[:, 2 * dc2:2 * dc2 + 2, p0:p0 + pw],
                                s_embT[:, 2 * dc2:2 * dc2 + 2, src0:src0 + w],
                                start=(dc2 == 0), stop=False, perf_mode=DR)
                        nc.tensor.matmul(
                            ps[:pw, doff:doff + w], ones_row[:, :pw],
                            snh[:, src0:src0 + w], start=False, stop=True)
                    nc.scalar.activation(staging[:pw, j, dst0:dst0 + 960],
                                         ps[:pw], ACTF.Copy, scale=-2.0)
            # row-norm column (col 2880) for the gathered threshold
            nc.vector.tensor_copy(staging[:, :, OW:OW + 1],
                                  pnorm[:, c * CTILES:(c + 1) * CTILES, None])
            # write region, skipping the undefined pad rows of the last tile
            nc.sync.dma_start(
                _ap(p_dram.tensor, p_dram.offset + c * RROWS * PROW2,
                    [(PROW2, 128), (128 * PROW2, CTILES - 1), (1, PROW2)]),
                staging[:, :CTILES - 1])
            nc.sync.dma_start(
                _ap(p_dram.tensor,
                    p_dram.offset + (c * RROWS + (CTILES - 1) * 128) * PROW2,
                    [(PROW2, 80), (1, PROW2)]),
                staging[:80, CTILES - 1])
            # ---- D tiles ----
            for it in range(ITILES):
                d_bf = dph.tile([128, RROWS], BF16, tag="d_bf")
                for sc in range(2):
                    ps = dps.tile([128, 360], F32, tag="d_ps", name="dps")
                    s0 = c * S + sc * 360
                    for dc2 in range(NDR):
                        nc.tensor.matmul(
                            ps[:], q_embT[:, 2 * dc2:2 * dc2 + 2, it * 128:(it + 1) * 128],
                            s_embT[:, 2 * dc2:2 * dc2 + 2, s0:s0 + 360],
                            start=(dc2 == 0), stop=False, perf_mode=DR)
                    nc.tensor.matmul(ps[:], ones_row[:], snh[:, s0:s0 + 360],
                                     start=False, stop=True)
                    nc.scalar.activation(d_bf[:, sc * 360:(sc + 1) * 360], ps[:],
                                         ACTF.Sqrt, bias=qnorm[:, it:it + 1],
                                         scale=-2.0)
                nc.vector.memset(d_bf[:, S:RROWS], 0.0)
                # reductions
                nc.vector.tensor_reduce(
                    m16a[:, it], d_bf[:, :S].rearrange("p (a b) -> p b a", b=16),
                    X, AT.max)
                nc.vector.tensor_reduce(dmax_all[:, it, c:c + 1], m16a[:, it],
                                        X, AT.max)
                mx8 = dsm.tile([128, 8], F32, tag="mx8")
                ix8 = dsm.tile([128, 8], U32, tag="ix8")
                nc.vector.tensor_copy(
                    mx8[:], dmax_all[:, it, c:c + 1].to_broadcast((128, 8)))
                nc.vector.max_index(ix8[:], mx8[:], d_bf[:, :S])
                posf = dsm.tile([128, 1], F32, tag="posf")
                nc.vector.tensor_scalar(posf[:], ix8[:, 0:1], 0.0, None, AT.add)
                nc.vector.tensor_copy(pos16[:, it, c:c + 1], posf[:])
                # transpose into dT (fp8)
                psT = tps.tile([128, CTILES * 128], BF16, tag="psT")
                for j in range(CTILES):
                    nc.tensor.matmul(psT[:, j * 128:(j + 1) * 128],
                                     d_bf[:, j * 128:(j + 1) * 128], ident[:],
                                     start=True, stop=True, is_transpose=True)
                if it % 2 == 0:
                    nc.vector.tensor_copy(
                        dT[c][:, :, it * 128:(it + 1) * 128],
                        psT[:].rearrange("p (j q) -> p j q", j=CTILES))
                else:
                    nc.scalar.activation(
                        dT[c][:, :, it * 128:(it + 1) * 128],
                        psT[:].rearrange("p (j q) -> p j q", j=CTILES), ACTF.Copy)
            nc.vector.memset(dT[c][:, :, R:RHAT], 0.0)
            # batched per-class stats: nave2 = -(asum/16)^2 ; dmax
            asum = dsm.tile([128, ITILES], F32, tag="asum")
            nc.vector.tensor_reduce(asum[:], m16a[:], X, AT.add)
            nc.vector.tensor_scalar(asum[:, ITILES - 1:ITILES],
                                    asum[:, ITILES - 1:ITILES], padv[:], None, AT.add)
            nc.vector.tensor_tensor(asum[:], asum[:], asum[:], AT.mult)
            nc.vector.tensor_scalar(nave2[:, :, c], asum[:], -1.0 / 256.0, None,
                                    AT.mult)
            if c == WAY - 1:
                psD = ssps.tile([WAY, NQ], F32, tag="ss_ps", name="dmps")
                for it in range(ITILES):
                    nc.tensor.matmul(psD[:], dmax_all[:, it, :], sel_sb[:, it],
                                     start=(it == 0), stop=(it == ITILES - 1))
                nc.scalar.activation(dmq[:], psD[:], ACTF.Copy, scale=1.0 / T)

            # ---- gather + rec ----
            nc.sync.dma_start(
                _ap(posw_dram.tensor, posw_dram.offset + c * 16 * ITILES * 8,
                    [(1, 8), (ITILES * 8, 16), (8, ITILES)]),
                pos16[:, :, c])
            idxs = cdp.tile([128, ITILES * 8], I16, tag="idxs")
            nc.sync.dma_start(
                idxs[:],
                _ap(posw_dram.tensor, posw_dram.offset + c * 16 * ITILES * 8,
                    [(0, 8), (ITILES * 8, 16), (1, ITILES * 8)]))
            acc = accp.tile([128, OW], BF16, tag="accb")
            nc.vector.memset(acc[:, WA:], 0.0)
            nbneg = cdp.tile([128, ITILES], F32, tag="nbneg")
            region = _ap(p_dram.tensor, p_dram.offset + c * RROWS * PROW2,
                         [(PROW2, RROWS), (1, PROW2)])
            # software-pipelined: gather(g)/nbias(g) run one stage ahead of
            # sign(g)/add(g) so the ACT sign stream never waits on DVE.
            nball = cdp.tile([128, ITILES], F32, tag="nball")
            cds = {}

            def _compare(g):
                cd = cds.pop(g)
                cmp = cmpp.tile([128, WA], BF16, tag="cmp")
                nc.scalar.activation(cmp[:], cd[:, 0, :WA], ACTF.Sign,
                                     bias=nball[:, g:g + 1])
                nc.vector.scalar_tensor_tensor(
                    acc[:, WA:], cd[:, 0, WA:OW], nbneg[:, g:g + 1], acc[:, WA:],
                    op0=AT.is_gt, op1=AT.add)
                if g == 0:
                    nc.vector.tensor_copy(acc[:, :WA], cmp[:])
                else:
                    nc.vector.tensor_tensor(acc[:, :WA], acc[:, :WA], cmp[:],
                                            AT.add)

            for g in range(ITILES):
                cd = cdg.tile([128, 1, PROW2], BF16, tag="cd")
                nc.gpsimd.dma_gather(
                    cd[:], region, idxs[:, g * 8:(g + 1) * 8],
                    128, 128, PROW2, queue_num=g % 4)
                # bias = n_i - ave^2 : sign(cd_raw + bias) = sign(d^2 - ave^2)
                nc.vector.tensor_tensor(nball[:, g:g + 1], cd[:, 0, OW:OW + 1],
                                        nave2[:, g, c:c + 1], AT.add)
                nc.vector.tensor_scalar(nbneg[:, g:g + 1], nball[:, g:g + 1],
                                        -1.0, None, AT.mult)
                cds[g] = cd
                if g >= 1:
                    _compare(g - 1)
            _compare(ITILES - 1)
            for k in range(6):
                ps = ssps.tile([1, 480], F32, tag="ss_ps", name="recps")
                nc.tensor.matmul(ps[:], ones_bf[:],
                                 acc[:, k * 480:(k + 1) * 480],
                                 start=True, stop=True)
                rc = rrp.tile([1, 480], F32, tag="recc")
                nc.scalar.activation(rc[:], ps[:], ACTF.Copy)
                nc.sync.dma_start(cc_in[c:c + 1, k * 480:(k + 1) * 480], rc[:])

    # ================= AllReduce rec =================
    if sim1:
        nc.sync.dma_start(cc_out[:, :], cc_in[:, :])
    else:
        nc.gpsimd.collective_compute(
            "AllReduce", mybir.AluOpType.add,
            replica_groups=[list(range(NCORES))],
            ins=[cc_in[:, :].opt()], outs=[cc_out[:, :].opt()])

    # ================= Phase 5: thr/mask =================
    with tc.tile_pool(name="thrp", bufs=2) as thrp, \
         tc.tile_pool(name="thrbig", bufs=1) as thrbig:
        recf = thrbig.tile([WAY, OW], F32, tag="recf")
        nc.sync.dma_start(recf[:], cc_out[:, :])
        # sign region holds sum(+-1) over 8*1920 rows; counts = (x+15360)/2
        nc.vector.tensor_scalar(recf[:, :WA], recf[:, :WA],
                                float(NCORES * RHAT), 0.5, AT.add, op1=AT.mult)
        rec = recf[:].rearrange("c (k s) -> c k s", k=WAY - 1)
        if dbg:
            with tc.tile_pool(name="dbgr", bufs=1) as dbgr:
                rg = dbgr.tile([WAY, OW], F32, tag="rg")
                nc.vector.tensor_copy(rg[:], recf[:])
                nc.sync.dma_start(dbg["rec"].ap(), rg[:])
        rsum = thrp.tile([WAY, WAY - 1], F32, tag="rsum")
        nc.vector.tensor_reduce(rsum[:], rec[:], X, AT.add)
        gt0 = thrbig.tile([WAY, WAY - 1, S], F32, tag="gt0")
        nc.vector.tensor_scalar(gt0[:], rec[:], 0.5, None, AT.is_gt)
        nz = thrp.tile([WAY, WAY - 1], F32, tag="nz")
        nc.vector.tensor_reduce(nz[:], gt0[:], X, AT.add)
        nc.vector.tensor_scalar(nz[:], nz[:], 1.0, None, AT.max)
        thr = thrp.tile([WAY, WAY - 1], F32, tag="thr")
        nc.vector.reciprocal(thr[:], nz[:])
        nc.vector.tensor_tensor(thr[:], thr[:], rsum[:], AT.mult)
        mask_slots = thrbig.tile([WAY, WAY - 1, RROWS], BF16, tag="mask_slots")
        nc.vector.memset(mask_slots[:, :, S:RROWS], 0.0)
        nc.vector.tensor_tensor(
            mask_slots[:, :, :S], rec[:],
            thr[:, :, None].to_broadcast((WAY, WAY - 1, S)), AT.is_lt)
        if dbg:
            with tc.tile_pool(name="dbgm", bufs=1) as dbgm:
                mg = dbgm.tile([WAY, WAY - 1, S], F32, tag="mg")
                nc.vector.tensor_copy(mg[:], mask_slots[:, :, :S])
                nc.sync.dma_start(dbg["mask"].ap(), mg[:])
        msum = thrp.tile([WAY, 1], F32, tag="msum")
        nc.vector.tensor_reduce(
            msum[:], mask_slots[:].rearrange("c k s -> c (k s)"), X, AT.add)
        nc.vector.tensor_scalar(msum[:], msum[:], 1.0, None, AT.max)
        scv = thrp.tile([WAY, 1], F32, tag="scv")
        nc.vector.reciprocal(scv[:], msum[:])
        nc.vector.tensor_scalar(scv[:], scv[:], 1.0 / (4.0 * T), None, AT.mult)
        nc.sync.dma_start(mask_dram[:, :, :], mask_slots[:])

        # ============= Phase 6: contrast sums + finals =============
        with tc.tile_pool(name="p6", bufs=1) as p6, \
             tc.tile_pool(name="p6ps", bufs=1, space="PSUM") as p6ps:
            maskT = p6.tile([128, WAY * CTILES, WAY], BF16, tag="maskT")
            nc.vector.memset(maskT[:], 0.0)
            for c in range(WAY):
                if c > 0:
                    nc.sync.dma_start(
                        maskT[:, 0:c * CTILES, c],
                        _ap(mask_dram.tensor,
                            mask_dram.offset + c * (WAY - 1) * RROWS,
                            [(1, 128), (128, c * CTILES)]))
                if c < WAY - 1:
                    nc.sync.dma_start(
                        maskT[:, (c + 1) * CTILES:WAY * CTILES, c],
                        _ap(mask_dram.tensor,
                            mask_dram.offset + (c * (WAY - 1) + c) * RROWS,
                            [(1, 128), (128, (WAY - 1 - c) * CTILES)]))
            psC = [p6ps.tile([WAY, 480], F32, tag=f"ct_ps{qc}", name=f"ctps{qc}")
                   for qc in range(4)]
            for cr in range(WAY):
                for j in range(CTILES):
                    for qc in range(4):
                        nc.tensor.matmul(
                            psC[qc][:], maskT[:, cr * CTILES + j, :],
                            dT[cr][:, j, qc * 480:(qc + 1) * 480],
                            start=(cr == 0 and j == 0),
                            stop=(cr == WAY - 1 and j == CTILES - 1))
            ctrows = p6.tile([WAY, RHAT], F32, tag="ctrows")
            for qc in range(4):
                nc.scalar.activation(ctrows[:, qc * 480:(qc + 1) * 480], psC[qc][:],
                                     ACTF.Copy)
            if dbg:
                nc.sync.dma_start(dbg["ct"].ap(), ctrows[:])
            ctq = p6.tile([WAY, NQ], F32, tag="ctq")
            nc.vector.tensor_reduce(
                ctq[:], ctrows[:].rearrange("c (s q) -> c q s", q=NQ), X, AT.add)
            nc.vector.tensor_scalar(ctq[:], ctq[:], scv[:], None, AT.mult)

            if dbg:
                nc.sync.dma_start(dbg["dmax"].ap(), dmax_all[:])
                nc.sync.dma_start(dbg["nave2"].ap(), nave2[:])
                with tc.tile_pool(name="dbgp", bufs=1) as dbgp:
                    pf = dbgp.tile([128, ITILES, WAY], F32, tag="pf")
                    nc.vector.tensor_copy(pf[:], pos16[:])
                    nc.sync.dma_start(dbg["pos"].ap(), pf[:])

            ssum = p6.tile([WAY, NQ], F32, tag="ssum")
            nc.vector.tensor_tensor(ssum[:], dmq[:], ctq[:], AT.add)
            rcp = p6.tile([WAY, NQ], F32, tag="rcp")
            nc.vector.reciprocal(rcp[:], ssum[:])
            lg = p6.tile([WAY, NQ], F32, tag="lg")
            nc.vector.tensor_tensor(lg[:], dmq[:], rcp[:], AT.mult)
            nc.sync.dma_start(_ap(out_d, 0, [(1, WAY), (WAY, NQ)]), dmq[:])
            nc.sync.dma_start(_ap(out_d, NQ * WAY, [(1, WAY), (WAY, NQ)]), lg[:])

    dtp.release()
    persist.release()
    dram.release()


# ---------------- host side ----------------

def _sel_host():
    sel = np.zeros((ITILES, 128, NQ), np.float32)
    for i in range(R):
        sel[i // 128, i % 128, i % NQ] = 1.0
    return sel


def _prep_inputs(support_set, queries, support_labels, W, b):
    import ml_dtypes
    f8 = ml_dtypes.float8_e4m3fn
    support_set = np.asarray(support_set, dtype=np.float32)
    queries = np.asarray(queries, dtype=np.float32)
    labels = np.asarray(support_labels).astype(np.int64)
    W = np.asarray(W, dtype=np.float32)
    b = np.asarray(b, dtype=np.float32)
    assert not np.any(b), "kernel built without bias support (reference b==0)"
    order = np.argsort(labels, kind="stable")
    support_sorted = support_set[order]

    # wT [2, 8, 128, 2, 1152]: wT[half, kc2, p, h2, d] =
    #   64*W[d, half*2048 + kc2*256 + h2*128 + p]
    w8 = (W * 64.0).astype(f8)                     # [1152, 4096]
    wT = np.ascontiguousarray(
        w8.reshape(DOUT, 2, 8, 2, 128).transpose(1, 2, 4, 3, 0))

    s8 = support_sorted.astype(f8)                 # [80, 10, 2048]
    sd = np.ascontiguousarray(
        s8.reshape(80, SEQ_LEN, 16, 128).transpose(3, 2, 1, 0)
          .reshape(128, 16, SEQ_LEN * 80))
    q8 = queries.astype(f8)                        # [320, 10, 2048]
    sel = _sel_host()
    padv = np.zeros((128, 1), np.float32)
    padv[8:] = 1.0e15
    ident = np.eye(128).astype(ml_dtypes.bfloat16)
    cconst = np.zeros((WAY, 2), np.float32)
    for c in range(WAY):
        if c in SIGN_CLASSES:
            cconst[c] = (NCORES * RHAT, 0.5)
        else:
            cconst[c] = (0.0, 1.0)
    out = []
    for k in range(NCORES):
        qk = q8[k * NQ:(k + 1) * NQ]               # [40, 10, 2048]
        qd = np.ascontiguousarray(
            qk.reshape(NQ, SEQ_LEN, 16, 128).transpose(3, 2, 1, 0)
              .reshape(128, 16, SEQ_LEN * NQ))
        out.append({
            "qd": qd,
            "sd": sd,
            "wT": wT,
            "ident": ident,
            "sel": sel,
            "padv": padv,
            "cconst": cconst,
        })
    return out


def kernel(**inputs):
    per_core = _prep_inputs(**inputs)
    if "nc" not in _CACHE:
        _CACHE["nc"] = build(debug=bool(os.environ.get("BIMACL_DEBUG")))
    nc = _CACHE["nc"]
    res = run_bass_kernel_spmd(nc, per_core, core_ids=list(range(NCORES)))
    _CACHE["last_results"] = res
    full = np.concatenate([res.results[k]["out"] for k in range(NCORES)], axis=1)
    return np.ascontiguousarray(full.astype(np.float32))
